# revision 1
# baseline (speedup 1.0000x reference)
"""Sparse attention (ProbSparse-style) Trainium2 Bass kernel.

Problem (per batch element b, data-parallel over 8 NeuronCores):
  Q = x @ Wq.T ; K = x @ Wk.T ; V = x @ Wv.T            [L=2048, D=512]
  QK_sample[l,s] = Q[l] . K[index_sample[l,s]]           [L, 40]
  M[l] = max_s QK_sample - sum_s QK_sample / L
  sel = top40(M)  (as a set; the reference scatter makes order irrelevant)
  scores = Q[sel] @ K.T / sqrt(D); attn = softmax(scores)
  ctx = broadcast(mean(V)); ctx[sel] = attn @ V

Numerics strategy (top-40 boundary gaps are as small as 0.02 in M):
  - K and V are computed with a 3-term bf16x2 split matmul
    (xh*wh + xl*wh + xh*wl, host-split halves) -> ~1e-5 absolute error,
    fp32-class, at full bf16 PE rate.
  - Approx M for ALL rows uses bf16 Q and bf16 K (error sigma ~0.2),
    extracted from per-chunk S = Q K^T PSUM blocks with fused
    tensor_tensor_reduce against a shipped u8 sample mask
    (multiply-mask max is safe: sampled max > 0 w.p. 1-2^-40;
    dup-count correction is deferred to the exact stage).
  - Candidates = { M_approx >= approx-top40 - DELTA }, DELTA=1.5 covers
    ~8 sigma; measured rank-40 to rank-64 M gap is 2.5-4.8 so the
    candidate count stays well under the 128-slot budget.
  - Exact stage on <= 128 candidate rows: gather x rows from DRAM
    (indirect DMA), exact fp32 Q_cand, exact S_cand vs the fp32-class K,
    TTR with gathered u8 mask+count rows -> exact M_cand -> exact top-40
    threshold -> softmax over S_cand -> upd = attn @ V -> indirect
    scatter of the 40 selected rows into ctx (bounds_check skips the
    rest).

kernel(**inputs) accepts the FULL inputs and returns the FULL
[8, 2048, 512] f32 output; batch is sharded over 8 cores.
"""

import math

import numpy as np
import ml_dtypes

import concourse.bacc as bacc
import concourse.bass as bass
import concourse.mybir as mybir
import concourse.tile as tile
from concourse.bass_utils import run_bass_kernel_spmd
from concourse.masks import make_identity

P = 128
L = 2048
D = 512
B = 8
NL = L // P        # 16 query chunks
ND = D // P        # 4 feature chunks
NJ = L // 512      # 4 key blocks of 512
NT = 40
SCALE = 1.0 / math.sqrt(D)
DELTA = 1.5        # candidate band below approx T40
NEG = -3.0e38
SKIP_IDX = 99999.0  # scatter index sentinel (> bounds_check -> row skipped)

f32 = mybir.dt.float32
bf16 = mybir.dt.bfloat16
u8 = mybir.dt.uint8
i32 = mybir.dt.int32
u32 = mybir.dt.uint32
AX = mybir.AxisListType
OP = mybir.AluOpType
ACTF = mybir.ActivationFunctionType


def build():
    nc = bacc.Bacc("TRN2", target_bir_lowering=False)

    x_d = nc.dram_tensor("x_nat", [L, D], f32, kind="ExternalInput")
    xth_d = nc.dram_tensor("xTh", [D, L], bf16, kind="ExternalInput")
    xtl_d = nc.dram_tensor("xTl", [D, L], bf16, kind="ExternalInput")
    xmh_d = nc.dram_tensor("xmeanTh", [D, 1], bf16, kind="ExternalInput")
    xml_d = nc.dram_tensor("xmeanTl", [D, 1], bf16, kind="ExternalInput")
    wqh_d = nc.dram_tensor("wqTh", [D, D], bf16, kind="ExternalInput")
    wkh_d = nc.dram_tensor("wkTh", [D, D], bf16, kind="ExternalInput")
    wkl_d = nc.dram_tensor("wkTl", [D, D], bf16, kind="ExternalInput")
    wvh_d = nc.dram_tensor("wvTh", [D, D], bf16, kind="ExternalInput")
    wvl_d = nc.dram_tensor("wvTl", [D, D], bf16, kind="ExternalInput")
    wq_d = nc.dram_tensor("wqT", [D, D], f32, kind="ExternalInput")
    perm_d = nc.dram_tensor("perm16", [16, 8 * P], f32, kind="ExternalInput")
    mask_d = nc.dram_tensor("mask01", [L, L], u8, kind="ExternalInput")
    cnt_d = nc.dram_tensor("countf", [L, L], u8, kind="ExternalInput")
    ctx_d = nc.dram_tensor("ctx", [L, D], f32, kind="ExternalOutput")

    with tile.TileContext(nc) as tc:
        with (
            tc.tile_pool(name="const", bufs=1) as cst,
            tc.tile_pool(name="proj", bufs=1) as proj,       # KT/KTb/QTb/V resident
            tc.tile_pool(name="mstuff", bufs=1) as mst,      # M / topk / sel smalls
            tc.tile_pool(name="mstream", bufs=3) as mstr,    # mask chunks
            tc.tile_pool(name="scr", bufs=3) as scr,         # TTR scratch
            tc.tile_pool(name="acc", bufs=2) as accp,        # per-chunk accums
            tc.tile_pool(name="cand", bufs=1) as cnd,        # exact-stage tiles
            tc.tile_pool(name="ps", bufs=3, space="PSUM") as ps,
            tc.tile_pool(name="ps_s", bufs=4, space="PSUM") as ps_s,  # S_cand (held)
            tc.tile_pool(name="dram", bufs=1, space="DRAM") as drp,
        ):
            # ---------------- constants ----------------
            ident = cst.tile([P, P], f32, tag="ident")
            make_identity(nc, ident[:])
            ones_r1 = cst.tile([1, P], f32, tag="ones_r1")
            nc.vector.memset(ones_r1[:], 1.0)
            negone = cst.tile([P, 1], f32, tag="negone")
            nc.vector.memset(negone[:], -1.0)
            negbig = cst.tile([P, 1], f32, tag="negbig")
            nc.vector.memset(negbig[:], NEG)
            big9 = cst.tile([P, 1], f32, tag="big9")
            nc.vector.memset(big9[:], SKIP_IDX)
            perm16 = cst.tile([16, 8 * P], f32, tag="perm16")
            nc.sync.dma_start(perm16[:], perm_d[:])
            qidx_i = cst.tile([P, 16], i32, tag="qidx_i")     # value p + 128*c
            nc.gpsimd.iota(qidx_i[:], pattern=[[P, 16]], base=0, channel_multiplier=1)
            qidx_f = cst.tile([P, 16], f32, tag="qidx_f")
            nc.vector.tensor_copy(qidx_f[:], qidx_i[:])
            # preload the (large) attn GPSIMD library now so kth_largest does
            # not pay the ucode reload inside the serial tail
            from concourse import library_config
            nc.gpsimd.load_library(library_config.attn)

            # resident projection outputs
            KT = [proj.tile([P, L], f32, tag=f"KT{ic}", name=f"KT{ic}") for ic in range(ND)]
            KTb = [proj.tile([P, L], bf16, tag=f"KTb{ic}", name=f"KTb{ic}") for ic in range(ND)]
            QTb = [proj.tile([P, L], bf16, tag=f"QTb{ic}", name=f"QTb{ic}") for ic in range(ND)]
            Vb = [proj.tile([P, D], bf16, tag=f"Vb{jc}", name=f"Vb{jc}") for jc in range(NL)]

            with tc.tile_pool(name="xw", bufs=1) as xw:
                # ---------------- phase 0: loads ----------------
                xTh = [xw.tile([P, L], bf16, tag=f"xTh{dc}", name=f"xTh{dc}") for dc in range(ND)]
                xTl = [xw.tile([P, L], bf16, tag=f"xTl{dc}", name=f"xTl{dc}") for dc in range(ND)]
                wqh = [xw.tile([P, D], bf16, tag=f"wqh{dc}", name=f"wqh{dc}") for dc in range(ND)]
                wkh = [xw.tile([P, D], bf16, tag=f"wkh{dc}", name=f"wkh{dc}") for dc in range(ND)]
                wkl = [xw.tile([P, D], bf16, tag=f"wkl{dc}", name=f"wkl{dc}") for dc in range(ND)]
                wvh = [xw.tile([P, D], bf16, tag=f"wvh{dc}", name=f"wvh{dc}") for dc in range(ND)]
                wvl = [xw.tile([P, D], bf16, tag=f"wvl{dc}", name=f"wvl{dc}") for dc in range(ND)]
                wqT = [xw.tile([P, D], f32, tag=f"wqT{dc}", name=f"wqT{dc}") for dc in range(ND)]
                xmh = [xw.tile([P, 1], bf16, tag=f"xmh{dc}", name=f"xmh{dc}") for dc in range(ND)]
                xml = [xw.tile([P, 1], bf16, tag=f"xml{dc}", name=f"xml{dc}") for dc in range(ND)]
                for dc in range(ND):
                    sl = slice(dc * P, (dc + 1) * P)
                    nc.sync.dma_start(xTh[dc][:], xth_d[sl, :])
                    nc.sync.dma_start(xTl[dc][:], xtl_d[sl, :])
                    nc.sync.dma_start(wqh[dc][:], wqh_d[sl, :])
                    nc.sync.dma_start(wkh[dc][:], wkh_d[sl, :])
                    nc.sync.dma_start(wkl[dc][:], wkl_d[sl, :])
                    nc.sync.dma_start(wvh[dc][:], wvh_d[sl, :])
                    nc.sync.dma_start(wvl[dc][:], wvl_d[sl, :])
                    nc.sync.dma_start(wqT[dc][:], wq_d[sl, :])
                    nc.sync.dma_start(xmh[dc][:], xmh_d[sl, :])
                    nc.sync.dma_start(xml[dc][:], xml_d[sl, :])

                # ---------------- phase 1: projections ----------------
                # K: 3-term bf16x2 (fp32-class), into KT f32 + KTb bf16
                for ic in range(ND):
                    isl = slice(ic * P, (ic + 1) * P)
                    for jb in range(NJ):
                        jsl = slice(jb * 512, (jb + 1) * 512)
                        pk = ps.tile([P, 512], f32, tag="blk")
                        n = 0
                        for dc in range(ND):
                            for lh, rh in (
                                (wkh[dc][:, isl], xTh[dc][:, jsl]),
                                (wkh[dc][:, isl], xTl[dc][:, jsl]),
                                (wkl[dc][:, isl], xTh[dc][:, jsl]),
                            ):
                                nc.tensor.matmul(
                                    pk[:], lh, rh,
                                    start=(n == 0), stop=(n == 3 * ND - 1),
                                )
                                n += 1
                        nc.scalar.copy(KT[ic][:, jsl], pk[:])
                        nc.vector.tensor_copy(KTb[ic][:, jsl], pk[:])

                # Q approx: single bf16 term
                for ic in range(ND):
                    isl = slice(ic * P, (ic + 1) * P)
                    for jb in range(NJ):
                        jsl = slice(jb * 512, (jb + 1) * 512)
                        pq = ps.tile([P, 512], f32, tag="blk")
                        for dc in range(ND):
                            nc.tensor.matmul(
                                pq[:], wqh[dc][:, isl], xTh[dc][:, jsl],
                                start=(dc == 0), stop=(dc == ND - 1),
                            )
                        nc.scalar.copy(QTb[ic][:, jsl], pq[:])

                # ---------------- phase 2: approx M (bf16 S) ----------------
                # per (lc, jb) block: one STT (masked product -> bf16 scratch,
                # fused sum accum) + one reduce_max. Combines batched at end.
                M_all = mst.tile([P, 16], f32, tag="M_all")
                amax_all = mst.tile([P, NL * NJ], f32, tag="amax_all")
                asum_all = mst.tile([P, NL * NJ], f32, tag="asum_all")
                for lc in range(NL):
                    lsl = slice(lc * P, (lc + 1) * P)
                    mk = mstr.tile([P, L], u8, tag="mk")
                    nc.sync.dma_start(mk[:], mask_d[lsl, :])
                    # V chunk lc interleaved here (single bf16 term: the upd
                    # matmul consumes bf16 anyway) to keep PE warm while the
                    # vector engine drains the S-extraction
                    pv = ps.tile([P, 512], f32, tag="blk")
                    for dc in range(ND):
                        nc.tensor.matmul(
                            pv[:], xTh[dc][:, lsl], wvh[dc][:],
                            start=(dc == 0), stop=(dc == ND - 1),
                        )
                    nc.scalar.copy(Vb[lc][:], pv[:])
                    for jb in range(NJ):
                        jsl = slice(jb * 512, (jb + 1) * 512)
                        k = lc * NJ + jb
                        pss = ps_s.tile([P, 512], f32, tag="psSc", name="pssa")
                        for ic in range(ND):
                            nc.tensor.matmul(
                                pss[:], QTb[ic][:, lsl], KTb[ic][:, jsl],
                                start=(ic == 0), stop=(ic == ND - 1),
                            )
                        s1 = scr.tile([P, 512], bf16, tag="scrt")
                        nc.vector.scalar_tensor_tensor(
                            out=s1[:], in0=pss[:], scalar=1.0, in1=mk[:, jsl],
                            op0=OP.mult, op1=OP.mult,
                            accum_out=asum_all[:, k : k + 1],
                        )
                        nc.vector.reduce_max(
                            amax_all[:, k : k + 1], s1[:], axis=AX.X
                        )
                t1 = accp.tile([P, 16], f32, tag="t1")
                t2 = accp.tile([P, 16], f32, tag="t2")
                nc.vector.reduce_max(
                    t1[:], amax_all[:].rearrange("p (c j) -> p c j", j=NJ),
                    axis=AX.X,
                )
                nc.vector.reduce_sum(
                    t2[:], asum_all[:].rearrange("p (c j) -> p c j", j=NJ),
                    axis=AX.X,
                )
                nc.vector.tensor_scalar_mul(t2[:], t2[:], -1.0 / L)
                nc.vector.tensor_tensor(
                    out=M_all[:], in0=t1[:], in1=t2[:], op=OP.add
                )

                # Vmean = xmean.T @ Wv.T via bf16x2 3-term, broadcast, ctx init
                pvm = ps.tile([1, 512], f32, tag="blk")
                n = 0
                for dc in range(ND):
                    for lh, rh in (
                        (xmh[dc][:], wvh[dc][:]),
                        (xml[dc][:], wvh[dc][:]),
                        (xmh[dc][:], wvl[dc][:]),
                    ):
                        nc.tensor.matmul(
                            pvm[:1, :], lh, rh,
                            start=(n == 0), stop=(n == 3 * ND - 1),
                        )
                        n += 1
                vmean = mst.tile([1, 512], f32, tag="vmean")
                nc.scalar.copy(vmean[:], pvm[:1, :])
                pvb = ps.tile([P, 512], f32, tag="blk")
                nc.tensor.matmul(pvb[:], ones_r1[:], vmean[:], start=True, stop=True)
                vmean_bc = mst.tile([P, 512], f32, tag="vmean_bc")
                nc.vector.tensor_copy(vmean_bc[:], pvb[:])
                for jc in range(NL):
                    nc.sync.dma_start(ctx_d[jc * P : (jc + 1) * P, :], vmean_bc[:])

                # ---------------- phase 3: approx top-40 -> candidates ------
                # approx threshold ~= 40th largest of M (rank +-1 is fine:
                # the DELTA band absorbs it) via one GPSIMD kth_largest
                kout = mst.tile([1, 2], f32, tag="kout")
                nc.gpsimd.kth_largest(
                    out_ap=kout[:], in_ap=M_all[:], n_per_lane=16, k=64,
                    quantile=1.0 - 38.5 / 2047.0,
                )
                ptb = ps.tile([P, 1], f32, tag="blk")
                nc.tensor.matmul(
                    ptb[:P, :1], ones_r1[:], kout[:, 1:2], start=True, stop=True
                )
                tbc = mst.tile([P, 1], f32, tag="tbc")
                nc.vector.tensor_copy(tbc[:], ptb[:P, :1])

                # selmask = (M - T40) >= -DELTA, one fused op
                selmask = mst.tile([P, 16], u8, tag="selmask")
                nc.vector.tensor_scalar(
                    selmask[:], M_all[:], tbc[:], -DELTA,
                    op0=OP.subtract, op1=OP.is_ge,
                )
                midx = mst.tile([P, 16], f32, tag="midx")
                nc.vector.tensor_copy(midx[:], negone[:].to_broadcast([P, 16]))
                nc.vector.copy_predicated(midx[:], selmask[:], qidx_f[:])

                pwr = ps.tile([16, P], f32, tag="blk", name="pwr")
                nc.tensor.transpose(pwr[:16, :P], midx[:], ident[:])
                wrap_in = mst.tile([16, P], f32, tag="wrap_in")
                nc.vector.tensor_copy(wrap_in[:], pwr[:16, :P])
                spg = mst.tile([16, 8], f32, tag="spg")
                nfound = mst.tile([1, 1], u32, tag="nfound")
                nc.gpsimd.sparse_gather(out=spg[:], in_=wrap_in[:], num_found=nfound[:])
                spg_cl = mst.tile([16, 8], f32, tag="spg_cl")
                nc.vector.tensor_scalar_max(spg_cl[:], spg[:], 0.0)
                nc.vector.tensor_scalar_min(spg_cl[:], spg_cl[:], float(L - 1))

                # unwrap [16,8] -> [128,1] with 8 tiny one-hot matmuls
                # (perm16[p, f*128+u] = 1 iff u == p + 16*f, shipped constant)
                pcq = ps.tile([P, 1], f32, tag="blk", name="pcq")
                for f in range(8):
                    nc.tensor.matmul(
                        pcq[:P, :1], perm16[:, f * P : (f + 1) * P],
                        spg_cl[:, f : f + 1],
                        start=(f == 0), stop=(f == 7),
                    )
                candq_f = mst.tile([P, 1], f32, tag="candq_f")
                nc.vector.tensor_copy(candq_f[:], pcq[:P, :1])
                candq_i = mst.tile([P, 1], i32, tag="candq_i")
                nc.vector.tensor_copy(candq_i[:], pcq[:P, :1])

                nf_f = mst.tile([1, 1], f32, tag="nf_f")
                nc.vector.tensor_copy(nf_f[:], nfound[:])
                pnb = ps.tile([P, 1], f32, tag="blk")
                nc.tensor.matmul(pnb[:P, :1], ones_r1[:], nf_f[:], start=True, stop=True)
                nbc = mst.tile([P, 1], f32, tag="nbc")
                nc.vector.tensor_copy(nbc[:], pnb[:P, :1])
                invalid = mst.tile([P, 1], u8, tag="invalid")
                nc.vector.tensor_tensor(
                    out=invalid[:], in0=qidx_f[:, 0:1], in1=nbc[:], op=OP.is_ge
                )

                # ---------------- phase 4a: exact candidates ----------------
                x_cand = cnd.tile([P, D], f32, tag="x_cand")
                nc.gpsimd.indirect_dma_start(
                    out=x_cand[:], out_offset=None, in_=x_d[:],
                    in_offset=bass.IndirectOffsetOnAxis(ap=candq_i[:, :1], axis=0),
                )
                xcT = [cnd.tile([P, P], f32, tag=f"xcT{dc}", name=f"xcT{dc}") for dc in range(ND)]
                for dc in range(ND):
                    pxc = ps.tile([P, P], f32, tag="blk")
                    nc.tensor.transpose(
                        pxc[:P, :P], x_cand[:, dc * P : (dc + 1) * P], ident[:]
                    )
                    nc.vector.tensor_copy(xcT[dc][:], pxc[:P, :P])

                QcT = [cnd.tile([P, P], f32, tag=f"QcT{ic}", name=f"QcT{ic}") for ic in range(ND)]
                for ic in range(ND):
                    isl = slice(ic * P, (ic + 1) * P)
                    pqc = ps.tile([P, P], f32, tag="blk")
                    for dc in range(ND):
                        nc.tensor.matmul(
                            pqc[:P, :P], wqT[dc][:, isl], xcT[dc][:],
                            start=(dc == 0), stop=(dc == ND - 1),
                        )
                    nc.vector.tensor_copy(QcT[ic][:], pqc[:P, :P])

                gm = cnd.tile([P, L], u8, tag="gm")
                nc.gpsimd.indirect_dma_start(
                    out=gm[:], out_offset=None, in_=mask_d[:],
                    in_offset=bass.IndirectOffsetOnAxis(ap=candq_i[:, :1], axis=0),
                )
                gc = cnd.tile([P, L], u8, tag="gc")
                nc.gpsimd.indirect_dma_start(
                    out=gc[:], out_offset=None, in_=cnt_d[:],
                    in_offset=bass.IndirectOffsetOnAxis(ap=candq_i[:, :1], axis=0),
                )

                psS = []
                cmax = cnd.tile([P, NJ], f32, tag="cmax")
                csum = cnd.tile([P, NJ], f32, tag="csum")
                for jb in range(NJ):
                    jsl = slice(jb * 512, (jb + 1) * 512)
                    pss2 = ps_s.tile([P, 512], f32, tag="psSc")
                    psS.append(pss2)
                    for ic in range(ND):
                        nc.tensor.matmul(
                            pss2[:], QcT[ic][:], KT[ic][:, jsl],
                            start=(ic == 0), stop=(ic == ND - 1),
                        )
                    s3 = scr.tile([P, 512], f32, tag="scrt")
                    nc.vector.tensor_tensor(
                        out=s3[:], in0=pss2[:], in1=gm[:, jsl], op=OP.mult
                    )
                    nc.vector.reduce_max(cmax[:, jb : jb + 1], s3[:], axis=AX.X)
                    s4 = scr.tile([P, 512], f32, tag="scrt")
                    nc.vector.scalar_tensor_tensor(
                        out=s4[:], in0=pss2[:], scalar=-1.0 / L, in1=gc[:, jsl],
                        op0=OP.mult, op1=OP.mult,
                        accum_out=csum[:, jb : jb + 1],
                    )
                u1 = cnd.tile([P, 1], f32, tag="u1")
                u2 = cnd.tile([P, 1], f32, tag="u2")
                M_cand = cnd.tile([P, 1], f32, tag="M_cand")
                nc.vector.reduce_max(u1[:], cmax[:], axis=AX.X)
                nc.vector.reduce_sum(u2[:], csum[:], axis=AX.X)
                nc.vector.tensor_tensor(out=M_cand[:], in0=u1[:], in1=u2[:], op=OP.add)
                nc.vector.copy_predicated(M_cand[:], invalid[:], negbig[:])

                # exact top-40 threshold among candidates
                pmc = ps.tile([1, P], f32, tag="blk")
                nc.tensor.transpose(pmc[:1, :P], M_cand[:], ident[:])
                mcT = cnd.tile([1, P], f32, tag="mcT")
                nc.vector.tensor_copy(mcT[:], pmc[:1, :P])
                etop = cnd.tile([1, NT], f32, tag="etop")
                for r in range(5):
                    nc.vector.max(out=etop[:, 8 * r : 8 * r + 8], in_=mcT[:])
                    if r < 4:
                        nc.vector.match_replace(
                            out=mcT[:], in_to_replace=etop[:, 8 * r : 8 * r + 8],
                            in_values=mcT[:], imm_value=NEG,
                        )
                pte = ps.tile([P, 1], f32, tag="blk")
                nc.tensor.matmul(
                    pte[:P, :1], ones_r1[:], etop[:, NT - 1 : NT], start=True, stop=True
                )
                tebc = cnd.tile([P, 1], f32, tag="tebc")
                nc.vector.tensor_copy(tebc[:], pte[:P, :1])
                sel2 = cnd.tile([P, 1], u8, tag="sel2")
                nc.vector.tensor_tensor(
                    out=sel2[:], in0=M_cand[:], in1=tebc[:], op=OP.is_ge
                )
                scat_f = cnd.tile([P, 1], f32, tag="scat_f")
                nc.vector.tensor_copy(scat_f[:], big9[:])
                nc.vector.copy_predicated(scat_f[:], sel2[:], candq_f[:])
                scat_i = cnd.tile([P, 1], i32, tag="scat_i")
                nc.vector.tensor_copy(scat_i[:], scat_f[:])

            # xTh/xTl/weights freed here
            with tc.tile_pool(name="expp", bufs=1) as expp:
                # ---------------- phase 4b: softmax + update ----------------
                rmax4 = expp.tile([P, NJ], f32, tag="rmax4")
                for jb in range(NJ):
                    nc.vector.reduce_max(rmax4[:, jb : jb + 1], psS[jb][:], axis=AX.X)
                rmax = expp.tile([P, 1], f32, tag="rmax")
                nc.vector.reduce_max(rmax[:], rmax4[:], axis=AX.X)
                negb = expp.tile([P, 1], f32, tag="negb")
                nc.vector.tensor_scalar_mul(negb[:], rmax[:], -SCALE)
                exp_sb = expp.tile([P, L], f32, tag="exp_sb")
                sume4 = expp.tile([P, NJ], f32, tag="sume4")
                for jb in range(NJ):
                    jsl = slice(jb * 512, (jb + 1) * 512)
                    nc.scalar.activation(
                        out=exp_sb[:, jsl], in_=psS[jb][:], func=ACTF.Exp,
                        bias=negb[:], scale=SCALE,
                        accum_out=sume4[:, jb : jb + 1],
                    )
                sume = expp.tile([P, 1], f32, tag="sume")
                nc.vector.reduce_sum(sume[:], sume4[:], axis=AX.X)
                recip = expp.tile([P, 1], f32, tag="recip")
                nc.vector.reciprocal(recip[:], sume[:])

                expT = [expp.tile([P, P], bf16, tag=f"expT{jc}", name=f"expT{jc}") for jc in range(NL)]
                for jc in range(NL):
                    pet = ps.tile([P, P], f32, tag="blk")
                    nc.tensor.transpose(
                        pet[:P, :P], exp_sb[:, jc * P : (jc + 1) * P], ident[:]
                    )
                    nc.vector.tensor_copy(expT[jc][:], pet[:P, :P])

                pu = ps.tile([P, 512], f32, tag="blk")
                for jc in range(NL):
                    nc.tensor.matmul(
                        pu[:], expT[jc][:], Vb[jc][:],
                        start=(jc == 0), stop=(jc == NL - 1),
                    )
                upd = expp.tile([P, D], f32, tag="upd")
                nc.scalar.activation(
                    out=upd[:], in_=pu[:], func=ACTF.Copy, bias=0.0, scale=recip[:]
                )
                nc.gpsimd.indirect_dma_start(
                    out=ctx_d[:],
                    out_offset=bass.IndirectOffsetOnAxis(ap=scat_i[:, :1], axis=0),
                    in_=upd[:], in_offset=None,
                    bounds_check=L - 1, oob_is_err=False,
                )

    nc.compile()
    return nc


_NC = None


def _get_nc():
    global _NC
    if _NC is None:
        _NC = build()
    return _NC


def _split_bf16(a):
    hi = a.astype(ml_dtypes.bfloat16)
    lo = (a - hi.astype(np.float32)).astype(ml_dtypes.bfloat16)
    return hi, lo


def _host_prep(x, Wq, Wk, Wv, index_sample):
    x = np.asarray(x, dtype=np.float32)
    Wq = np.asarray(Wq, dtype=np.float32)
    Wk = np.asarray(Wk, dtype=np.float32)
    Wv = np.asarray(Wv, dtype=np.float32)
    idx = np.asarray(index_sample)

    wqT = np.ascontiguousarray(Wq.T)
    wqh, _ = _split_bf16(wqT)
    wkh, wkl = _split_bf16(np.ascontiguousarray(Wk.T))
    wvh, wvl = _split_bf16(np.ascontiguousarray(Wv.T))

    rows = np.arange(L)[:, None]
    mask01 = np.zeros((L, L), dtype=np.uint8)
    mask01[rows, idx] = 1
    countf = np.zeros((L, L), dtype=np.uint8)
    np.add.at(countf, (rows, idx), 1)

    perm16 = np.zeros((16, 8 * P), dtype=np.float32)
    for f in range(8):
        for p in range(16):
            perm16[p, f * P + p + 16 * f] = 1.0
    shared = {
        "wqTh": wqh, "wkTh": wkh, "wkTl": wkl, "wvTh": wvh, "wvTl": wvl,
        "wqT": wqT, "mask01": mask01, "countf": countf, "perm16": perm16,
    }
    in_maps = []
    for b in range(B):
        xb = np.ascontiguousarray(x[b])
        xT = np.ascontiguousarray(xb.T)
        xth, xtl = _split_bf16(xT)
        xmean = (xb.astype(np.float64).mean(axis=0) / 1.0).astype(np.float32)
        xmeh, xmel = _split_bf16(xmean.reshape(D, 1))
        in_maps.append(
            {
                "x_nat": xb,
                "xTh": xth,
                "xTl": xtl,
                "xmeanTh": xmeh,
                "xmeanTl": xmel,
                **shared,
            }
        )
    return in_maps


def kernel(x, Wq, Wk, Wv, index_sample, _trace=False, _result_box=None):
    in_maps = _host_prep(x, Wq, Wk, Wv, index_sample)
    nc = _get_nc()
    res = run_bass_kernel_spmd(nc, in_maps, core_ids=list(range(B)), trace=_trace)
    if _result_box is not None:
        _result_box.append(res)
    out = np.stack([np.asarray(res.results[b]["ctx"]) for b in range(B)], axis=0)
    return out



# revision 8
# speedup vs baseline: 1.1417x; 1.1417x over previous
"""Sparse attention (ProbSparse-style) Trainium2 Bass kernel.

Problem (per batch element b, data-parallel over 8 NeuronCores):
  Q = x @ Wq.T ; K = x @ Wk.T ; V = x @ Wv.T            [L=2048, D=512]
  QK_sample[l,s] = Q[l] . K[index_sample[l,s]]           [L, 40]
  M[l] = max_s QK_sample - sum_s QK_sample / L
  sel = top40(M)  (as a set; the reference scatter makes order irrelevant)
  scores = Q[sel] @ K.T / sqrt(D); attn = softmax(scores)
  ctx = broadcast(mean(V)); ctx[sel] = attn @ V

Numerics strategy (top-40 boundary gaps are as small as 0.02 in M):
  - K and V are computed with a 3-term bf16x2 split matmul
    (xh*wh + xl*wh + xh*wl, host-split halves) -> ~1e-5 absolute error,
    fp32-class, at full bf16 PE rate.
  - Approx M for ALL rows uses bf16 Q and bf16 K (error sigma ~0.2),
    extracted from per-chunk S = Q K^T PSUM blocks with fused
    tensor_tensor_reduce against a shipped u8 sample mask
    (multiply-mask max is safe: sampled max > 0 w.p. 1-2^-40;
    dup-count correction is deferred to the exact stage).
  - Candidates = { M_approx >= approx-top40 - DELTA }, DELTA=1.5 covers
    ~8 sigma; measured rank-40 to rank-64 M gap is 2.5-4.8 so the
    candidate count stays well under the 128-slot budget.
  - Exact stage on <= 128 candidate rows: gather x rows from DRAM
    (indirect DMA), exact fp32 Q_cand, exact S_cand vs the fp32-class K,
    TTR with gathered u8 mask+count rows -> exact M_cand -> exact top-40
    threshold -> softmax over S_cand -> upd = attn @ V -> indirect
    scatter of the 40 selected rows into ctx (bounds_check skips the
    rest).

kernel(**inputs) accepts the FULL inputs and returns the FULL
[8, 2048, 512] f32 output; batch is sharded over 8 cores.
"""

import math

import numpy as np
import ml_dtypes

import concourse.bacc as bacc
import concourse.bass as bass
import concourse.mybir as mybir
import concourse.tile as tile
from concourse.bass_utils import run_bass_kernel_spmd

P = 128
L = 2048
D = 512
B = 8
NL = L // P        # 16 query chunks
ND = D // P        # 4 feature chunks
NJ = L // 512      # 4 key blocks of 512
NT = 40
SCALE = 1.0 / math.sqrt(D)
DELTA = 1.5        # candidate band below approx T40
NEG = -3.0e38
SKIP_IDX = 99999.0  # scatter index sentinel (> bounds_check -> row skipped)

f32 = mybir.dt.float32
bf16 = mybir.dt.bfloat16
u8 = mybir.dt.uint8
i32 = mybir.dt.int32
u32 = mybir.dt.uint32
AX = mybir.AxisListType
OP = mybir.AluOpType
ACTF = mybir.ActivationFunctionType


def build():
    nc = bacc.Bacc("TRN2", target_bir_lowering=False)

    x_d = nc.dram_tensor("x_nat", [L, D], f32, kind="ExternalInput")
    xth_d = nc.dram_tensor("xTh", [D, L], bf16, kind="ExternalInput")
    xtl_d = nc.dram_tensor("xTl", [D, L], bf16, kind="ExternalInput")
    xmh_d = nc.dram_tensor("xmeanTh", [D, 1], bf16, kind="ExternalInput")
    xml_d = nc.dram_tensor("xmeanTl", [D, 1], bf16, kind="ExternalInput")
    wqh_d = nc.dram_tensor("wqTh", [D, D], bf16, kind="ExternalInput")
    wkh_d = nc.dram_tensor("wkTh", [D, D], bf16, kind="ExternalInput")
    wkl_d = nc.dram_tensor("wkTl", [D, D], bf16, kind="ExternalInput")
    wvh_d = nc.dram_tensor("wvTh", [D, D], bf16, kind="ExternalInput")
    wvl_d = nc.dram_tensor("wvTl", [D, D], bf16, kind="ExternalInput")
    wq_d = nc.dram_tensor("wqT", [D, D], f32, kind="ExternalInput")
    ident_d = nc.dram_tensor("identf", [P, P], f32, kind="ExternalInput")
    qidx_d = nc.dram_tensor("qidxf", [P, 16], f32, kind="ExternalInput")
    perm_d = nc.dram_tensor("perm16", [16, 8 * P], f32, kind="ExternalInput")
    mask_d = nc.dram_tensor("mask01", [L, L], u8, kind="ExternalInput")
    cnt_d = nc.dram_tensor("countf", [L, L], u8, kind="ExternalInput")
    ctx_d = nc.dram_tensor("ctx", [L, D], f32, kind="ExternalOutput")

    with tile.TileContext(nc) as tc:
        with (
            tc.tile_pool(name="const", bufs=1) as cst,
            tc.tile_pool(name="proj", bufs=1) as proj,       # KT/KTb/QTb/V resident
            tc.tile_pool(name="mstuff", bufs=1) as mst,      # M / topk / sel smalls
            tc.tile_pool(name="mstream", bufs=3) as mstr,    # mask chunks
            tc.tile_pool(name="scr", bufs=3) as scr,         # TTR scratch
            tc.tile_pool(name="acc", bufs=2) as accp,        # per-chunk accums
            tc.tile_pool(name="cand", bufs=1) as cnd,        # exact-stage tiles
            tc.tile_pool(name="ps", bufs=3, space="PSUM") as ps,
            tc.tile_pool(name="ps_s", bufs=4, space="PSUM") as ps_s,  # S_cand (held)
            tc.tile_pool(name="dram", bufs=1, space="DRAM") as drp,
        ):
            # ---------------- constants ----------------
            # sparse_gather is the only library-tracked GPSIMD op left;
            # preload its (small) library before the serial tail
            from concourse import library_config
            nc.gpsimd.load_library(library_config.sparse_gather)
            ident = cst.tile([P, P], f32, tag="ident")
            nc.sync.dma_start(ident[:], ident_d[:])
            ones_r1 = cst.tile([1, P], f32, tag="ones_r1")
            nc.vector.memset(ones_r1[:], 1.0)
            negone = cst.tile([P, 1], f32, tag="negone")
            nc.vector.memset(negone[:], -1.0)
            negbig = cst.tile([P, 1], f32, tag="negbig")
            nc.vector.memset(negbig[:], NEG)
            big9 = cst.tile([P, 1], f32, tag="big9")
            nc.vector.memset(big9[:], SKIP_IDX)
            perm16 = cst.tile([16, 8 * P], f32, tag="perm16")
            nc.sync.dma_start(perm16[:], perm_d[:])
            qidx_f = cst.tile([P, 16], f32, tag="qidx_f")    # value p + 128*c
            nc.sync.dma_start(qidx_f[:], qidx_d[:])

            # resident projection outputs
            KT = [proj.tile([P, L], f32, tag=f"KT{ic}", name=f"KT{ic}") for ic in range(ND)]
            KTb = [proj.tile([P, L], bf16, tag=f"KTb{ic}", name=f"KTb{ic}") for ic in range(ND)]
            QTb = [proj.tile([P, L], bf16, tag=f"QTb{ic}", name=f"QTb{ic}") for ic in range(ND)]
            Vb = [proj.tile([P, D], bf16, tag=f"Vb{jc}", name=f"Vb{jc}") for jc in range(NL)]

            with tc.tile_pool(name="xw", bufs=1) as xw:
                # ---------------- phase 0: loads ----------------
                xTh = [xw.tile([P, L], bf16, tag=f"xTh{dc}", name=f"xTh{dc}") for dc in range(ND)]
                xTl = [xw.tile([P, L], bf16, tag=f"xTl{dc}", name=f"xTl{dc}") for dc in range(ND)]
                wqh = [xw.tile([P, D], bf16, tag=f"wqh{dc}", name=f"wqh{dc}") for dc in range(ND)]
                wkh = [xw.tile([P, D], bf16, tag=f"wkh{dc}", name=f"wkh{dc}") for dc in range(ND)]
                wkl = [xw.tile([P, D], bf16, tag=f"wkl{dc}", name=f"wkl{dc}") for dc in range(ND)]
                wvh = [xw.tile([P, D], bf16, tag=f"wvh{dc}", name=f"wvh{dc}") for dc in range(ND)]
                wvl = [xw.tile([P, D], bf16, tag=f"wvl{dc}", name=f"wvl{dc}") for dc in range(ND)]
                wqT = [xw.tile([P, D], f32, tag=f"wqT{dc}", name=f"wqT{dc}") for dc in range(ND)]
                xmh = [xw.tile([P, 1], bf16, tag=f"xmh{dc}", name=f"xmh{dc}") for dc in range(ND)]
                xml = [xw.tile([P, 1], bf16, tag=f"xml{dc}", name=f"xml{dc}") for dc in range(ND)]
                for dc in range(ND):
                    sl = slice(dc * P, (dc + 1) * P)
                    nc.sync.dma_start(xTh[dc][:], xth_d[sl, :])
                    nc.sync.dma_start(xTl[dc][:], xtl_d[sl, :])
                    nc.sync.dma_start(wqh[dc][:], wqh_d[sl, :])
                    nc.sync.dma_start(wkh[dc][:], wkh_d[sl, :])
                    nc.sync.dma_start(wkl[dc][:], wkl_d[sl, :])
                    nc.sync.dma_start(wvh[dc][:], wvh_d[sl, :])
                    nc.sync.dma_start(wvl[dc][:], wvl_d[sl, :])
                    nc.sync.dma_start(wqT[dc][:], wq_d[sl, :])
                    nc.sync.dma_start(xmh[dc][:], xmh_d[sl, :])
                    nc.sync.dma_start(xml[dc][:], xml_d[sl, :])

                # ---------------- phase 1: projections ----------------
                # K: 3-term bf16x2 (fp32-class), into KT f32 + KTb bf16
                for ic in range(ND):
                    isl = slice(ic * P, (ic + 1) * P)
                    for jb in range(NJ):
                        jsl = slice(jb * 512, (jb + 1) * 512)
                        pk = ps.tile([P, 512], f32, tag="blk")
                        n = 0
                        for dc in range(ND):
                            for lh, rh in (
                                (wkh[dc][:, isl], xTh[dc][:, jsl]),
                                (wkh[dc][:, isl], xTl[dc][:, jsl]),
                                (wkl[dc][:, isl], xTh[dc][:, jsl]),
                            ):
                                nc.tensor.matmul(
                                    pk[:], lh, rh,
                                    start=(n == 0), stop=(n == 3 * ND - 1),
                                )
                                n += 1
                        nc.scalar.copy(KT[ic][:, jsl], pk[:])
                        nc.vector.tensor_copy(KTb[ic][:, jsl], pk[:])

                # Q approx: single bf16 term
                for ic in range(ND):
                    isl = slice(ic * P, (ic + 1) * P)
                    for jb in range(NJ):
                        jsl = slice(jb * 512, (jb + 1) * 512)
                        pq = ps.tile([P, 512], f32, tag="blk")
                        for dc in range(ND):
                            nc.tensor.matmul(
                                pq[:], wqh[dc][:, isl], xTh[dc][:, jsl],
                                start=(dc == 0), stop=(dc == ND - 1),
                            )
                        nc.scalar.copy(QTb[ic][:, jsl], pq[:])

                # ---------------- phase 2: approx M (bf16 S) ----------------
                # per (lc, jb) block: one STT (masked product -> bf16 scratch,
                # fused sum accum) + one reduce_max. Combines batched at end.
                M_all = mst.tile([P, 16], f32, tag="M_all")
                amax_all = mst.tile([P, NL * NJ], f32, tag="amax_all")
                asum_all = mst.tile([P, NL * NJ], f32, tag="asum_all")
                for lc in range(NL):
                    lsl = slice(lc * P, (lc + 1) * P)
                    mk = mstr.tile([P, L], u8, tag="mk")
                    nc.sync.dma_start(mk[:], mask_d[lsl, :])
                    for jb in range(NJ):
                        jsl = slice(jb * 512, (jb + 1) * 512)
                        k = lc * NJ + jb
                        pss = ps_s.tile([P, 512], f32, tag="psSc", name="pssa")
                        for ic in range(ND):
                            nc.tensor.matmul(
                                pss[:], QTb[ic][:, lsl], KTb[ic][:, jsl],
                                start=(ic == 0), stop=(ic == ND - 1),
                            )
                        s1 = scr.tile([P, 512], bf16, tag="scrt")
                        nc.vector.scalar_tensor_tensor(
                            out=s1[:], in0=pss[:], scalar=1.0, in1=mk[:, jsl],
                            op0=OP.mult, op1=OP.mult,
                            accum_out=asum_all[:, k : k + 1],
                        )
                        nc.vector.reduce_max(
                            amax_all[:, k : k + 1], s1[:], axis=AX.X
                        )
                t1 = accp.tile([P, 16], f32, tag="t1")
                t2 = accp.tile([P, 16], f32, tag="t2")
                nc.vector.reduce_max(
                    t1[:], amax_all[:].rearrange("p (c j) -> p c j", j=NJ),
                    axis=AX.X,
                )
                nc.vector.reduce_sum(
                    t2[:], asum_all[:].rearrange("p (c j) -> p c j", j=NJ),
                    axis=AX.X,
                )
                nc.vector.tensor_scalar_mul(t2[:], t2[:], -1.0 / L)
                nc.vector.tensor_tensor(
                    out=M_all[:], in0=t1[:], in1=t2[:], op=OP.add
                )

                # ---------------- phase 3: approx top-40 -> candidates ------
                # exact T40 of M_approx without GPSIMD kth_largest (its attn
                # library reload cost ~50us of dead time on the serial tail):
                # per-chunk top-16 via vector max8/match_replace on M^T
                # [16,128], union (256 vals) holds the global top-40 w.p.
                # 1-2e-8, pack union into one [1,256] row via one-hot matmul
                # unwrap + transposes, then 5 rounds max8/match_replace.
                pmt = ps.tile([16, P], f32, tag="blk", name="pmt")
                nc.tensor.transpose(pmt[:16, :P], M_all[:], ident[:])
                MT = mst.tile([16, P], f32, tag="MT")
                nc.vector.tensor_copy(MT[:], pmt[:16, :P])
                w16 = mst.tile([16, 16], f32, tag="w16")
                nc.vector.max(out=w16[:, 0:8], in_=MT[:])
                nc.vector.match_replace(
                    out=MT[:], in_to_replace=w16[:, 0:8],
                    in_values=MT[:], imm_value=NEG,
                )
                nc.vector.max(out=w16[:, 8:16], in_=MT[:])

                # V projection here (single bf16 term: the upd matmul consumes
                # bf16 anyway): keeps PE busy while the vector engine runs the
                # threshold rounds and GPSIMD compacts candidates
                for lc in range(NL):
                    lsl = slice(lc * P, (lc + 1) * P)
                    pv = ps.tile([P, 512], f32, tag="blk")
                    for dc in range(ND):
                        nc.tensor.matmul(
                            pv[:], xTh[dc][:, lsl], wvh[dc][:],
                            start=(dc == 0), stop=(dc == ND - 1),
                        )
                    nc.scalar.copy(Vb[lc][:], pv[:])

                # Vmean = xmean.T @ Wv.T via bf16x2 3-term, broadcast, ctx init
                pvm = ps.tile([1, 512], f32, tag="blk")
                n = 0
                for dc in range(ND):
                    for lh, rh in (
                        (xmh[dc][:], wvh[dc][:]),
                        (xml[dc][:], wvh[dc][:]),
                        (xmh[dc][:], wvl[dc][:]),
                    ):
                        nc.tensor.matmul(
                            pvm[:1, :], lh, rh,
                            start=(n == 0), stop=(n == 3 * ND - 1),
                        )
                        n += 1
                vmean = mst.tile([1, 512], f32, tag="vmean")
                nc.scalar.copy(vmean[:], pvm[:1, :])
                pvb = ps.tile([P, 512], f32, tag="blk")
                nc.tensor.matmul(pvb[:], ones_r1[:], vmean[:], start=True, stop=True)
                vmean_bc = mst.tile([P, 512], f32, tag="vmean_bc")
                nc.vector.tensor_copy(vmean_bc[:], pvb[:])
                for jc in range(NL):
                    nc.sync.dma_start(ctx_d[jc * P : (jc + 1) * P, :], vmean_bc[:])

                # unwrap w16 [16,16] -> two [128,1] columns (one-hot matmuls),
                # then -> [1,256] row via two PE transposes
                pcu = ps.tile([P, 2], f32, tag="blk", name="pcu")
                for f in range(8):
                    nc.tensor.matmul(
                        pcu[:P, 0:1], perm16[:, f * P : (f + 1) * P],
                        w16[:, f : f + 1],
                        start=(f == 0), stop=(f == 7),
                    )
                for f in range(8):
                    nc.tensor.matmul(
                        pcu[:P, 1:2], perm16[:, f * P : (f + 1) * P],
                        w16[:, 8 + f : 9 + f],
                        start=(f == 0), stop=(f == 7),
                    )
                crow = mst.tile([P, 2], f32, tag="crow")
                nc.vector.tensor_copy(crow[:], pcu[:P, :2])
                pr1 = ps.tile([1, P], f32, tag="blk", name="pr1")
                nc.tensor.transpose(pr1[:1, :P], crow[:, 0:1], ident[:])
                wrow = mst.tile([1, 2 * P], f32, tag="wrow")
                nc.vector.tensor_copy(wrow[:, 0:P], pr1[:1, :P])
                pr2 = ps.tile([1, P], f32, tag="blk", name="pr2")
                nc.tensor.transpose(pr2[:1, :P], crow[:, 1:2], ident[:])
                nc.vector.tensor_copy(wrow[:, P : 2 * P], pr2[:1, :P])
                etop40 = mst.tile([1, NT], f32, tag="etop40")
                for r in range(5):
                    nc.vector.max(out=etop40[:, 8 * r : 8 * r + 8], in_=wrow[:])
                    if r < 4:
                        nc.vector.match_replace(
                            out=wrow[:], in_to_replace=etop40[:, 8 * r : 8 * r + 8],
                            in_values=wrow[:], imm_value=NEG,
                        )
                ptb = ps.tile([P, 1], f32, tag="blk")
                nc.tensor.matmul(
                    ptb[:P, :1], ones_r1[:], etop40[:, NT - 1 : NT],
                    start=True, stop=True,
                )
                tbc = mst.tile([P, 1], f32, tag="tbc")
                nc.vector.tensor_copy(tbc[:], ptb[:P, :1])

                # selmask = (M - T40) >= -DELTA, one fused op
                selmask = mst.tile([P, 16], u8, tag="selmask")
                nc.vector.tensor_scalar(
                    selmask[:], M_all[:], tbc[:], -DELTA,
                    op0=OP.subtract, op1=OP.is_ge,
                )
                midx = mst.tile([P, 16], f32, tag="midx")
                nc.vector.tensor_copy(midx[:], negone[:].to_broadcast([P, 16]))
                nc.vector.copy_predicated(midx[:], selmask[:], qidx_f[:])

                pwr = ps.tile([16, P], f32, tag="blk", name="pwr")
                nc.tensor.transpose(pwr[:16, :P], midx[:], ident[:])
                wrap_in = mst.tile([16, P], f32, tag="wrap_in")
                nc.vector.tensor_copy(wrap_in[:], pwr[:16, :P])
                spg = mst.tile([16, 8], f32, tag="spg")
                nfound = mst.tile([1, 1], u32, tag="nfound")
                nc.gpsimd.sparse_gather(out=spg[:], in_=wrap_in[:], num_found=nfound[:])
                spg_cl = mst.tile([16, 8], f32, tag="spg_cl")
                nc.vector.tensor_scalar_max(spg_cl[:], spg[:], 0.0)
                nc.vector.tensor_scalar_min(spg_cl[:], spg_cl[:], float(L - 1))

                # unwrap [16,8] -> [128,1] with 8 tiny one-hot matmuls
                # (perm16[p, f*128+u] = 1 iff u == p + 16*f, shipped constant)
                pcq = ps.tile([P, 1], f32, tag="blk", name="pcq")
                for f in range(8):
                    nc.tensor.matmul(
                        pcq[:P, :1], perm16[:, f * P : (f + 1) * P],
                        spg_cl[:, f : f + 1],
                        start=(f == 0), stop=(f == 7),
                    )
                candq_f = mst.tile([P, 1], f32, tag="candq_f")
                nc.vector.tensor_copy(candq_f[:], pcq[:P, :1])
                candq_i = mst.tile([P, 1], i32, tag="candq_i")
                nc.vector.tensor_copy(candq_i[:], pcq[:P, :1])

                nf_f = mst.tile([1, 1], f32, tag="nf_f")
                nc.vector.tensor_copy(nf_f[:], nfound[:])
                pnb = ps.tile([P, 1], f32, tag="blk")
                nc.tensor.matmul(pnb[:P, :1], ones_r1[:], nf_f[:], start=True, stop=True)
                nbc = mst.tile([P, 1], f32, tag="nbc")
                nc.vector.tensor_copy(nbc[:], pnb[:P, :1])
                invalid = mst.tile([P, 1], u8, tag="invalid")
                nc.vector.tensor_tensor(
                    out=invalid[:], in0=qidx_f[:, 0:1], in1=nbc[:], op=OP.is_ge
                )

                # ---------------- phase 4a: exact candidates ----------------
                x_cand = cnd.tile([P, D], f32, tag="x_cand")
                nc.gpsimd.indirect_dma_start(
                    out=x_cand[:], out_offset=None, in_=x_d[:],
                    in_offset=bass.IndirectOffsetOnAxis(ap=candq_i[:, :1], axis=0),
                )
                xcT = [cnd.tile([P, P], f32, tag=f"xcT{dc}", name=f"xcT{dc}") for dc in range(ND)]
                for dc in range(ND):
                    pxc = ps.tile([P, P], f32, tag="blk")
                    nc.tensor.transpose(
                        pxc[:P, :P], x_cand[:, dc * P : (dc + 1) * P], ident[:]
                    )
                    nc.vector.tensor_copy(xcT[dc][:], pxc[:P, :P])

                QcT = [cnd.tile([P, P], f32, tag=f"QcT{ic}", name=f"QcT{ic}") for ic in range(ND)]
                for ic in range(ND):
                    isl = slice(ic * P, (ic + 1) * P)
                    pqc = ps.tile([P, P], f32, tag="blk")
                    for dc in range(ND):
                        nc.tensor.matmul(
                            pqc[:P, :P], wqT[dc][:, isl], xcT[dc][:],
                            start=(dc == 0), stop=(dc == ND - 1),
                        )
                    nc.vector.tensor_copy(QcT[ic][:], pqc[:P, :P])

                gm = cnd.tile([P, L], u8, tag="gm")
                nc.gpsimd.indirect_dma_start(
                    out=gm[:], out_offset=None, in_=mask_d[:],
                    in_offset=bass.IndirectOffsetOnAxis(ap=candq_i[:, :1], axis=0),
                )
                gc = cnd.tile([P, L], u8, tag="gc")
                nc.gpsimd.indirect_dma_start(
                    out=gc[:], out_offset=None, in_=cnt_d[:],
                    in_offset=bass.IndirectOffsetOnAxis(ap=candq_i[:, :1], axis=0),
                )

                psS = []
                cmax = cnd.tile([P, NJ], f32, tag="cmax")
                csum = cnd.tile([P, NJ], f32, tag="csum")
                for jb in range(NJ):
                    jsl = slice(jb * 512, (jb + 1) * 512)
                    pss2 = ps_s.tile([P, 512], f32, tag="psSc")
                    psS.append(pss2)
                    for ic in range(ND):
                        nc.tensor.matmul(
                            pss2[:], QcT[ic][:], KT[ic][:, jsl],
                            start=(ic == 0), stop=(ic == ND - 1),
                        )
                    s3 = scr.tile([P, 512], f32, tag="scrt")
                    nc.vector.tensor_tensor(
                        out=s3[:], in0=pss2[:], in1=gm[:, jsl], op=OP.mult
                    )
                    nc.vector.reduce_max(cmax[:, jb : jb + 1], s3[:], axis=AX.X)
                    s4 = scr.tile([P, 512], f32, tag="scrt")
                    nc.vector.scalar_tensor_tensor(
                        out=s4[:], in0=pss2[:], scalar=-1.0 / L, in1=gc[:, jsl],
                        op0=OP.mult, op1=OP.mult,
                        accum_out=csum[:, jb : jb + 1],
                    )
                u1 = cnd.tile([P, 1], f32, tag="u1")
                u2 = cnd.tile([P, 1], f32, tag="u2")
                M_cand = cnd.tile([P, 1], f32, tag="M_cand")
                nc.vector.reduce_max(u1[:], cmax[:], axis=AX.X)
                nc.vector.reduce_sum(u2[:], csum[:], axis=AX.X)
                nc.vector.tensor_tensor(out=M_cand[:], in0=u1[:], in1=u2[:], op=OP.add)
                nc.vector.copy_predicated(M_cand[:], invalid[:], negbig[:])

                # exact top-40 threshold among candidates
                pmc = ps.tile([1, P], f32, tag="blk")
                nc.tensor.transpose(pmc[:1, :P], M_cand[:], ident[:])
                mcT = cnd.tile([1, P], f32, tag="mcT")
                nc.vector.tensor_copy(mcT[:], pmc[:1, :P])
                etop = cnd.tile([1, NT], f32, tag="etop")
                for r in range(5):
                    nc.vector.max(out=etop[:, 8 * r : 8 * r + 8], in_=mcT[:])
                    if r < 4:
                        nc.vector.match_replace(
                            out=mcT[:], in_to_replace=etop[:, 8 * r : 8 * r + 8],
                            in_values=mcT[:], imm_value=NEG,
                        )
                pte = ps.tile([P, 1], f32, tag="blk")
                nc.tensor.matmul(
                    pte[:P, :1], ones_r1[:], etop[:, NT - 1 : NT], start=True, stop=True
                )
                tebc = cnd.tile([P, 1], f32, tag="tebc")
                nc.vector.tensor_copy(tebc[:], pte[:P, :1])
                sel2 = cnd.tile([P, 1], u8, tag="sel2")
                nc.vector.tensor_tensor(
                    out=sel2[:], in0=M_cand[:], in1=tebc[:], op=OP.is_ge
                )
                scat_f = cnd.tile([P, 1], f32, tag="scat_f")
                nc.vector.tensor_copy(scat_f[:], big9[:])
                nc.vector.copy_predicated(scat_f[:], sel2[:], candq_f[:])
                scat_i = cnd.tile([P, 1], i32, tag="scat_i")
                nc.vector.tensor_copy(scat_i[:], scat_f[:])

            # xTh/xTl/weights freed here
            with tc.tile_pool(name="expp", bufs=1) as expp:
                # ---------------- phase 4b: softmax + update ----------------
                # no max-subtraction: scores*SCALE is O(10), exp is fp32-safe
                exp_sb = expp.tile([P, L], f32, tag="exp_sb")
                sume4 = expp.tile([P, NJ], f32, tag="sume4")
                for jb in range(NJ):
                    jsl = slice(jb * 512, (jb + 1) * 512)
                    nc.scalar.activation(
                        out=exp_sb[:, jsl], in_=psS[jb][:], func=ACTF.Exp,
                        bias=0.0, scale=SCALE,
                        accum_out=sume4[:, jb : jb + 1],
                    )
                sume = expp.tile([P, 1], f32, tag="sume")
                nc.vector.reduce_sum(sume[:], sume4[:], axis=AX.X)
                recip = expp.tile([P, 1], f32, tag="recip")
                nc.vector.reciprocal(recip[:], sume[:])

                expT = [expp.tile([P, P], bf16, tag=f"expT{jc}", name=f"expT{jc}") for jc in range(NL)]
                for jc in range(NL):
                    pet = ps.tile([P, P], f32, tag="blk")
                    nc.tensor.transpose(
                        pet[:P, :P], exp_sb[:, jc * P : (jc + 1) * P], ident[:]
                    )
                    nc.vector.tensor_copy(expT[jc][:], pet[:P, :P])

                pu = ps.tile([P, 512], f32, tag="blk")
                for jc in range(NL):
                    nc.tensor.matmul(
                        pu[:], expT[jc][:], Vb[jc][:],
                        start=(jc == 0), stop=(jc == NL - 1),
                    )
                upd = expp.tile([P, D], f32, tag="upd")
                nc.scalar.activation(
                    out=upd[:], in_=pu[:], func=ACTF.Copy, bias=0.0, scale=recip[:]
                )
                nc.gpsimd.indirect_dma_start(
                    out=ctx_d[:],
                    out_offset=bass.IndirectOffsetOnAxis(ap=scat_i[:, :1], axis=0),
                    in_=upd[:], in_offset=None,
                    bounds_check=L - 1, oob_is_err=False,
                )

    nc.compile()
    return nc


_NC = None


def _get_nc():
    global _NC
    if _NC is None:
        _NC = build()
    return _NC


def _split_bf16(a):
    hi = a.astype(ml_dtypes.bfloat16)
    lo = (a - hi.astype(np.float32)).astype(ml_dtypes.bfloat16)
    return hi, lo


def _host_prep(x, Wq, Wk, Wv, index_sample):
    x = np.asarray(x, dtype=np.float32)
    Wq = np.asarray(Wq, dtype=np.float32)
    Wk = np.asarray(Wk, dtype=np.float32)
    Wv = np.asarray(Wv, dtype=np.float32)
    idx = np.asarray(index_sample)

    wqT = np.ascontiguousarray(Wq.T)
    wqh, _ = _split_bf16(wqT)
    wkh, wkl = _split_bf16(np.ascontiguousarray(Wk.T))
    wvh, wvl = _split_bf16(np.ascontiguousarray(Wv.T))

    rows = np.arange(L)[:, None]
    mask01 = np.zeros((L, L), dtype=np.uint8)
    mask01[rows, idx] = 1
    countf = np.zeros((L, L), dtype=np.uint8)
    np.add.at(countf, (rows, idx), 1)

    perm16 = np.zeros((16, 8 * P), dtype=np.float32)
    for f in range(8):
        for p in range(16):
            perm16[p, f * P + p + 16 * f] = 1.0
    identf = np.eye(P, dtype=np.float32)
    qidxf = (np.arange(P, dtype=np.float32)[:, None]
             + 128.0 * np.arange(16, dtype=np.float32)[None, :])
    shared = {
        "wqTh": wqh, "wkTh": wkh, "wkTl": wkl, "wvTh": wvh, "wvTl": wvl,
        "wqT": wqT, "mask01": mask01, "countf": countf, "perm16": perm16,
        "identf": identf, "qidxf": qidxf,
    }
    in_maps = []
    for b in range(B):
        xb = np.ascontiguousarray(x[b])
        xT = np.ascontiguousarray(xb.T)
        xth, xtl = _split_bf16(xT)
        xmean = (xb.astype(np.float64).mean(axis=0) / 1.0).astype(np.float32)
        xmeh, xmel = _split_bf16(xmean.reshape(D, 1))
        in_maps.append(
            {
                "x_nat": xb,
                "xTh": xth,
                "xTl": xtl,
                "xmeanTh": xmeh,
                "xmeanTl": xmel,
                **shared,
            }
        )
    return in_maps


def kernel(x, Wq, Wk, Wv, index_sample, _trace=False, _result_box=None):
    in_maps = _host_prep(x, Wq, Wk, Wv, index_sample)
    nc = _get_nc()
    res = run_bass_kernel_spmd(nc, in_maps, core_ids=list(range(B)), trace=_trace)
    if _result_box is not None:
        _result_box.append(res)
    out = np.stack([np.asarray(res.results[b]["ctx"]) for b in range(B)], axis=0)
    return out



# revision 20
# speedup vs baseline: 1.2202x; 1.0688x over previous
"""Sparse attention (ProbSparse-style) Trainium2 Bass kernel.

Problem (per batch element b, data-parallel over 8 NeuronCores):
  Q = x @ Wq.T ; K = x @ Wk.T ; V = x @ Wv.T            [L=2048, D=512]
  QK_sample[l,s] = Q[l] . K[index_sample[l,s]]           [L, 40]
  M[l] = max_s QK_sample - sum_s QK_sample / L
  sel = top40(M)  (as a set; the reference scatter makes order irrelevant)
  scores = Q[sel] @ K.T / sqrt(D); attn = softmax(scores)
  ctx = broadcast(mean(V)); ctx[sel] = attn @ V

Numerics strategy (top-40 boundary gaps are as small as 0.02 in M):
  - K and V are computed with a 3-term bf16x2 split matmul
    (xh*wh + xl*wh + xh*wl, host-split halves) -> ~1e-5 absolute error,
    fp32-class, at full bf16 PE rate.
  - Approx M for ALL rows uses bf16 Q and bf16 K (error sigma ~0.2),
    extracted from per-chunk S = Q K^T PSUM blocks with fused
    tensor_tensor_reduce against a shipped u8 sample mask
    (multiply-mask max is safe: sampled max > 0 w.p. 1-2^-40;
    dup-count correction is deferred to the exact stage).
  - Candidates = { M_approx >= approx-top40 - DELTA }, DELTA=1.5 covers
    ~8 sigma; measured rank-40 to rank-64 M gap is 2.5-4.8 so the
    candidate count stays well under the 128-slot budget.
  - Exact stage on <= 128 candidate rows: gather x rows from DRAM
    (indirect DMA), exact fp32 Q_cand, exact S_cand vs the fp32-class K,
    TTR with gathered u8 mask+count rows -> exact M_cand -> exact top-40
    threshold -> softmax over S_cand -> upd = attn @ V -> indirect
    scatter of the 40 selected rows into ctx (bounds_check skips the
    rest).

kernel(**inputs) accepts the FULL inputs and returns the FULL
[8, 2048, 512] f32 output; batch is sharded over 8 cores.
"""

import math

import numpy as np
import ml_dtypes

import concourse.bacc as bacc
import concourse.bass as bass
import concourse.mybir as mybir
import concourse.tile as tile
from concourse.bass_utils import run_bass_kernel_spmd

P = 128
L = 2048
D = 512
B = 8
NL = L // P        # 16 query chunks
ND = D // P        # 4 feature chunks
NJ = L // 512      # 4 key blocks of 512
NT = 40
SCALE = 1.0 / math.sqrt(D)
# candidate band below approx T40: covers 2x bf16 dot error (~8 sigma =
# 1.5) plus the omitted -sum_s/L term in approx M (|sum/L| <= ~0.25
# per row at 3.5 sigma, both directions -> +0.5)
DELTA = 2.2
NEG = -3.0e38
SKIP_IDX = 99999.0  # scatter index sentinel (> bounds_check -> row skipped)

f32 = mybir.dt.float32
bf16 = mybir.dt.bfloat16
u8 = mybir.dt.uint8
i32 = mybir.dt.int32
u32 = mybir.dt.uint32
AX = mybir.AxisListType
OP = mybir.AluOpType
ACTF = mybir.ActivationFunctionType


def build():
    nc = bacc.Bacc("TRN2", target_bir_lowering=False)

    x_d = nc.dram_tensor("x_nat", [L, D], f32, kind="ExternalInput")
    xth_d = nc.dram_tensor("xTh", [D, L], bf16, kind="ExternalInput")
    xtl_d = nc.dram_tensor("xTl", [D, L], bf16, kind="ExternalInput")
    xmh_d = nc.dram_tensor("xmeanTh", [D, 1], bf16, kind="ExternalInput")
    xml_d = nc.dram_tensor("xmeanTl", [D, 1], bf16, kind="ExternalInput")
    wqh_d = nc.dram_tensor("wqTh", [D, D], bf16, kind="ExternalInput")
    wkh_d = nc.dram_tensor("wkTh", [D, D], bf16, kind="ExternalInput")
    wkl_d = nc.dram_tensor("wkTl", [D, D], bf16, kind="ExternalInput")
    wvh_d = nc.dram_tensor("wvTh", [D, D], bf16, kind="ExternalInput")
    wvl_d = nc.dram_tensor("wvTl", [D, D], bf16, kind="ExternalInput")
    wq_d = nc.dram_tensor("wqT", [D, D], f32, kind="ExternalInput")
    ident_d = nc.dram_tensor("identf", [P, P], f32, kind="ExternalInput")
    qidx_d = nc.dram_tensor("qidxf", [P, 16], f32, kind="ExternalInput")
    perm_d = nc.dram_tensor("perm16", [16, 8 * P], f32, kind="ExternalInput")
    mask_d = nc.dram_tensor("mask01", [L, L], u8, kind="ExternalInput")
    cnt_d = nc.dram_tensor("countf", [L, L], u8, kind="ExternalInput")
    ctx_d = nc.dram_tensor("ctx", [L, D], f32, kind="ExternalOutput")

    with tile.TileContext(nc) as tc:
        with (
            tc.tile_pool(name="const", bufs=1) as cst,
            tc.tile_pool(name="proj", bufs=1) as proj,       # KT/KTb/QTb/V resident
            tc.tile_pool(name="mstuff", bufs=1) as mst,      # M / topk / sel smalls
            tc.tile_pool(name="mstream", bufs=3) as mstr,    # mask chunks
            tc.tile_pool(name="scr", bufs=3) as scr,         # TTR scratch
            tc.tile_pool(name="acc", bufs=2) as accp,        # per-chunk accums
            tc.tile_pool(name="cand", bufs=1) as cnd,        # exact-stage tiles
            tc.tile_pool(name="ps", bufs=3, space="PSUM") as ps,
            tc.tile_pool(name="ps_s", bufs=4, space="PSUM") as ps_s,  # S_cand (held)
            tc.tile_pool(name="dram", bufs=1, space="DRAM") as drp,
        ):
            # ---------------- constants ----------------
            # sparse_gather is the only library-tracked GPSIMD op left;
            # preload its (small) library before the serial tail
            from concourse import library_config
            nc.gpsimd.load_library(library_config.sparse_gather)
            ident = cst.tile([P, P], f32, tag="ident")
            nc.sync.dma_start(ident[:], ident_d[:])
            ones_r1 = cst.tile([1, P], f32, tag="ones_r1")
            nc.vector.memset(ones_r1[:], 1.0)
            negone = cst.tile([P, 1], f32, tag="negone")
            nc.vector.memset(negone[:], -1.0)
            negbig = cst.tile([P, 1], f32, tag="negbig")
            nc.vector.memset(negbig[:], NEG)
            big9 = cst.tile([P, 1], f32, tag="big9")
            nc.vector.memset(big9[:], SKIP_IDX)
            perm16 = cst.tile([16, 8 * P], f32, tag="perm16")
            nc.sync.dma_start(perm16[:], perm_d[:])
            qidx_f = cst.tile([P, 16], f32, tag="qidx_f")    # value p + 128*c
            nc.sync.dma_start(qidx_f[:], qidx_d[:])

            # resident projection outputs
            KT = [proj.tile([P, L], f32, tag=f"KT{ic}", name=f"KT{ic}") for ic in range(ND)]
            KTb = [proj.tile([P, L], bf16, tag=f"KTb{ic}", name=f"KTb{ic}") for ic in range(ND)]
            QTb = [proj.tile([P, L], bf16, tag=f"QTb{ic}", name=f"QTb{ic}") for ic in range(ND)]
            Vb = [proj.tile([P, D], bf16, tag=f"Vb{jc}", name=f"Vb{jc}") for jc in range(NL)]

            with tc.tile_pool(name="xw", bufs=1) as xw:
                # ---------------- phase 0: loads ----------------
                xTh = [xw.tile([P, L], bf16, tag=f"xTh{dc}", name=f"xTh{dc}") for dc in range(ND)]
                xTl = [xw.tile([P, L], bf16, tag=f"xTl{dc}", name=f"xTl{dc}") for dc in range(ND)]
                wqh = [xw.tile([P, D], bf16, tag=f"wqh{dc}", name=f"wqh{dc}") for dc in range(ND)]
                wkh = [xw.tile([P, D], bf16, tag=f"wkh{dc}", name=f"wkh{dc}") for dc in range(ND)]
                wkl = [xw.tile([P, D], bf16, tag=f"wkl{dc}", name=f"wkl{dc}") for dc in range(ND)]
                wvh = [xw.tile([P, D], bf16, tag=f"wvh{dc}", name=f"wvh{dc}") for dc in range(ND)]
                wvl = [xw.tile([P, D], bf16, tag=f"wvl{dc}", name=f"wvl{dc}") for dc in range(ND)]
                wqT = [xw.tile([P, D], f32, tag=f"wqT{dc}", name=f"wqT{dc}") for dc in range(ND)]
                xmh = [xw.tile([P, 1], bf16, tag=f"xmh{dc}", name=f"xmh{dc}") for dc in range(ND)]
                xml = [xw.tile([P, 1], bf16, tag=f"xml{dc}", name=f"xml{dc}") for dc in range(ND)]
                # weights first (small, needed by every block), then x
                # sliced per 512-col block so phase-1 jb=0 starts early
                for dc in range(ND):
                    sl = slice(dc * P, (dc + 1) * P)
                    nc.sync.dma_start(wkh[dc][:], wkh_d[sl, :])
                    nc.sync.dma_start(wkl[dc][:], wkl_d[sl, :])
                    nc.sync.dma_start(wqh[dc][:], wqh_d[sl, :])
                    nc.sync.dma_start(wvh[dc][:], wvh_d[sl, :])
                    nc.sync.dma_start(wvl[dc][:], wvl_d[sl, :])
                    nc.sync.dma_start(wqT[dc][:], wq_d[sl, :])
                    nc.sync.dma_start(xmh[dc][:], xmh_d[sl, :])
                    nc.sync.dma_start(xml[dc][:], xml_d[sl, :])
                for jb in range(NJ):
                    jsl = slice(jb * 512, (jb + 1) * 512)
                    for dc in range(ND):
                        sl = slice(dc * P, (dc + 1) * P)
                        nc.sync.dma_start(xTh[dc][:, jsl], xth_d[sl, jsl])
                        nc.sync.dma_start(xTl[dc][:, jsl], xtl_d[sl, jsl])

                # ---------------- phase 1: projections ----------------
                # jb-major so the first 512 columns of KTb/QTb finish first
                # and the S blocks can start while later jb still project.
                # K: 3-term bf16x2 (fp32-class), into KT f32 + KTb bf16
                for jb in range(NJ):
                    jsl = slice(jb * 512, (jb + 1) * 512)
                    for ic in range(ND):
                        isl = slice(ic * P, (ic + 1) * P)
                        pk = ps.tile([P, 512], f32, tag="blk")
                        n = 0
                        for dc in range(ND):
                            for lh, rh in (
                                (wkh[dc][:, isl], xTh[dc][:, jsl]),
                                (wkh[dc][:, isl], xTl[dc][:, jsl]),
                                (wkl[dc][:, isl], xTh[dc][:, jsl]),
                            ):
                                nc.tensor.matmul(
                                    pk[:], lh, rh,
                                    start=(n == 0), stop=(n == 3 * ND - 1),
                                )
                                n += 1
                        nc.scalar.copy(KT[ic][:, jsl], pk[:])
                        nc.vector.tensor_copy(KTb[ic][:, jsl], pk[:])
                    # Q approx: single bf16 term
                    for ic in range(ND):
                        isl = slice(ic * P, (ic + 1) * P)
                        pq = ps.tile([P, 512], f32, tag="blk")
                        for dc in range(ND):
                            nc.tensor.matmul(
                                pq[:], wqh[dc][:, isl], xTh[dc][:, jsl],
                                start=(dc == 0), stop=(dc == ND - 1),
                            )
                        nc.scalar.copy(QTb[ic][:, jsl], pq[:])

                # ---------------- phase 2: approx M (bf16 S) ----------------
                # per (lc, jb) block: ONE fused TTR (masked product -> bf16
                # scratch, fused max accum). The -sum_s/L term is omitted in
                # the approx M (absorbed into DELTA); the exact stage still
                # uses the full formula.
                M_all = mst.tile([P, 16], f32, tag="M_all")
                amax_all = mst.tile([P, NL * NJ], f32, tag="amax_all")
                for lc in range(NL):
                    lsl = slice(lc * P, (lc + 1) * P)
                    mk = mstr.tile([P, L], u8, tag="mk")
                    nc.sync.dma_start(mk[:], mask_d[lsl, :])
                    for jb in range(NJ):
                        jsl = slice(jb * 512, (jb + 1) * 512)
                        k = lc * NJ + jb
                        pss = ps_s.tile([P, 512], f32, tag="psSc", name="pssa")
                        for ic in range(ND):
                            nc.tensor.matmul(
                                pss[:], QTb[ic][:, lsl], KTb[ic][:, jsl],
                                start=(ic == 0), stop=(ic == ND - 1),
                            )
                        s1 = scr.tile([P, 512], bf16, tag="scrt")
                        nc.vector.tensor_tensor(
                            out=s1[:], in0=pss[:], in1=mk[:, jsl], op=OP.mult
                        )
                        nc.vector.reduce_max(
                            amax_all[:, k : k + 1], s1[:], axis=AX.X
                        )
                nc.vector.reduce_max(
                    M_all[:], amax_all[:].rearrange("p (c j) -> p c j", j=NJ),
                    axis=AX.X,
                )

                # ---------------- phase 3: approx top-40 -> candidates ------
                # exact T40 of M_approx without GPSIMD kth_largest (its attn
                # library reload cost ~50us of dead time on the serial tail):
                # per-chunk top-16 via vector max8/match_replace on M^T
                # [16,128], union (256 vals) holds the global top-40 w.p.
                # 1-2e-8, pack union into one [1,256] row via one-hot matmul
                # unwrap + transposes, then 5 rounds max8/match_replace.
                pmt = ps.tile([16, P], f32, tag="blk", name="pmt")
                nc.tensor.transpose(pmt[:16, :P], M_all[:], ident[:])
                MT = mst.tile([16, P], f32, tag="MT")
                nc.vector.tensor_copy(MT[:], pmt[:16, :P])
                w16 = mst.tile([16, 16], f32, tag="w16")
                nc.vector.max(out=w16[:, 0:8], in_=MT[:])
                nc.vector.match_replace(
                    out=MT[:], in_to_replace=w16[:, 0:8],
                    in_values=MT[:], imm_value=NEG,
                )
                nc.vector.max(out=w16[:, 8:16], in_=MT[:])

                # V projection (single bf16 term: the upd matmul consumes
                # bf16 anyway) is interleaved through the threshold/compaction
                # chain in 4-chunk groups so PE stays busy while the vector
                # engine and GPSIMD work through the serial tail.
                def v_chunks(lo, hi):
                    for lc in range(lo, hi):
                        lsl = slice(lc * P, (lc + 1) * P)
                        pv = ps.tile([P, 512], f32, tag="blk")
                        for dc in range(ND):
                            nc.tensor.matmul(
                                pv[:], xTh[dc][:, lsl], wvh[dc][:],
                                start=(dc == 0), stop=(dc == ND - 1),
                            )
                        nc.scalar.copy(Vb[lc][:], pv[:])

                v_chunks(0, 4)

                # unwrap w16 [16,16] -> two [128,1] columns (one-hot matmuls),
                # then -> [1,256] row via two PE transposes
                pcu = ps.tile([P, 2], f32, tag="blk", name="pcu")
                for f in range(8):
                    nc.tensor.matmul(
                        pcu[:P, 0:1], perm16[:, f * P : (f + 1) * P],
                        w16[:, f : f + 1],
                        start=(f == 0), stop=(f == 7),
                    )
                for f in range(8):
                    nc.tensor.matmul(
                        pcu[:P, 1:2], perm16[:, f * P : (f + 1) * P],
                        w16[:, 8 + f : 9 + f],
                        start=(f == 0), stop=(f == 7),
                    )
                crow = mst.tile([P, 2], f32, tag="crow")
                nc.vector.tensor_copy(crow[:], pcu[:P, :2])
                pr1 = ps.tile([1, P], f32, tag="blk", name="pr1")
                nc.tensor.transpose(pr1[:1, :P], crow[:, 0:1], ident[:])
                wrow = mst.tile([1, 2 * P], f32, tag="wrow")
                nc.vector.tensor_copy(wrow[:, 0:P], pr1[:1, :P])
                pr2 = ps.tile([1, P], f32, tag="blk", name="pr2")
                nc.tensor.transpose(pr2[:1, :P], crow[:, 1:2], ident[:])
                nc.vector.tensor_copy(wrow[:, P : 2 * P], pr2[:1, :P])

                v_chunks(4, 8)

                etop40 = mst.tile([1, NT], f32, tag="etop40")
                for r in range(5):
                    nc.vector.max(out=etop40[:, 8 * r : 8 * r + 8], in_=wrow[:])
                    if r < 4:
                        nc.vector.match_replace(
                            out=wrow[:], in_to_replace=etop40[:, 8 * r : 8 * r + 8],
                            in_values=wrow[:], imm_value=NEG,
                        )
                ptb = ps.tile([P, 1], f32, tag="blk")
                nc.tensor.matmul(
                    ptb[:P, :1], ones_r1[:], etop40[:, NT - 1 : NT],
                    start=True, stop=True,
                )
                tbc = mst.tile([P, 1], f32, tag="tbc")
                nc.vector.tensor_copy(tbc[:], ptb[:P, :1])

                # selmask = (M - T40) >= -DELTA, one fused op
                selmask = mst.tile([P, 16], u8, tag="selmask")
                nc.vector.tensor_scalar(
                    selmask[:], M_all[:], tbc[:], -DELTA,
                    op0=OP.subtract, op1=OP.is_ge,
                )
                midx = mst.tile([P, 16], f32, tag="midx")
                nc.vector.tensor_copy(midx[:], negone[:].to_broadcast([P, 16]))
                nc.vector.copy_predicated(midx[:], selmask[:], qidx_f[:])

                pwr = ps.tile([16, P], f32, tag="blk", name="pwr")
                nc.tensor.transpose(pwr[:16, :P], midx[:], ident[:])
                wrap_in = mst.tile([16, P], f32, tag="wrap_in")
                nc.vector.tensor_copy(wrap_in[:], pwr[:16, :P])
                spg = mst.tile([16, 8], f32, tag="spg")
                nfound = mst.tile([1, 1], u32, tag="nfound")
                nc.gpsimd.sparse_gather(out=spg[:], in_=wrap_in[:], num_found=nfound[:])

                v_chunks(8, 12)

                spg_cl = mst.tile([16, 8], f32, tag="spg_cl")
                nc.vector.tensor_scalar_max(spg_cl[:], spg[:], 0.0)
                nc.vector.tensor_scalar_min(spg_cl[:], spg_cl[:], float(L - 1))

                # unwrap [16,8] -> [128,1] with 8 tiny one-hot matmuls
                # (perm16[p, f*128+u] = 1 iff u == p + 16*f, shipped constant)
                pcq = ps.tile([P, 1], f32, tag="blk", name="pcq")
                for f in range(8):
                    nc.tensor.matmul(
                        pcq[:P, :1], perm16[:, f * P : (f + 1) * P],
                        spg_cl[:, f : f + 1],
                        start=(f == 0), stop=(f == 7),
                    )
                candq_f = mst.tile([P, 1], f32, tag="candq_f")
                nc.vector.tensor_copy(candq_f[:], pcq[:P, :1])
                candq_i = mst.tile([P, 1], i32, tag="candq_i")
                nc.vector.tensor_copy(candq_i[:], pcq[:P, :1])

                nf_f = mst.tile([1, 1], f32, tag="nf_f")
                nc.vector.tensor_copy(nf_f[:], nfound[:])
                pnb = ps.tile([P, 1], f32, tag="blk")
                nc.tensor.matmul(pnb[:P, :1], ones_r1[:], nf_f[:], start=True, stop=True)
                nbc = mst.tile([P, 1], f32, tag="nbc")
                nc.vector.tensor_copy(nbc[:], pnb[:P, :1])
                invalid = mst.tile([P, 1], u8, tag="invalid")
                nc.vector.tensor_tensor(
                    out=invalid[:], in0=qidx_f[:, 0:1], in1=nbc[:], op=OP.is_ge
                )

                v_chunks(12, 16)

                # Vmean = xmean.T @ Wv.T via bf16x2 3-term, broadcast, ctx init
                pvm = ps.tile([1, 512], f32, tag="blk")
                n = 0
                for dc in range(ND):
                    for lh, rh in (
                        (xmh[dc][:], wvh[dc][:]),
                        (xml[dc][:], wvh[dc][:]),
                        (xmh[dc][:], wvl[dc][:]),
                    ):
                        nc.tensor.matmul(
                            pvm[:1, :], lh, rh,
                            start=(n == 0), stop=(n == 3 * ND - 1),
                        )
                        n += 1
                vmean = mst.tile([1, 512], f32, tag="vmean")
                nc.scalar.copy(vmean[:], pvm[:1, :])
                pvb = ps.tile([P, 512], f32, tag="blk")
                nc.tensor.matmul(pvb[:], ones_r1[:], vmean[:], start=True, stop=True)
                vmean_bc = mst.tile([P, 512], f32, tag="vmean_bc")
                nc.vector.tensor_copy(vmean_bc[:], pvb[:])
                for jc in range(NL):
                    nc.sync.dma_start(ctx_d[jc * P : (jc + 1) * P, :], vmean_bc[:])

                # ---------------- phase 4a: exact candidates ----------------
                x_cand = cnd.tile([P, D], f32, tag="x_cand")
                nc.gpsimd.indirect_dma_start(
                    out=x_cand[:], out_offset=None, in_=x_d[:],
                    in_offset=bass.IndirectOffsetOnAxis(ap=candq_i[:, :1], axis=0),
                )
                xcT = [cnd.tile([P, P], f32, tag=f"xcT{dc}", name=f"xcT{dc}") for dc in range(ND)]
                for dc in range(ND):
                    pxc = ps.tile([P, P], f32, tag="blk")
                    nc.tensor.transpose(
                        pxc[:P, :P], x_cand[:, dc * P : (dc + 1) * P], ident[:]
                    )
                    nc.vector.tensor_copy(xcT[dc][:], pxc[:P, :P])

                QcT = [cnd.tile([P, P], f32, tag=f"QcT{ic}", name=f"QcT{ic}") for ic in range(ND)]
                for ic in range(ND):
                    isl = slice(ic * P, (ic + 1) * P)
                    pqc = ps.tile([P, P], f32, tag="blk")
                    for dc in range(ND):
                        nc.tensor.matmul(
                            pqc[:P, :P], wqT[dc][:, isl], xcT[dc][:],
                            start=(dc == 0), stop=(dc == ND - 1),
                        )
                    nc.vector.tensor_copy(QcT[ic][:], pqc[:P, :P])

                gm = cnd.tile([P, L], u8, tag="gm")
                nc.gpsimd.indirect_dma_start(
                    out=gm[:], out_offset=None, in_=mask_d[:],
                    in_offset=bass.IndirectOffsetOnAxis(ap=candq_i[:, :1], axis=0),
                )
                gc = cnd.tile([P, L], u8, tag="gc")
                nc.gpsimd.indirect_dma_start(
                    out=gc[:], out_offset=None, in_=cnt_d[:],
                    in_offset=bass.IndirectOffsetOnAxis(ap=candq_i[:, :1], axis=0),
                )

                psS = []
                cmax = cnd.tile([P, NJ], f32, tag="cmax")
                csum = cnd.tile([P, NJ], f32, tag="csum")
                for jb in range(NJ):
                    jsl = slice(jb * 512, (jb + 1) * 512)
                    pss2 = ps_s.tile([P, 512], f32, tag="psSc")
                    psS.append(pss2)
                    for ic in range(ND):
                        nc.tensor.matmul(
                            pss2[:], QcT[ic][:], KT[ic][:, jsl],
                            start=(ic == 0), stop=(ic == ND - 1),
                        )
                    s3 = scr.tile([P, 512], f32, tag="scrt")
                    nc.vector.tensor_tensor(
                        out=s3[:], in0=pss2[:], in1=gm[:, jsl], op=OP.mult
                    )
                    nc.vector.reduce_max(cmax[:, jb : jb + 1], s3[:], axis=AX.X)
                    s4 = scr.tile([P, 512], f32, tag="scrt")
                    nc.vector.scalar_tensor_tensor(
                        out=s4[:], in0=pss2[:], scalar=-1.0 / L, in1=gc[:, jsl],
                        op0=OP.mult, op1=OP.mult,
                        accum_out=csum[:, jb : jb + 1],
                    )
                u1 = cnd.tile([P, 1], f32, tag="u1")
                u2 = cnd.tile([P, 1], f32, tag="u2")
                M_cand = cnd.tile([P, 1], f32, tag="M_cand")
                nc.vector.reduce_max(u1[:], cmax[:], axis=AX.X)
                nc.vector.reduce_sum(u2[:], csum[:], axis=AX.X)
                nc.vector.tensor_tensor(out=M_cand[:], in0=u1[:], in1=u2[:], op=OP.add)
                nc.vector.copy_predicated(M_cand[:], invalid[:], negbig[:])

                # exact top-40 threshold among candidates
                pmc = ps.tile([1, P], f32, tag="blk")
                nc.tensor.transpose(pmc[:1, :P], M_cand[:], ident[:])
                mcT = cnd.tile([1, P], f32, tag="mcT")
                nc.vector.tensor_copy(mcT[:], pmc[:1, :P])
                etop = cnd.tile([1, NT], f32, tag="etop")
                for r in range(5):
                    nc.vector.max(out=etop[:, 8 * r : 8 * r + 8], in_=mcT[:])
                    if r < 4:
                        nc.vector.match_replace(
                            out=mcT[:], in_to_replace=etop[:, 8 * r : 8 * r + 8],
                            in_values=mcT[:], imm_value=NEG,
                        )
                pte = ps.tile([P, 1], f32, tag="blk")
                nc.tensor.matmul(
                    pte[:P, :1], ones_r1[:], etop[:, NT - 1 : NT], start=True, stop=True
                )
                tebc = cnd.tile([P, 1], f32, tag="tebc")
                nc.vector.tensor_copy(tebc[:], pte[:P, :1])
                sel2 = cnd.tile([P, 1], u8, tag="sel2")
                nc.vector.tensor_tensor(
                    out=sel2[:], in0=M_cand[:], in1=tebc[:], op=OP.is_ge
                )
                scat_f = cnd.tile([P, 1], f32, tag="scat_f")
                nc.vector.tensor_copy(scat_f[:], big9[:])
                nc.vector.copy_predicated(scat_f[:], sel2[:], candq_f[:])
                scat_i = cnd.tile([P, 1], i32, tag="scat_i")
                nc.vector.tensor_copy(scat_i[:], scat_f[:])

            # xTh/xTl/weights freed here
            with tc.tile_pool(name="expp", bufs=1) as expp:
                # ---------------- phase 4b: softmax + update ----------------
                # no max-subtraction: scores*SCALE is O(10), exp is fp32-safe
                exp_sb = expp.tile([P, L], f32, tag="exp_sb")
                sume4 = expp.tile([P, NJ], f32, tag="sume4")
                for jb in range(NJ):
                    jsl = slice(jb * 512, (jb + 1) * 512)
                    nc.scalar.activation(
                        out=exp_sb[:, jsl], in_=psS[jb][:], func=ACTF.Exp,
                        bias=0.0, scale=SCALE,
                        accum_out=sume4[:, jb : jb + 1],
                    )
                sume = expp.tile([P, 1], f32, tag="sume")
                nc.vector.reduce_sum(sume[:], sume4[:], axis=AX.X)
                recip = expp.tile([P, 1], f32, tag="recip")
                nc.vector.reciprocal(recip[:], sume[:])

                expT = [expp.tile([P, P], bf16, tag=f"expT{jc}", name=f"expT{jc}") for jc in range(NL)]
                for jc in range(NL):
                    pet = ps.tile([P, P], f32, tag="blk")
                    nc.tensor.transpose(
                        pet[:P, :P], exp_sb[:, jc * P : (jc + 1) * P], ident[:]
                    )
                    nc.vector.tensor_copy(expT[jc][:], pet[:P, :P])

                pu = ps.tile([P, 512], f32, tag="blk")
                for jc in range(NL):
                    nc.tensor.matmul(
                        pu[:], expT[jc][:], Vb[jc][:],
                        start=(jc == 0), stop=(jc == NL - 1),
                    )
                upd = expp.tile([P, D], f32, tag="upd")
                nc.scalar.activation(
                    out=upd[:], in_=pu[:], func=ACTF.Copy, bias=0.0, scale=recip[:]
                )
                nc.gpsimd.indirect_dma_start(
                    out=ctx_d[:],
                    out_offset=bass.IndirectOffsetOnAxis(ap=scat_i[:, :1], axis=0),
                    in_=upd[:], in_offset=None,
                    bounds_check=L - 1, oob_is_err=False,
                )

    nc.compile()
    return nc


_NC = None


def _get_nc():
    global _NC
    if _NC is None:
        _NC = build()
    return _NC


def _split_bf16(a):
    hi = a.astype(ml_dtypes.bfloat16)
    lo = (a - hi.astype(np.float32)).astype(ml_dtypes.bfloat16)
    return hi, lo


def _host_prep(x, Wq, Wk, Wv, index_sample):
    x = np.asarray(x, dtype=np.float32)
    Wq = np.asarray(Wq, dtype=np.float32)
    Wk = np.asarray(Wk, dtype=np.float32)
    Wv = np.asarray(Wv, dtype=np.float32)
    idx = np.asarray(index_sample)

    wqT = np.ascontiguousarray(Wq.T)
    wqh, _ = _split_bf16(wqT)
    wkh, wkl = _split_bf16(np.ascontiguousarray(Wk.T))
    wvh, wvl = _split_bf16(np.ascontiguousarray(Wv.T))

    rows = np.arange(L)[:, None]
    mask01 = np.zeros((L, L), dtype=np.uint8)
    mask01[rows, idx] = 1
    countf = np.zeros((L, L), dtype=np.uint8)
    np.add.at(countf, (rows, idx), 1)

    perm16 = np.zeros((16, 8 * P), dtype=np.float32)
    for f in range(8):
        for p in range(16):
            perm16[p, f * P + p + 16 * f] = 1.0
    identf = np.eye(P, dtype=np.float32)
    qidxf = (np.arange(P, dtype=np.float32)[:, None]
             + 128.0 * np.arange(16, dtype=np.float32)[None, :])
    shared = {
        "wqTh": wqh, "wkTh": wkh, "wkTl": wkl, "wvTh": wvh, "wvTl": wvl,
        "wqT": wqT, "mask01": mask01, "countf": countf, "perm16": perm16,
        "identf": identf, "qidxf": qidxf,
    }
    in_maps = []
    for b in range(B):
        xb = np.ascontiguousarray(x[b])
        xT = np.ascontiguousarray(xb.T)
        xth, xtl = _split_bf16(xT)
        xmean = (xb.astype(np.float64).mean(axis=0) / 1.0).astype(np.float32)
        xmeh, xmel = _split_bf16(xmean.reshape(D, 1))
        in_maps.append(
            {
                "x_nat": xb,
                "xTh": xth,
                "xTl": xtl,
                "xmeanTh": xmeh,
                "xmeanTl": xmel,
                **shared,
            }
        )
    return in_maps


def kernel(x, Wq, Wk, Wv, index_sample, _trace=False, _result_box=None):
    in_maps = _host_prep(x, Wq, Wk, Wv, index_sample)
    nc = _get_nc()
    res = run_bass_kernel_spmd(nc, in_maps, core_ids=list(range(B)), trace=_trace)
    if _result_box is not None:
        _result_box.append(res)
    out = np.stack([np.asarray(res.results[b]["ctx"]) for b in range(B)], axis=0)
    return out



# revision 26
# speedup vs baseline: 1.4290x; 1.1711x over previous
"""Sparse attention (ProbSparse-style) Trainium2 Bass kernel.

Problem (per batch element b, data-parallel over 8 NeuronCores):
  Q = x @ Wq.T ; K = x @ Wk.T ; V = x @ Wv.T            [L=2048, D=512]
  QK_sample[l,s] = Q[l] . K[index_sample[l,s]]           [L, 40]
  M[l] = max_s QK_sample - sum_s QK_sample / L
  sel = top40(M)  (as a set; the reference scatter makes order irrelevant)
  scores = Q[sel] @ K.T / sqrt(D); attn = softmax(scores)
  ctx = broadcast(mean(V)); ctx[sel] = attn @ V

Numerics strategy (top-40 boundary gaps are as small as 0.02 in M):
  - K and V are computed with a 3-term bf16x2 split matmul
    (xh*wh + xl*wh + xh*wl, host-split halves) -> ~1e-5 absolute error,
    fp32-class, at full bf16 PE rate.
  - Approx M for ALL rows uses bf16 Q and bf16 K (error sigma ~0.2),
    extracted from per-chunk S = Q K^T PSUM blocks with fused
    tensor_tensor_reduce against a shipped u8 sample mask
    (multiply-mask max is safe: sampled max > 0 w.p. 1-2^-40;
    dup-count correction is deferred to the exact stage).
  - Candidates = { M_approx >= approx-top40 - DELTA }, DELTA=1.5 covers
    ~8 sigma; measured rank-40 to rank-64 M gap is 2.5-4.8 so the
    candidate count stays well under the 128-slot budget.
  - Exact stage on <= 128 candidate rows: gather x rows from DRAM
    (indirect DMA), exact fp32 Q_cand, exact S_cand vs the fp32-class K,
    TTR with gathered u8 mask+count rows -> exact M_cand -> exact top-40
    threshold -> softmax over S_cand -> upd = attn @ V -> indirect
    scatter of the 40 selected rows into ctx (bounds_check skips the
    rest).

kernel(**inputs) accepts the FULL inputs and returns the FULL
[8, 2048, 512] f32 output; batch is sharded over 8 cores.
"""

import math

import numpy as np
import ml_dtypes

import concourse.bacc as bacc
import concourse.bass as bass
import concourse.mybir as mybir
import concourse.tile as tile
from concourse.bass_utils import run_bass_kernel_spmd

P = 128
L = 2048
D = 512
B = 8
NL = L // P        # 16 query chunks
ND = D // P        # 4 feature chunks
NJ = L // 512      # 4 key blocks of 512
NT = 40
SCALE = 1.0 / math.sqrt(D)
# candidate band below approx T40: covers 2x bf16 dot error (~8 sigma =
# 1.5) plus the omitted -sum_s/L term in approx M (|sum/L| <= ~0.25
# per row at 3.5 sigma, both directions -> +0.5)
DELTA = 2.2
NEG = -3.0e38
SKIP_IDX = 99999.0  # scatter index sentinel (> bounds_check -> row skipped)

f32 = mybir.dt.float32
bf16 = mybir.dt.bfloat16
u8 = mybir.dt.uint8
i32 = mybir.dt.int32
u32 = mybir.dt.uint32
AX = mybir.AxisListType
OP = mybir.AluOpType
ACTF = mybir.ActivationFunctionType


def build():
    nc = bacc.Bacc("TRN2", target_bir_lowering=False)

    x_d = nc.dram_tensor("x_nat", [L, D], f32, kind="ExternalInput")
    xth_d = nc.dram_tensor("xTh", [D, L], bf16, kind="ExternalInput")
    xtl_d = nc.dram_tensor("xTl", [D, L], bf16, kind="ExternalInput")
    # all bf16 weight tiles + xmean hi/lo packed into one wide row-major
    # tensor: [wqh|wkh|wkl|wvh|wvl] each 4x512 cols, then xmh(4), xml(4)
    wcat_d = nc.dram_tensor("wcat", [P, 20 * 512 + 8], bf16, kind="ExternalInput")
    wq_d = nc.dram_tensor("wqTr", [P, 4 * D], f32, kind="ExternalInput")
    maskb_d = nc.dram_tensor("maskb", [L, L], bf16, kind="ExternalInput")
    ident_d = nc.dram_tensor("identf", [P, P], f32, kind="ExternalInput")
    qidx_d = nc.dram_tensor("qidxf", [P, 16], f32, kind="ExternalInput")
    perm_d = nc.dram_tensor("perm16", [16, 8 * P], f32, kind="ExternalInput")
    mask_d = nc.dram_tensor("mask01", [L, L], u8, kind="ExternalInput")
    cnt_d = nc.dram_tensor("countf", [L, L], u8, kind="ExternalInput")
    ctx_d = nc.dram_tensor("ctx", [L, D], f32, kind="ExternalOutput")

    with tile.TileContext(nc) as tc:
        with (
            tc.tile_pool(name="const", bufs=1) as cst,
            tc.tile_pool(name="proj", bufs=1) as proj,       # KT/KTb/QTb/V resident
            tc.tile_pool(name="mstuff", bufs=1) as mst,      # M / topk / sel smalls
            tc.tile_pool(name="mstream", bufs=3) as mstr,    # mask chunks
            tc.tile_pool(name="scr", bufs=3) as scr,         # TTR scratch
            tc.tile_pool(name="acc", bufs=2) as accp,        # per-chunk accums
            tc.tile_pool(name="cand", bufs=1) as cnd,        # exact-stage tiles
            tc.tile_pool(name="ps", bufs=3, space="PSUM") as ps,
            tc.tile_pool(name="ps_s", bufs=4, space="PSUM") as ps_s,  # S_cand (held)
            tc.tile_pool(name="dram", bufs=1, space="DRAM") as drp,
        ):
            # ---------------- constants ----------------
            # sparse_gather is the only library-tracked GPSIMD op left;
            # preload its (small) library before the serial tail
            from concourse import library_config
            nc.gpsimd.load_library(library_config.sparse_gather)
            ident = cst.tile([P, P], f32, tag="ident")
            nc.sync.dma_start(ident[:], ident_d[:])
            ones_r1 = cst.tile([1, P], f32, tag="ones_r1")
            nc.vector.memset(ones_r1[:], 1.0)
            negone = cst.tile([P, 1], f32, tag="negone")
            nc.vector.memset(negone[:], -1.0)
            negbig = cst.tile([P, 1], f32, tag="negbig")
            nc.vector.memset(negbig[:], NEG)
            big9 = cst.tile([P, 1], f32, tag="big9")
            nc.vector.memset(big9[:], SKIP_IDX)
            perm16 = cst.tile([16, 8 * P], f32, tag="perm16")
            nc.sync.dma_start(perm16[:], perm_d[:])
            qidx_f = cst.tile([P, 16], f32, tag="qidx_f")    # value p + 128*c
            nc.sync.dma_start(qidx_f[:], qidx_d[:])

            # resident projection outputs
            KT = [proj.tile([P, L], f32, tag=f"KT{ic}", name=f"KT{ic}") for ic in range(ND)]
            KTb = [proj.tile([P, L], bf16, tag=f"KTb{ic}", name=f"KTb{ic}") for ic in range(ND)]
            QTb = [proj.tile([P, L], bf16, tag=f"QTb{ic}", name=f"QTb{ic}") for ic in range(ND)]
            Vb = [proj.tile([P, D], bf16, tag=f"Vb{jc}", name=f"Vb{jc}") for jc in range(NL)]

            with tc.tile_pool(name="xw", bufs=1) as xw:
                # ---------------- phase 0: loads ----------------
                xTh = [xw.tile([P, L], bf16, tag=f"xTh{dc}", name=f"xTh{dc}") for dc in range(ND)]
                xTl = [xw.tile([P, L], bf16, tag=f"xTl{dc}", name=f"xTl{dc}") for dc in range(ND)]
                wcat = xw.tile([P, 20 * 512 + 8], bf16, tag="wcat")
                wqTr = xw.tile([P, 4 * D], f32, tag="wqTr")
                # weight-tile views into the packed wcat
                def wview(group, dc):
                    off = group * 4 * 512 + dc * 512
                    return wcat[:, off : off + 512]
                wqh = [wview(0, dc) for dc in range(ND)]
                wkh = [wview(1, dc) for dc in range(ND)]
                wkl = [wview(2, dc) for dc in range(ND)]
                wvh = [wview(3, dc) for dc in range(ND)]
                wvl = [wview(4, dc) for dc in range(ND)]
                xmh = [wcat[:, 20 * 512 + dc : 20 * 512 + dc + 1] for dc in range(ND)]
                xml = [wcat[:, 20 * 512 + 4 + dc : 20 * 512 + 5 + dc] for dc in range(ND)]
                wqT = [wqTr[:, dc * 512 : (dc + 1) * 512] for dc in range(ND)]
                # DMA order: wqh + xTh first (Q projection starts earliest),
                # then wk/x-lo for K, then wv/xmean, then the f32 wqT (tail
                # only). 1024-col chunks = 2KB rows, spread across queues.
                nc.sync.dma_start(wcat[:, 0:1024], wcat_d[:, 0:1024])
                nc.sync.dma_start(wcat[:, 1024:2048], wcat_d[:, 1024:2048])
                for dc in range(ND):
                    sl = slice(dc * P, (dc + 1) * P)
                    nc.sync.dma_start(xTh[dc][:, 0:1024], xth_d[sl, 0:1024])
                    nc.sync.dma_start(xTh[dc][:, 1024:2048], xth_d[sl, 1024:2048])
                for c0 in range(2048, 6144, 1024):       # wkh, wkl
                    nc.sync.dma_start(wcat[:, c0 : c0 + 1024], wcat_d[:, c0 : c0 + 1024])
                for dc in range(ND):
                    sl = slice(dc * P, (dc + 1) * P)
                    nc.sync.dma_start(xTl[dc][:, 0:1024], xtl_d[sl, 0:1024])
                    nc.sync.dma_start(xTl[dc][:, 1024:2048], xtl_d[sl, 1024:2048])
                for c0 in range(6144, 10240, 1024):      # wvh, wvl
                    nc.sync.dma_start(wcat[:, c0 : c0 + 1024], wcat_d[:, c0 : c0 + 1024])
                nc.sync.dma_start(wcat[:, 10240:10248], wcat_d[:, 10240:10248])
                nc.sync.dma_start(wqTr[:, 0:1024], wq_d[:, 0:1024])
                nc.sync.dma_start(wqTr[:, 1024:2048], wq_d[:, 1024:2048])

                # ---------------- phase 1: projections ----------------
                # Q first (single bf16 term, needs only wqh+xTh), jb-major;
                # then K 3-term jb-major so S blocks can start after K jb=0.
                for jb in range(NJ):
                    jsl = slice(jb * 512, (jb + 1) * 512)
                    for ic in range(ND):
                        isl = slice(ic * P, (ic + 1) * P)
                        pq = ps.tile([P, 512], f32, tag="blk")
                        for dc in range(ND):
                            nc.tensor.matmul(
                                pq[:], wqh[dc][:, isl], xTh[dc][:, jsl],
                                start=(dc == 0), stop=(dc == ND - 1),
                            )
                        nc.scalar.copy(QTb[ic][:, jsl], pq[:])
                # K: 3-term bf16x2 (fp32-class), into KT f32 + KTb bf16
                for jb in range(NJ):
                    jsl = slice(jb * 512, (jb + 1) * 512)
                    for ic in range(ND):
                        isl = slice(ic * P, (ic + 1) * P)
                        pk = ps.tile([P, 512], f32, tag="blk")
                        n = 0
                        for dc in range(ND):
                            for lh, rh in (
                                (wkh[dc][:, isl], xTh[dc][:, jsl]),
                                (wkh[dc][:, isl], xTl[dc][:, jsl]),
                                (wkl[dc][:, isl], xTh[dc][:, jsl]),
                            ):
                                nc.tensor.matmul(
                                    pk[:], lh, rh,
                                    start=(n == 0), stop=(n == 3 * ND - 1),
                                )
                                n += 1
                        nc.scalar.copy(KT[ic][:, jsl], pk[:])
                        nc.vector.tensor_copy(KTb[ic][:, jsl], pk[:])

                # ---------------- phase 2: approx M (bf16 S) ----------------
                # per (lc, jb) block: ONE fused TTR (masked product -> bf16
                # scratch, fused max accum). The -sum_s/L term is omitted in
                # the approx M (absorbed into DELTA); the exact stage still
                # uses the full formula.
                M_all = mst.tile([P, 16], f32, tag="M_all")
                amax_all = mst.tile([P, NL * NJ], f32, tag="amax_all")
                for lc in range(NL):
                    lsl = slice(lc * P, (lc + 1) * P)
                    mk = mstr.tile([P, L], bf16, tag="mk")
                    nc.sync.dma_start(mk[:], maskb_d[lsl, :])
                    for jb in range(NJ):
                        jsl = slice(jb * 512, (jb + 1) * 512)
                        k = lc * NJ + jb
                        pss = ps_s.tile([P, 512], f32, tag="psSc", name="pssa")
                        for ic in range(ND):
                            nc.tensor.matmul(
                                pss[:], QTb[ic][:, lsl], KTb[ic][:, jsl],
                                start=(ic == 0), stop=(ic == ND - 1),
                            )
                        # scalar engine drains PSUM to bf16 SBUF so the DVE
                        # mask-multiply runs in 2x packed mode (both operands
                        # bf16 SBUF); reduce_max is 1x regardless.
                        s0 = scr.tile([P, 512], bf16, tag="s0t")
                        nc.scalar.copy(s0[:], pss[:])
                        s1 = scr.tile([P, 512], bf16, tag="scrt")
                        nc.vector.tensor_tensor(
                            out=s1[:], in0=s0[:], in1=mk[:, jsl], op=OP.mult
                        )
                        nc.vector.reduce_max(
                            amax_all[:, k : k + 1], s1[:], axis=AX.X
                        )
                nc.vector.reduce_max(
                    M_all[:], amax_all[:].rearrange("p (c j) -> p c j", j=NJ),
                    axis=AX.X,
                )

                # ---------------- phase 3: approx top-40 -> candidates ------
                # exact T40 of M_approx without GPSIMD kth_largest (its attn
                # library reload cost ~50us of dead time on the serial tail):
                # per-chunk top-16 via vector max8/match_replace on M^T
                # [16,128], union (256 vals) holds the global top-40 w.p.
                # 1-2e-8, pack union into one [1,256] row via one-hot matmul
                # unwrap + transposes, then 5 rounds max8/match_replace.
                pmt = ps.tile([16, P], f32, tag="blk", name="pmt")
                nc.tensor.transpose(pmt[:16, :P], M_all[:], ident[:])
                MT = mst.tile([16, P], f32, tag="MT")
                nc.vector.tensor_copy(MT[:], pmt[:16, :P])
                w16 = mst.tile([16, 16], f32, tag="w16")
                nc.vector.max(out=w16[:, 0:8], in_=MT[:])
                nc.vector.match_replace(
                    out=MT[:], in_to_replace=w16[:, 0:8],
                    in_values=MT[:], imm_value=NEG,
                )
                nc.vector.max(out=w16[:, 8:16], in_=MT[:])

                # V projection (single bf16 term: the upd matmul consumes
                # bf16 anyway) is interleaved through the threshold/compaction
                # chain in 4-chunk groups so PE stays busy while the vector
                # engine and GPSIMD work through the serial tail.
                def v_chunks(lo, hi):
                    for lc in range(lo, hi):
                        lsl = slice(lc * P, (lc + 1) * P)
                        pv = ps.tile([P, 512], f32, tag="blk")
                        for dc in range(ND):
                            nc.tensor.matmul(
                                pv[:], xTh[dc][:, lsl], wvh[dc][:],
                                start=(dc == 0), stop=(dc == ND - 1),
                            )
                        nc.scalar.copy(Vb[lc][:], pv[:])

                v_chunks(0, 4)

                # unwrap w16 [16,16] -> two [128,1] columns (one-hot matmuls),
                # then -> [1,256] row via two PE transposes
                pcu = ps.tile([P, 2], f32, tag="blk", name="pcu")
                for f in range(8):
                    nc.tensor.matmul(
                        pcu[:P, 0:1], perm16[:, f * P : (f + 1) * P],
                        w16[:, f : f + 1],
                        start=(f == 0), stop=(f == 7),
                    )
                for f in range(8):
                    nc.tensor.matmul(
                        pcu[:P, 1:2], perm16[:, f * P : (f + 1) * P],
                        w16[:, 8 + f : 9 + f],
                        start=(f == 0), stop=(f == 7),
                    )
                crow = mst.tile([P, 2], f32, tag="crow")
                nc.vector.tensor_copy(crow[:], pcu[:P, :2])
                pr1 = ps.tile([1, P], f32, tag="blk", name="pr1")
                nc.tensor.transpose(pr1[:1, :P], crow[:, 0:1], ident[:])
                wrow = mst.tile([1, 2 * P], f32, tag="wrow")
                nc.vector.tensor_copy(wrow[:, 0:P], pr1[:1, :P])
                pr2 = ps.tile([1, P], f32, tag="blk", name="pr2")
                nc.tensor.transpose(pr2[:1, :P], crow[:, 1:2], ident[:])
                nc.vector.tensor_copy(wrow[:, P : 2 * P], pr2[:1, :P])

                v_chunks(4, 8)

                etop40 = mst.tile([1, NT], f32, tag="etop40")
                for r in range(5):
                    nc.vector.max(out=etop40[:, 8 * r : 8 * r + 8], in_=wrow[:])
                    if r < 4:
                        nc.vector.match_replace(
                            out=wrow[:], in_to_replace=etop40[:, 8 * r : 8 * r + 8],
                            in_values=wrow[:], imm_value=NEG,
                        )
                ptb = ps.tile([P, 1], f32, tag="blk")
                nc.tensor.matmul(
                    ptb[:P, :1], ones_r1[:], etop40[:, NT - 1 : NT],
                    start=True, stop=True,
                )
                tbc = mst.tile([P, 1], f32, tag="tbc")
                nc.vector.tensor_copy(tbc[:], ptb[:P, :1])

                # selmask = (M - T40) >= -DELTA, one fused op
                selmask = mst.tile([P, 16], u8, tag="selmask")
                nc.vector.tensor_scalar(
                    selmask[:], M_all[:], tbc[:], -DELTA,
                    op0=OP.subtract, op1=OP.is_ge,
                )
                midx = mst.tile([P, 16], f32, tag="midx")
                nc.vector.tensor_copy(midx[:], negone[:].to_broadcast([P, 16]))
                nc.vector.copy_predicated(midx[:], selmask[:], qidx_f[:])

                pwr = ps.tile([16, P], f32, tag="blk", name="pwr")
                nc.tensor.transpose(pwr[:16, :P], midx[:], ident[:])
                wrap_in = mst.tile([16, P], f32, tag="wrap_in")
                nc.vector.tensor_copy(wrap_in[:], pwr[:16, :P])
                spg = mst.tile([16, 8], f32, tag="spg")
                nfound = mst.tile([1, 1], u32, tag="nfound")
                nc.gpsimd.sparse_gather(out=spg[:], in_=wrap_in[:], num_found=nfound[:])

                v_chunks(8, 12)

                spg_cl = mst.tile([16, 8], f32, tag="spg_cl")
                nc.vector.tensor_scalar_max(spg_cl[:], spg[:], 0.0)
                nc.vector.tensor_scalar_min(spg_cl[:], spg_cl[:], float(L - 1))

                # unwrap [16,8] -> [128,1] with 8 tiny one-hot matmuls
                # (perm16[p, f*128+u] = 1 iff u == p + 16*f, shipped constant)
                pcq = ps.tile([P, 1], f32, tag="blk", name="pcq")
                for f in range(8):
                    nc.tensor.matmul(
                        pcq[:P, :1], perm16[:, f * P : (f + 1) * P],
                        spg_cl[:, f : f + 1],
                        start=(f == 0), stop=(f == 7),
                    )
                candq_f = mst.tile([P, 1], f32, tag="candq_f")
                nc.vector.tensor_copy(candq_f[:], pcq[:P, :1])
                candq_i = mst.tile([P, 1], i32, tag="candq_i")
                nc.vector.tensor_copy(candq_i[:], pcq[:P, :1])

                nf_f = mst.tile([1, 1], f32, tag="nf_f")
                nc.vector.tensor_copy(nf_f[:], nfound[:])
                pnb = ps.tile([P, 1], f32, tag="blk")
                nc.tensor.matmul(pnb[:P, :1], ones_r1[:], nf_f[:], start=True, stop=True)
                nbc = mst.tile([P, 1], f32, tag="nbc")
                nc.vector.tensor_copy(nbc[:], pnb[:P, :1])
                invalid = mst.tile([P, 1], u8, tag="invalid")
                nc.vector.tensor_tensor(
                    out=invalid[:], in0=qidx_f[:, 0:1], in1=nbc[:], op=OP.is_ge
                )

                v_chunks(12, 16)

                # Vmean = xmean.T @ Wv.T via bf16x2 3-term, broadcast, ctx init
                pvm = ps.tile([1, 512], f32, tag="blk")
                n = 0
                for dc in range(ND):
                    for lh, rh in (
                        (xmh[dc][:], wvh[dc][:]),
                        (xml[dc][:], wvh[dc][:]),
                        (xmh[dc][:], wvl[dc][:]),
                    ):
                        nc.tensor.matmul(
                            pvm[:1, :], lh, rh,
                            start=(n == 0), stop=(n == 3 * ND - 1),
                        )
                        n += 1
                vmean = mst.tile([1, 512], f32, tag="vmean")
                nc.scalar.copy(vmean[:], pvm[:1, :])
                pvb = ps.tile([P, 512], f32, tag="blk")
                nc.tensor.matmul(pvb[:], ones_r1[:], vmean[:], start=True, stop=True)
                vmean_bc = mst.tile([P, 512], f32, tag="vmean_bc")
                nc.vector.tensor_copy(vmean_bc[:], pvb[:])
                for jc in range(NL):
                    nc.sync.dma_start(ctx_d[jc * P : (jc + 1) * P, :], vmean_bc[:])

                # ---------------- phase 4a: exact candidates ----------------
                x_cand = cnd.tile([P, D], f32, tag="x_cand")
                nc.gpsimd.indirect_dma_start(
                    out=x_cand[:], out_offset=None, in_=x_d[:],
                    in_offset=bass.IndirectOffsetOnAxis(ap=candq_i[:, :1], axis=0),
                )
                xcT = [cnd.tile([P, P], f32, tag=f"xcT{dc}", name=f"xcT{dc}") for dc in range(ND)]
                for dc in range(ND):
                    pxc = ps.tile([P, P], f32, tag="blk")
                    nc.tensor.transpose(
                        pxc[:P, :P], x_cand[:, dc * P : (dc + 1) * P], ident[:]
                    )
                    nc.vector.tensor_copy(xcT[dc][:], pxc[:P, :P])

                QcT = [cnd.tile([P, P], f32, tag=f"QcT{ic}", name=f"QcT{ic}") for ic in range(ND)]
                for ic in range(ND):
                    isl = slice(ic * P, (ic + 1) * P)
                    pqc = ps.tile([P, P], f32, tag="blk")
                    for dc in range(ND):
                        nc.tensor.matmul(
                            pqc[:P, :P], wqT[dc][:, isl], xcT[dc][:],
                            start=(dc == 0), stop=(dc == ND - 1),
                        )
                    nc.vector.tensor_copy(QcT[ic][:], pqc[:P, :P])

                gm = cnd.tile([P, L], u8, tag="gm")
                nc.gpsimd.indirect_dma_start(
                    out=gm[:], out_offset=None, in_=mask_d[:],
                    in_offset=bass.IndirectOffsetOnAxis(ap=candq_i[:, :1], axis=0),
                )
                gc = cnd.tile([P, L], u8, tag="gc")
                nc.gpsimd.indirect_dma_start(
                    out=gc[:], out_offset=None, in_=cnt_d[:],
                    in_offset=bass.IndirectOffsetOnAxis(ap=candq_i[:, :1], axis=0),
                )

                psS = []
                cmax = cnd.tile([P, NJ], f32, tag="cmax")
                csum = cnd.tile([P, NJ], f32, tag="csum")
                for jb in range(NJ):
                    jsl = slice(jb * 512, (jb + 1) * 512)
                    pss2 = ps_s.tile([P, 512], f32, tag="psSc")
                    psS.append(pss2)
                    for ic in range(ND):
                        nc.tensor.matmul(
                            pss2[:], QcT[ic][:], KT[ic][:, jsl],
                            start=(ic == 0), stop=(ic == ND - 1),
                        )
                    s3 = scr.tile([P, 512], f32, tag="scrt")
                    nc.vector.tensor_tensor(
                        out=s3[:], in0=pss2[:], in1=gm[:, jsl], op=OP.mult
                    )
                    nc.vector.reduce_max(cmax[:, jb : jb + 1], s3[:], axis=AX.X)
                    s4 = scr.tile([P, 512], f32, tag="scrt")
                    nc.vector.scalar_tensor_tensor(
                        out=s4[:], in0=pss2[:], scalar=-1.0 / L, in1=gc[:, jsl],
                        op0=OP.mult, op1=OP.mult,
                        accum_out=csum[:, jb : jb + 1],
                    )
                u1 = cnd.tile([P, 1], f32, tag="u1")
                u2 = cnd.tile([P, 1], f32, tag="u2")
                M_cand = cnd.tile([P, 1], f32, tag="M_cand")
                nc.vector.reduce_max(u1[:], cmax[:], axis=AX.X)
                nc.vector.reduce_sum(u2[:], csum[:], axis=AX.X)
                nc.vector.tensor_tensor(out=M_cand[:], in0=u1[:], in1=u2[:], op=OP.add)
                nc.vector.copy_predicated(M_cand[:], invalid[:], negbig[:])

                # exact top-40 threshold among candidates
                pmc = ps.tile([1, P], f32, tag="blk")
                nc.tensor.transpose(pmc[:1, :P], M_cand[:], ident[:])
                mcT = cnd.tile([1, P], f32, tag="mcT")
                nc.vector.tensor_copy(mcT[:], pmc[:1, :P])
                etop = cnd.tile([1, NT], f32, tag="etop")
                for r in range(5):
                    nc.vector.max(out=etop[:, 8 * r : 8 * r + 8], in_=mcT[:])
                    if r < 4:
                        nc.vector.match_replace(
                            out=mcT[:], in_to_replace=etop[:, 8 * r : 8 * r + 8],
                            in_values=mcT[:], imm_value=NEG,
                        )
                pte = ps.tile([P, 1], f32, tag="blk")
                nc.tensor.matmul(
                    pte[:P, :1], ones_r1[:], etop[:, NT - 1 : NT], start=True, stop=True
                )
                tebc = cnd.tile([P, 1], f32, tag="tebc")
                nc.vector.tensor_copy(tebc[:], pte[:P, :1])
                sel2 = cnd.tile([P, 1], u8, tag="sel2")
                nc.vector.tensor_tensor(
                    out=sel2[:], in0=M_cand[:], in1=tebc[:], op=OP.is_ge
                )
                scat_f = cnd.tile([P, 1], f32, tag="scat_f")
                nc.vector.tensor_copy(scat_f[:], big9[:])
                nc.vector.copy_predicated(scat_f[:], sel2[:], candq_f[:])
                scat_i = cnd.tile([P, 1], i32, tag="scat_i")
                nc.vector.tensor_copy(scat_i[:], scat_f[:])

            # xTh/xTl/weights freed here
            with tc.tile_pool(name="expp", bufs=1) as expp:
                # ---------------- phase 4b: softmax + update ----------------
                # no max-subtraction: scores*SCALE is O(10), exp is fp32-safe
                exp_sb = expp.tile([P, L], f32, tag="exp_sb")
                sume4 = expp.tile([P, NJ], f32, tag="sume4")
                for jb in range(NJ):
                    jsl = slice(jb * 512, (jb + 1) * 512)
                    nc.scalar.activation(
                        out=exp_sb[:, jsl], in_=psS[jb][:], func=ACTF.Exp,
                        bias=0.0, scale=SCALE,
                        accum_out=sume4[:, jb : jb + 1],
                    )
                sume = expp.tile([P, 1], f32, tag="sume")
                nc.vector.reduce_sum(sume[:], sume4[:], axis=AX.X)
                recip = expp.tile([P, 1], f32, tag="recip")
                nc.vector.reciprocal(recip[:], sume[:])

                # expT transposes and upd accumulation pipelined: the upd
                # matmul for chunk jc-1 issues right after transpose jc, so
                # PE never waits for the vector copies to drain.
                expT = [expp.tile([P, P], bf16, tag=f"expT{jc}", name=f"expT{jc}") for jc in range(NL)]
                pu = ps.tile([P, 512], f32, tag="blk", name="pu")
                for jc in range(NL):
                    pet = ps.tile([P, P], f32, tag="blk")
                    nc.tensor.transpose(
                        pet[:P, :P], exp_sb[:, jc * P : (jc + 1) * P], ident[:]
                    )
                    nc.vector.tensor_copy(expT[jc][:], pet[:P, :P])
                    if jc >= 1:
                        nc.tensor.matmul(
                            pu[:], expT[jc - 1][:], Vb[jc - 1][:],
                            start=(jc == 1), stop=False,
                        )
                nc.tensor.matmul(
                    pu[:], expT[NL - 1][:], Vb[NL - 1][:],
                    start=False, stop=True,
                )
                upd = expp.tile([P, D], f32, tag="upd")
                nc.scalar.activation(
                    out=upd[:], in_=pu[:], func=ACTF.Copy, bias=0.0, scale=recip[:]
                )
                nc.gpsimd.indirect_dma_start(
                    out=ctx_d[:],
                    out_offset=bass.IndirectOffsetOnAxis(ap=scat_i[:, :1], axis=0),
                    in_=upd[:], in_offset=None,
                    bounds_check=L - 1, oob_is_err=False,
                )

    nc.compile()
    return nc


_NC = None


def _get_nc():
    global _NC
    if _NC is None:
        _NC = build()
    return _NC


def _split_bf16(a):
    hi = a.astype(ml_dtypes.bfloat16)
    lo = (a - hi.astype(np.float32)).astype(ml_dtypes.bfloat16)
    return hi, lo


def _row_chunk(w):
    # [512, 512] -> [128, 4*512]: tile dc = rows dc*128..dc*128+127
    return np.concatenate([w[dc * P : (dc + 1) * P, :] for dc in range(4)], axis=1)


def _host_prep(x, Wq, Wk, Wv, index_sample):
    x = np.asarray(x, dtype=np.float32)
    Wq = np.asarray(Wq, dtype=np.float32)
    Wk = np.asarray(Wk, dtype=np.float32)
    Wv = np.asarray(Wv, dtype=np.float32)
    idx = np.asarray(index_sample)

    wqT = np.ascontiguousarray(Wq.T)
    wqh, _ = _split_bf16(wqT)
    wkh, wkl = _split_bf16(np.ascontiguousarray(Wk.T))
    wvh, wvl = _split_bf16(np.ascontiguousarray(Wv.T))

    rows = np.arange(L)[:, None]
    mask01 = np.zeros((L, L), dtype=np.uint8)
    mask01[rows, idx] = 1
    maskb = mask01.astype(ml_dtypes.bfloat16)
    countf = np.zeros((L, L), dtype=np.uint8)
    np.add.at(countf, (rows, idx), 1)

    perm16 = np.zeros((16, 8 * P), dtype=np.float32)
    for f in range(8):
        for p in range(16):
            perm16[p, f * P + p + 16 * f] = 1.0
    identf = np.eye(P, dtype=np.float32)
    qidxf = (np.arange(P, dtype=np.float32)[:, None]
             + 128.0 * np.arange(16, dtype=np.float32)[None, :])
    wqTr = np.ascontiguousarray(_row_chunk(wqT))
    shared = {
        "wqTr": wqTr, "mask01": mask01, "maskb": maskb, "countf": countf,
        "perm16": perm16, "identf": identf, "qidxf": qidxf,
    }
    wpart = np.concatenate(
        [_row_chunk(wqh), _row_chunk(wkh), _row_chunk(wkl),
         _row_chunk(wvh), _row_chunk(wvl)],
        axis=1,
    )
    in_maps = []
    for b in range(B):
        xb = np.ascontiguousarray(x[b])
        xT = np.ascontiguousarray(xb.T)
        xth, xtl = _split_bf16(xT)
        xmean = (xb.astype(np.float64).mean(axis=0) / 1.0).astype(np.float32)
        xmeh, xmel = _split_bf16(xmean.reshape(D, 1))
        wcat = np.concatenate(
            [wpart, xmeh.reshape(4, P).T, xmel.reshape(4, P).T], axis=1
        ).astype(ml_dtypes.bfloat16)
        in_maps.append(
            {
                "x_nat": xb,
                "xTh": xth,
                "xTl": xtl,
                "wcat": np.ascontiguousarray(wcat),
                **shared,
            }
        )
    return in_maps


def kernel(x, Wq, Wk, Wv, index_sample, _trace=False, _result_box=None):
    in_maps = _host_prep(x, Wq, Wk, Wv, index_sample)
    nc = _get_nc()
    res = run_bass_kernel_spmd(nc, in_maps, core_ids=list(range(B)), trace=_trace)
    if _result_box is not None:
        _result_box.append(res)
    out = np.stack([np.asarray(res.results[b]["ctx"]) for b in range(B)], axis=0)
    return out



# revision 37
# speedup vs baseline: 1.6199x; 1.1336x over previous
"""Sparse attention (ProbSparse-style) Trainium2 Bass kernel.

Problem (per batch element b, data-parallel over 8 NeuronCores):
  Q = x @ Wq.T ; K = x @ Wk.T ; V = x @ Wv.T            [L=2048, D=512]
  QK_sample[l,s] = Q[l] . K[index_sample[l,s]]           [L, 40]
  M[l] = max_s QK_sample - sum_s QK_sample / L
  sel = top40(M)  (as a set; the reference scatter makes order irrelevant)
  scores = Q[sel] @ K.T / sqrt(D); attn = softmax(scores)
  ctx = broadcast(mean(V)); ctx[sel] = attn @ V

Numerics strategy (top-40 boundary gaps are as small as 0.02 in M):
  - K and V are computed with a 3-term bf16x2 split matmul
    (xh*wh + xl*wh + xh*wl, host-split halves) -> ~1e-5 absolute error,
    fp32-class, at full bf16 PE rate.
  - Approx M for ALL rows uses bf16 Q and bf16 K (error sigma ~0.2),
    extracted from per-chunk S = Q K^T PSUM blocks with fused
    tensor_tensor_reduce against a shipped u8 sample mask
    (multiply-mask max is safe: sampled max > 0 w.p. 1-2^-40;
    dup-count correction is deferred to the exact stage).
  - Candidates = { M_approx >= approx-top40 - DELTA }, DELTA=1.5 covers
    ~8 sigma; measured rank-40 to rank-64 M gap is 2.5-4.8 so the
    candidate count stays well under the 128-slot budget.
  - Exact stage on <= 128 candidate rows: gather x rows from DRAM
    (indirect DMA), exact fp32 Q_cand, exact S_cand vs the fp32-class K,
    TTR with gathered u8 mask+count rows -> exact M_cand -> exact top-40
    threshold -> softmax over S_cand -> upd = attn @ V -> indirect
    scatter of the 40 selected rows into ctx (bounds_check skips the
    rest).

kernel(**inputs) accepts the FULL inputs and returns the FULL
[8, 2048, 512] f32 output; batch is sharded over 8 cores.
"""

import math

import numpy as np
import ml_dtypes

import concourse.bacc as bacc
import concourse.bass as bass
import concourse.mybir as mybir
import concourse.tile as tile
from concourse.bass_utils import run_bass_kernel_spmd

P = 128
L = 2048
D = 512
B = 8
NL = L // P        # 16 query chunks
ND = D // P        # 4 feature chunks
NJ = L // 512      # 4 key blocks of 512
NT = 40
SCALE = 1.0 / math.sqrt(D)
# candidate band below approx T40: covers 2x bf16 dot error (~8 sigma =
# 1.5) plus the omitted -sum_s/L term in approx M (|sum/L| <= ~0.25
# per row at 3.5 sigma, both directions -> +0.5)
DELTA = 2.2
NEG = -3.0e38
SKIP_IDX = 99999.0  # scatter index sentinel (> bounds_check -> row skipped)

f32 = mybir.dt.float32
bf16 = mybir.dt.bfloat16
u8 = mybir.dt.uint8
i32 = mybir.dt.int32
u32 = mybir.dt.uint32
AX = mybir.AxisListType
OP = mybir.AluOpType
ACTF = mybir.ActivationFunctionType


def build():
    nc = bacc.Bacc("TRN2", target_bir_lowering=False)

    x_d = nc.dram_tensor("x_nat", [L, D], f32, kind="ExternalInput")
    xth_d = nc.dram_tensor("xTh", [D, L], bf16, kind="ExternalInput")
    xtl_d = nc.dram_tensor("xTl", [D, L], bf16, kind="ExternalInput")
    # all bf16 weight tiles + xmean hi/lo packed into one wide row-major
    # tensor: [wqh|wkh|wkl|wvh|wvl] each 4x512 cols, then xmh(4), xml(4)
    wcat_d = nc.dram_tensor("wcat", [P, 20 * 512 + 8], bf16, kind="ExternalInput")
    # A = Wq^T @ Wk (host f64): scores = x_cand @ A @ x^T, so no exact K
    # projection is ever needed on-device
    acat_d = nc.dram_tensor("Acat", [P, 4 * D], f32, kind="ExternalInput")
    maskb_d = nc.dram_tensor("maskb", [L, L], bf16, kind="ExternalInput")
    ident_d = nc.dram_tensor("identf", [P, P], f32, kind="ExternalInput")
    qidx_d = nc.dram_tensor("qidxf", [P, 16], f32, kind="ExternalInput")
    perm_d = nc.dram_tensor("perm16", [16, 8 * P], f32, kind="ExternalInput")
    mask_d = nc.dram_tensor("mask01", [L, L], u8, kind="ExternalInput")
    cnt_d = nc.dram_tensor("countf", [L, L], u8, kind="ExternalInput")
    ctx_d = nc.dram_tensor("ctx", [L, D], f32, kind="ExternalOutput")

    with tile.TileContext(nc) as tc:
        with (
            tc.tile_pool(name="const", bufs=1) as cst,
            tc.tile_pool(name="proj", bufs=1) as proj,       # KT/KTb/QTb/V resident
            tc.tile_pool(name="mstuff", bufs=1) as mst,      # M / topk / sel smalls
            tc.tile_pool(name="mstream", bufs=3) as mstr,    # mask chunks
            tc.tile_pool(name="scr", bufs=3) as scr,         # TTR scratch
            tc.tile_pool(name="acc", bufs=2) as accp,        # per-chunk accums
            tc.tile_pool(name="cand", bufs=1) as cnd,        # exact-stage tiles
            tc.tile_pool(name="ps", bufs=3, space="PSUM") as ps,
            tc.tile_pool(name="ps_s", bufs=4, space="PSUM") as ps_s,  # S_cand (held)
            tc.tile_pool(name="dram", bufs=1, space="DRAM") as drp,
        ):
            # ---------------- constants ----------------
            # sparse_gather is the only library-tracked GPSIMD op left;
            # preload its (small) library before the serial tail
            from concourse import library_config
            nc.gpsimd.load_library(library_config.sparse_gather)
            ident = cst.tile([P, P], f32, tag="ident")
            nc.sync.dma_start(ident[:], ident_d[:])
            ones_r1 = cst.tile([1, P], f32, tag="ones_r1")
            nc.vector.memset(ones_r1[:], 1.0)
            negone = cst.tile([P, 1], f32, tag="negone")
            nc.vector.memset(negone[:], -1.0)
            negbig = cst.tile([P, 1], f32, tag="negbig")
            nc.vector.memset(negbig[:], NEG)
            big9 = cst.tile([P, 1], f32, tag="big9")
            nc.vector.memset(big9[:], SKIP_IDX)
            perm16 = cst.tile([16, 8 * P], f32, tag="perm16")
            nc.sync.dma_start(perm16[:], perm_d[:])
            qidx_f = cst.tile([P, 16], f32, tag="qidx_f")    # value p + 128*c
            nc.sync.dma_start(qidx_f[:], qidx_d[:])

            # resident projection outputs
            KTb = [proj.tile([P, L], bf16, tag=f"KTb{ic}", name=f"KTb{ic}") for ic in range(ND)]
            QTb = [proj.tile([P, L], bf16, tag=f"QTb{ic}", name=f"QTb{ic}") for ic in range(ND)]
            Vb = [proj.tile([P, D], bf16, tag=f"Vb{jc}", name=f"Vb{jc}") for jc in range(NL)]

            with tc.tile_pool(name="xw", bufs=1) as xw:
                # ---------------- phase 0: loads ----------------
                xTh = [xw.tile([P, L], bf16, tag=f"xTh{dc}", name=f"xTh{dc}") for dc in range(ND)]
                xTl = [xw.tile([P, L], bf16, tag=f"xTl{dc}", name=f"xTl{dc}") for dc in range(ND)]
                wcat = xw.tile([P, 20 * 512 + 8], bf16, tag="wcat")
                acat = xw.tile([P, 4 * D], f32, tag="acat")
                # weight-tile views into the packed wcat
                def wview(group, dc):
                    off = group * 4 * 512 + dc * 512
                    return wcat[:, off : off + 512]
                wqh = [wview(0, dc) for dc in range(ND)]
                wkh = [wview(1, dc) for dc in range(ND)]
                wkl = [wview(2, dc) for dc in range(ND)]
                wvh = [wview(3, dc) for dc in range(ND)]
                wvl = [wview(4, dc) for dc in range(ND)]
                xmh = [wcat[:, 20 * 512 + dc : 20 * 512 + dc + 1] for dc in range(ND)]
                xml = [wcat[:, 20 * 512 + 4 + dc : 20 * 512 + 5 + dc] for dc in range(ND)]
                Arc = [acat[:, dc * 512 : (dc + 1) * 512] for dc in range(ND)]
                # DMA order: wqh + xTh first (Q projection starts earliest),
                # then wk/x-lo for K, then wv/xmean, then the f32 wqT (tail
                # only). 1024-col chunks = 2KB rows, spread across queues.
                nc.sync.dma_start(wcat[:, 0:1024], wcat_d[:, 0:1024])
                nc.sync.dma_start(wcat[:, 1024:2048], wcat_d[:, 1024:2048])
                for dc in range(ND):
                    sl = slice(dc * P, (dc + 1) * P)
                    nc.sync.dma_start(xTh[dc][:, 0:1024], xth_d[sl, 0:1024])
                    nc.sync.dma_start(xTh[dc][:, 1024:2048], xth_d[sl, 1024:2048])
                for c0 in range(2048, 4096, 1024):       # wkh (wkl unused)
                    nc.sync.dma_start(wcat[:, c0 : c0 + 1024], wcat_d[:, c0 : c0 + 1024])
                for dc in range(ND):
                    sl = slice(dc * P, (dc + 1) * P)
                    nc.sync.dma_start(xTl[dc][:, 0:1024], xtl_d[sl, 0:1024])
                    nc.sync.dma_start(xTl[dc][:, 1024:2048], xtl_d[sl, 1024:2048])
                for c0 in range(6144, 10240, 1024):      # wvh, wvl
                    nc.sync.dma_start(wcat[:, c0 : c0 + 1024], wcat_d[:, c0 : c0 + 1024])
                nc.sync.dma_start(wcat[:, 10240:10248], wcat_d[:, 10240:10248])
                nc.sync.dma_start(acat[:, 0:1024], acat_d[:, 0:1024])
                nc.sync.dma_start(acat[:, 1024:2048], acat_d[:, 1024:2048])

                # ---------------- phase 1: projections ----------------
                # Q first (single bf16 term, needs only wqh+xTh), jb-major;
                # then K 3-term jb-major so S blocks can start after K jb=0.
                for jb in range(NJ):
                    jsl = slice(jb * 512, (jb + 1) * 512)
                    for ic in range(ND):
                        isl = slice(ic * P, (ic + 1) * P)
                        pq = ps.tile([P, 512], f32, tag="blk")
                        for dc in range(ND):
                            nc.tensor.matmul(
                                pq[:], wqh[dc][:, isl], xTh[dc][:, jsl],
                                start=(dc == 0), stop=(dc == ND - 1),
                            )
                        nc.scalar.copy(QTb[ic][:, jsl], pq[:])
                # K approx: single bf16 term (the exact stage goes through
                # A = Wq^T Wk and never needs an exact K)
                for jb in range(NJ):
                    jsl = slice(jb * 512, (jb + 1) * 512)
                    for ic in range(ND):
                        isl = slice(ic * P, (ic + 1) * P)
                        pk = ps.tile([P, 512], f32, tag="blk")
                        for dc in range(ND):
                            nc.tensor.matmul(
                                pk[:], wkh[dc][:, isl], xTh[dc][:, jsl],
                                start=(dc == 0), stop=(dc == ND - 1),
                            )
                        nc.scalar.copy(KTb[ic][:, jsl], pk[:])

                # ---------------- phase 2: approx M (bf16 S) ----------------
                # per (lc, jb) block: ONE fused TTR (masked product -> bf16
                # scratch, fused max accum). The -sum_s/L term is omitted in
                # the approx M (absorbed into DELTA); the exact stage still
                # uses the full formula.
                M_all = mst.tile([P, 16], f32, tag="M_all")
                amax_all = mst.tile([P, NL * NJ], f32, tag="amax_all")
                for lc in range(NL):
                    lsl = slice(lc * P, (lc + 1) * P)
                    mk = mstr.tile([P, L], bf16, tag="mk")
                    nc.sync.dma_start(mk[:], maskb_d[lsl, :])
                    for jb in range(NJ):
                        jsl = slice(jb * 512, (jb + 1) * 512)
                        k = lc * NJ + jb
                        pss = ps_s.tile([P, 512], f32, tag="psSc", name="pssa")
                        for ic in range(ND):
                            nc.tensor.matmul(
                                pss[:], QTb[ic][:, lsl], KTb[ic][:, jsl],
                                start=(ic == 0), stop=(ic == ND - 1),
                            )
                        # scalar engine drains PSUM to bf16 SBUF so the DVE
                        # mask-multiply runs in 2x packed mode (both operands
                        # bf16 SBUF); reduce_max is 1x regardless.
                        s0 = scr.tile([P, 512], bf16, tag="s0t")
                        nc.scalar.copy(s0[:], pss[:])
                        s1 = scr.tile([P, 512], bf16, tag="scrt")
                        nc.vector.tensor_tensor(
                            out=s1[:], in0=s0[:], in1=mk[:, jsl], op=OP.mult
                        )
                        nc.vector.reduce_max(
                            amax_all[:, k : k + 1], s1[:], axis=AX.X
                        )
                nc.vector.reduce_max(
                    M_all[:], amax_all[:].rearrange("p (c j) -> p c j", j=NJ),
                    axis=AX.X,
                )

                # ---------------- phase 3: approx top-40 -> candidates ------
                # exact T40 of M_approx without GPSIMD kth_largest (its attn
                # library reload cost ~50us of dead time on the serial tail):
                # per-chunk top-16 via vector max8/match_replace on M^T
                # [16,128], union (256 vals) holds the global top-40 w.p.
                # 1-2e-8, pack union into one [1,256] row via one-hot matmul
                # unwrap + transposes, then 5 rounds max8/match_replace.
                pmt = ps.tile([16, P], f32, tag="blk", name="pmt")
                nc.tensor.transpose(pmt[:16, :P], M_all[:], ident[:])
                MT = mst.tile([16, P], f32, tag="MT")
                nc.vector.tensor_copy(MT[:], pmt[:16, :P])
                w16 = mst.tile([16, 16], f32, tag="w16")
                nc.vector.max(out=w16[:, 0:8], in_=MT[:])
                nc.vector.match_replace(
                    out=MT[:], in_to_replace=w16[:, 0:8],
                    in_values=MT[:], imm_value=NEG,
                )
                nc.vector.max(out=w16[:, 8:16], in_=MT[:])

                # V projection (single bf16 term: the upd matmul consumes
                # bf16 anyway) is interleaved through the threshold/compaction
                # chain in 4-chunk groups so PE stays busy while the vector
                # engine and GPSIMD work through the serial tail.
                def v_chunks(lo, hi):
                    for lc in range(lo, hi):
                        lsl = slice(lc * P, (lc + 1) * P)
                        pv = ps.tile([P, 512], f32, tag="blk")
                        for dc in range(ND):
                            nc.tensor.matmul(
                                pv[:], xTh[dc][:, lsl], wvh[dc][:],
                                start=(dc == 0), stop=(dc == ND - 1),
                            )
                        nc.scalar.copy(Vb[lc][:], pv[:])

                v_chunks(0, 4)

                # unwrap w16 [16,16] -> two [128,1] columns (one-hot matmuls),
                # then -> [1,256] row via two PE transposes
                pcu = ps.tile([P, 2], f32, tag="blk", name="pcu")
                for f in range(8):
                    nc.tensor.matmul(
                        pcu[:P, 0:1], perm16[:, f * P : (f + 1) * P],
                        w16[:, f : f + 1],
                        start=(f == 0), stop=(f == 7),
                    )
                for f in range(8):
                    nc.tensor.matmul(
                        pcu[:P, 1:2], perm16[:, f * P : (f + 1) * P],
                        w16[:, 8 + f : 9 + f],
                        start=(f == 0), stop=(f == 7),
                    )
                crow = mst.tile([P, 2], f32, tag="crow")
                nc.vector.tensor_copy(crow[:], pcu[:P, :2])
                pr1 = ps.tile([1, P], f32, tag="blk", name="pr1")
                nc.tensor.transpose(pr1[:1, :P], crow[:, 0:1], ident[:])
                wrow = mst.tile([1, 2 * P], f32, tag="wrow")
                nc.vector.tensor_copy(wrow[:, 0:P], pr1[:1, :P])
                pr2 = ps.tile([1, P], f32, tag="blk", name="pr2")
                nc.tensor.transpose(pr2[:1, :P], crow[:, 1:2], ident[:])
                nc.vector.tensor_copy(wrow[:, P : 2 * P], pr2[:1, :P])

                v_chunks(4, 8)

                etop40 = mst.tile([1, NT], f32, tag="etop40")
                for r in range(5):
                    nc.vector.max(out=etop40[:, 8 * r : 8 * r + 8], in_=wrow[:])
                    if r < 4:
                        nc.vector.match_replace(
                            out=wrow[:], in_to_replace=etop40[:, 8 * r : 8 * r + 8],
                            in_values=wrow[:], imm_value=NEG,
                        )
                ptb = ps.tile([P, 1], f32, tag="blk")
                nc.tensor.matmul(
                    ptb[:P, :1], ones_r1[:], etop40[:, NT - 1 : NT],
                    start=True, stop=True,
                )
                tbc = mst.tile([P, 1], f32, tag="tbc")
                nc.vector.tensor_copy(tbc[:], ptb[:P, :1])

                # selmask = (M - T40) >= -DELTA, one fused op
                selmask = mst.tile([P, 16], u8, tag="selmask")
                nc.vector.tensor_scalar(
                    selmask[:], M_all[:], tbc[:], -DELTA,
                    op0=OP.subtract, op1=OP.is_ge,
                )
                midx = mst.tile([P, 16], f32, tag="midx")
                nc.vector.tensor_copy(midx[:], negone[:].to_broadcast([P, 16]))
                nc.vector.copy_predicated(midx[:], selmask[:], qidx_f[:])

                pwr = ps.tile([16, P], f32, tag="blk", name="pwr")
                nc.tensor.transpose(pwr[:16, :P], midx[:], ident[:])
                wrap_in = mst.tile([16, P], f32, tag="wrap_in")
                nc.vector.tensor_copy(wrap_in[:], pwr[:16, :P])
                spg = mst.tile([16, 8], f32, tag="spg")
                nfound = mst.tile([1, 1], u32, tag="nfound")
                nc.gpsimd.sparse_gather(out=spg[:], in_=wrap_in[:], num_found=nfound[:])

                v_chunks(8, 12)

                spg_cl = mst.tile([16, 8], f32, tag="spg_cl")
                nc.vector.tensor_scalar_max(spg_cl[:], spg[:], 0.0)
                nc.vector.tensor_scalar_min(spg_cl[:], spg_cl[:], float(L - 1))

                # unwrap [16,8] -> [128,1] with 8 tiny one-hot matmuls
                # (perm16[p, f*128+u] = 1 iff u == p + 16*f, shipped constant)
                pcq = ps.tile([P, 1], f32, tag="blk", name="pcq")
                for f in range(8):
                    nc.tensor.matmul(
                        pcq[:P, :1], perm16[:, f * P : (f + 1) * P],
                        spg_cl[:, f : f + 1],
                        start=(f == 0), stop=(f == 7),
                    )
                candq_f = mst.tile([P, 1], f32, tag="candq_f")
                nc.vector.tensor_copy(candq_f[:], pcq[:P, :1])
                candq_i = mst.tile([P, 1], i32, tag="candq_i")
                nc.vector.tensor_copy(candq_i[:], pcq[:P, :1])

                nf_f = mst.tile([1, 1], f32, tag="nf_f")
                nc.vector.tensor_copy(nf_f[:], nfound[:])
                pnb = ps.tile([P, 1], f32, tag="blk")
                nc.tensor.matmul(pnb[:P, :1], ones_r1[:], nf_f[:], start=True, stop=True)
                nbc = mst.tile([P, 1], f32, tag="nbc")
                nc.vector.tensor_copy(nbc[:], pnb[:P, :1])
                invalid = mst.tile([P, 1], u8, tag="invalid")
                nc.vector.tensor_tensor(
                    out=invalid[:], in0=qidx_f[:, 0:1], in1=nbc[:], op=OP.is_ge
                )

                v_chunks(12, 16)

                # Vmean = xmean.T @ Wv.T via bf16x2 3-term, broadcast, ctx init
                pvm = ps.tile([1, 512], f32, tag="blk")
                n = 0
                for dc in range(ND):
                    for lh, rh in (
                        (xmh[dc][:], wvh[dc][:]),
                        (xml[dc][:], wvh[dc][:]),
                        (xmh[dc][:], wvl[dc][:]),
                    ):
                        nc.tensor.matmul(
                            pvm[:1, :], lh, rh,
                            start=(n == 0), stop=(n == 3 * ND - 1),
                        )
                        n += 1
                vmean = mst.tile([1, 512], f32, tag="vmean")
                nc.scalar.copy(vmean[:], pvm[:1, :])
                pvb = ps.tile([P, 512], f32, tag="blk")
                nc.tensor.matmul(pvb[:], ones_r1[:], vmean[:], start=True, stop=True)
                vmean_bc = mst.tile([P, 512], f32, tag="vmean_bc")
                nc.vector.tensor_copy(vmean_bc[:], pvb[:])
                for jc in range(NL):
                    nc.sync.dma_start(ctx_d[jc * P : (jc + 1) * P, :], vmean_bc[:])

                # ---------------- phase 4a: exact candidates ----------------
                x_cand = cnd.tile([P, D], f32, tag="x_cand")
                nc.gpsimd.indirect_dma_start(
                    out=x_cand[:], out_offset=None, in_=x_d[:],
                    in_offset=bass.IndirectOffsetOnAxis(ap=candq_i[:, :1], axis=0),
                )
                xcT = [cnd.tile([P, P], f32, tag=f"xcT{dc}", name=f"xcT{dc}") for dc in range(ND)]
                for dc in range(ND):
                    pxc = ps.tile([P, P], f32, tag="blk")
                    nc.tensor.transpose(
                        pxc[:P, :P], x_cand[:, dc * P : (dc + 1) * P], ident[:]
                    )
                    nc.vector.tensor_copy(xcT[dc][:], pxc[:P, :P])

                # Y^T = (x_cand @ A)^T in f32 via PE, then bf16 hi/lo split
                # for the 3-term S_cand product against xTh/xTl
                YTh = [cnd.tile([P, P], bf16, tag=f"YTh{ic}", name=f"YTh{ic}") for ic in range(ND)]
                YTl = [cnd.tile([P, P], bf16, tag=f"YTl{ic}", name=f"YTl{ic}") for ic in range(ND)]
                for ic in range(ND):
                    isl = slice(ic * P, (ic + 1) * P)
                    pqc = ps.tile([P, P], f32, tag="blk")
                    for dc in range(ND):
                        nc.tensor.matmul(
                            pqc[:P, :P], Arc[dc][:, isl], xcT[dc][:],
                            start=(dc == 0), stop=(dc == ND - 1),
                        )
                    nc.vector.tensor_copy(YTh[ic][:], pqc[:P, :P])
                    nc.vector.tensor_tensor(
                        out=YTl[ic][:], in0=pqc[:P, :P], in1=YTh[ic][:],
                        op=OP.subtract,
                    )

                gm = cnd.tile([P, L], u8, tag="gm")
                nc.gpsimd.indirect_dma_start(
                    out=gm[:], out_offset=None, in_=mask_d[:],
                    in_offset=bass.IndirectOffsetOnAxis(ap=candq_i[:, :1], axis=0),
                )
                gc = cnd.tile([P, L], u8, tag="gc")
                nc.gpsimd.indirect_dma_start(
                    out=gc[:], out_offset=None, in_=cnt_d[:],
                    in_offset=bass.IndirectOffsetOnAxis(ap=candq_i[:, :1], axis=0),
                )

                psS = []
                cmax = cnd.tile([P, NJ], f32, tag="cmax")
                csum = cnd.tile([P, NJ], f32, tag="csum")
                for jb in range(NJ):
                    jsl = slice(jb * 512, (jb + 1) * 512)
                    pss2 = ps_s.tile([P, 512], f32, tag="psSc")
                    psS.append(pss2)
                    n = 0
                    for ic in range(ND):
                        for lh, rh in (
                            (YTh[ic][:], xTh[ic][:, jsl]),
                            (YTl[ic][:], xTh[ic][:, jsl]),
                            (YTh[ic][:], xTl[ic][:, jsl]),
                        ):
                            nc.tensor.matmul(
                                pss2[:], lh, rh,
                                start=(n == 0), stop=(n == 3 * ND - 1),
                            )
                            n += 1
                    s3 = scr.tile([P, 512], f32, tag="scrt")
                    nc.vector.tensor_tensor(
                        out=s3[:], in0=pss2[:], in1=gm[:, jsl], op=OP.mult
                    )
                    nc.vector.reduce_max(cmax[:, jb : jb + 1], s3[:], axis=AX.X)
                    s4 = scr.tile([P, 512], f32, tag="scrt")
                    nc.vector.scalar_tensor_tensor(
                        out=s4[:], in0=pss2[:], scalar=-1.0 / L, in1=gc[:, jsl],
                        op0=OP.mult, op1=OP.mult,
                        accum_out=csum[:, jb : jb + 1],
                    )
                u1 = cnd.tile([P, 1], f32, tag="u1")
                u2 = cnd.tile([P, 1], f32, tag="u2")
                M_cand = cnd.tile([P, 1], f32, tag="M_cand")
                nc.vector.reduce_max(u1[:], cmax[:], axis=AX.X)
                nc.vector.reduce_sum(u2[:], csum[:], axis=AX.X)
                nc.vector.tensor_tensor(out=M_cand[:], in0=u1[:], in1=u2[:], op=OP.add)
                nc.vector.copy_predicated(M_cand[:], invalid[:], negbig[:])

                # exact top-40 threshold among candidates
                pmc = ps.tile([1, P], f32, tag="blk")
                nc.tensor.transpose(pmc[:1, :P], M_cand[:], ident[:])
                mcT = cnd.tile([1, P], f32, tag="mcT")
                nc.vector.tensor_copy(mcT[:], pmc[:1, :P])
                etop = cnd.tile([1, NT], f32, tag="etop")
                for r in range(5):
                    nc.vector.max(out=etop[:, 8 * r : 8 * r + 8], in_=mcT[:])
                    if r < 4:
                        nc.vector.match_replace(
                            out=mcT[:], in_to_replace=etop[:, 8 * r : 8 * r + 8],
                            in_values=mcT[:], imm_value=NEG,
                        )
                pte = ps.tile([P, 1], f32, tag="blk")
                nc.tensor.matmul(
                    pte[:P, :1], ones_r1[:], etop[:, NT - 1 : NT], start=True, stop=True
                )
                tebc = cnd.tile([P, 1], f32, tag="tebc")
                nc.vector.tensor_copy(tebc[:], pte[:P, :1])
                sel2 = cnd.tile([P, 1], u8, tag="sel2")
                nc.vector.tensor_tensor(
                    out=sel2[:], in0=M_cand[:], in1=tebc[:], op=OP.is_ge
                )
                scat_f = cnd.tile([P, 1], f32, tag="scat_f")
                nc.vector.tensor_copy(scat_f[:], big9[:])
                nc.vector.copy_predicated(scat_f[:], sel2[:], candq_f[:])
                scat_i = cnd.tile([P, 1], i32, tag="scat_i")
                nc.vector.tensor_copy(scat_i[:], scat_f[:])

            # xTh/xTl/weights freed here
            with tc.tile_pool(name="expp", bufs=1) as expp:
                # ---------------- phase 4b: softmax + update ----------------
                # no max-subtraction: scores*SCALE is O(10), exp is fp32-safe
                exp_sb = expp.tile([P, L], f32, tag="exp_sb")
                sume4 = expp.tile([P, NJ], f32, tag="sume4")
                for jb in range(NJ):
                    jsl = slice(jb * 512, (jb + 1) * 512)
                    nc.scalar.activation(
                        out=exp_sb[:, jsl], in_=psS[jb][:], func=ACTF.Exp,
                        bias=0.0, scale=SCALE,
                        accum_out=sume4[:, jb : jb + 1],
                    )
                sume = expp.tile([P, 1], f32, tag="sume")
                nc.vector.reduce_sum(sume[:], sume4[:], axis=AX.X)
                recip = expp.tile([P, 1], f32, tag="recip")
                nc.vector.reciprocal(recip[:], sume[:])

                # expT transposes and upd accumulation pipelined: the upd
                # matmul for chunk jc-1 issues right after transpose jc, so
                # PE never waits for the vector copies to drain.
                expT = [expp.tile([P, P], bf16, tag=f"expT{jc}", name=f"expT{jc}") for jc in range(NL)]
                pu = ps.tile([P, 512], f32, tag="blk", name="pu")
                for jc in range(NL):
                    pet = ps.tile([P, P], f32, tag="blk")
                    nc.tensor.transpose(
                        pet[:P, :P], exp_sb[:, jc * P : (jc + 1) * P], ident[:]
                    )
                    nc.vector.tensor_copy(expT[jc][:], pet[:P, :P])
                    if jc >= 1:
                        nc.tensor.matmul(
                            pu[:], expT[jc - 1][:], Vb[jc - 1][:],
                            start=(jc == 1), stop=False,
                        )
                nc.tensor.matmul(
                    pu[:], expT[NL - 1][:], Vb[NL - 1][:],
                    start=False, stop=True,
                )
                upd = expp.tile([P, D], f32, tag="upd")
                nc.scalar.activation(
                    out=upd[:], in_=pu[:], func=ACTF.Copy, bias=0.0, scale=recip[:]
                )
                nc.gpsimd.indirect_dma_start(
                    out=ctx_d[:],
                    out_offset=bass.IndirectOffsetOnAxis(ap=scat_i[:, :1], axis=0),
                    in_=upd[:], in_offset=None,
                    bounds_check=L - 1, oob_is_err=False,
                )

    nc.compile()
    return nc


_NC = None


def _get_nc():
    global _NC
    if _NC is None:
        _NC = build()
    return _NC


def _split_bf16(a):
    hi = a.astype(ml_dtypes.bfloat16)
    lo = (a - hi.astype(np.float32)).astype(ml_dtypes.bfloat16)
    return hi, lo


def _row_chunk(w):
    # [512, 512] -> [128, 4*512]: tile dc = rows dc*128..dc*128+127
    return np.concatenate([w[dc * P : (dc + 1) * P, :] for dc in range(4)], axis=1)


def _host_prep(x, Wq, Wk, Wv, index_sample):
    x = np.asarray(x, dtype=np.float32)
    Wq = np.asarray(Wq, dtype=np.float32)
    Wk = np.asarray(Wk, dtype=np.float32)
    Wv = np.asarray(Wv, dtype=np.float32)
    idx = np.asarray(index_sample)

    wqT = np.ascontiguousarray(Wq.T)
    wqh, _ = _split_bf16(wqT)
    wkh, wkl = _split_bf16(np.ascontiguousarray(Wk.T))
    wvh, wvl = _split_bf16(np.ascontiguousarray(Wv.T))
    A = (Wq.T.astype(np.float64) @ Wk.astype(np.float64)).astype(np.float32)

    rows = np.arange(L)[:, None]
    mask01 = np.zeros((L, L), dtype=np.uint8)
    mask01[rows, idx] = 1
    maskb = mask01.astype(ml_dtypes.bfloat16)
    countf = np.zeros((L, L), dtype=np.uint8)
    np.add.at(countf, (rows, idx), 1)

    perm16 = np.zeros((16, 8 * P), dtype=np.float32)
    for f in range(8):
        for p in range(16):
            perm16[p, f * P + p + 16 * f] = 1.0
    identf = np.eye(P, dtype=np.float32)
    qidxf = (np.arange(P, dtype=np.float32)[:, None]
             + 128.0 * np.arange(16, dtype=np.float32)[None, :])
    acat = np.ascontiguousarray(_row_chunk(A))
    shared = {
        "Acat": acat, "mask01": mask01, "maskb": maskb, "countf": countf,
        "perm16": perm16, "identf": identf, "qidxf": qidxf,
    }
    wpart = np.concatenate(
        [_row_chunk(wqh), _row_chunk(wkh), _row_chunk(wkl),
         _row_chunk(wvh), _row_chunk(wvl)],
        axis=1,
    )
    in_maps = []
    for b in range(B):
        xb = np.ascontiguousarray(x[b])
        xT = np.ascontiguousarray(xb.T)
        xth, xtl = _split_bf16(xT)
        xmean = (xb.astype(np.float64).mean(axis=0) / 1.0).astype(np.float32)
        xmeh, xmel = _split_bf16(xmean.reshape(D, 1))
        wcat = np.concatenate(
            [wpart, xmeh.reshape(4, P).T, xmel.reshape(4, P).T], axis=1
        ).astype(ml_dtypes.bfloat16)
        in_maps.append(
            {
                "x_nat": xb,
                "xTh": xth,
                "xTl": xtl,
                "wcat": np.ascontiguousarray(wcat),
                **shared,
            }
        )
    return in_maps


def kernel(x, Wq, Wk, Wv, index_sample, _trace=False, _result_box=None):
    in_maps = _host_prep(x, Wq, Wk, Wv, index_sample)
    nc = _get_nc()
    res = run_bass_kernel_spmd(nc, in_maps, core_ids=list(range(B)), trace=_trace)
    if _result_box is not None:
        _result_box.append(res)
    out = np.stack([np.asarray(res.results[b]["ctx"]) for b in range(B)], axis=0)
    return out



# revision 39
# speedup vs baseline: 1.6395x; 1.0121x over previous
"""Sparse attention (ProbSparse-style) Trainium2 Bass kernel.

Problem (per batch element b, data-parallel over 8 NeuronCores):
  Q = x @ Wq.T ; K = x @ Wk.T ; V = x @ Wv.T            [L=2048, D=512]
  QK_sample[l,s] = Q[l] . K[index_sample[l,s]]           [L, 40]
  M[l] = max_s QK_sample - sum_s QK_sample / L
  sel = top40(M)  (as a set; the reference scatter makes order irrelevant)
  scores = Q[sel] @ K.T / sqrt(D); attn = softmax(scores)
  ctx = broadcast(mean(V)); ctx[sel] = attn @ V

Numerics strategy (top-40 boundary gaps are as small as 0.02 in M):
  - K and V are computed with a 3-term bf16x2 split matmul
    (xh*wh + xl*wh + xh*wl, host-split halves) -> ~1e-5 absolute error,
    fp32-class, at full bf16 PE rate.
  - Approx M for ALL rows uses bf16 Q and bf16 K (error sigma ~0.2),
    extracted from per-chunk S = Q K^T PSUM blocks with fused
    tensor_tensor_reduce against a shipped u8 sample mask
    (multiply-mask max is safe: sampled max > 0 w.p. 1-2^-40;
    dup-count correction is deferred to the exact stage).
  - Candidates = { M_approx >= approx-top40 - DELTA }, DELTA=1.5 covers
    ~8 sigma; measured rank-40 to rank-64 M gap is 2.5-4.8 so the
    candidate count stays well under the 128-slot budget.
  - Exact stage on <= 128 candidate rows: gather x rows from DRAM
    (indirect DMA), exact fp32 Q_cand, exact S_cand vs the fp32-class K,
    TTR with gathered u8 mask+count rows -> exact M_cand -> exact top-40
    threshold -> softmax over S_cand -> upd = attn @ V -> indirect
    scatter of the 40 selected rows into ctx (bounds_check skips the
    rest).

kernel(**inputs) accepts the FULL inputs and returns the FULL
[8, 2048, 512] f32 output; batch is sharded over 8 cores.
"""

import math

import numpy as np
import ml_dtypes

import concourse.bacc as bacc
import concourse.bass as bass
import concourse.mybir as mybir
import concourse.tile as tile
from concourse.bass_utils import run_bass_kernel_spmd

P = 128
L = 2048
D = 512
B = 8
NL = L // P        # 16 query chunks
ND = D // P        # 4 feature chunks
NJ = L // 512      # 4 key blocks of 512
NT = 40
SCALE = 1.0 / math.sqrt(D)
# candidate band below approx T40: covers 2x bf16 dot error (~8 sigma =
# 1.5) plus the omitted -sum_s/L term in approx M (|sum/L| <= ~0.25
# per row at 3.5 sigma, both directions -> +0.5)
DELTA = 2.2
NEG = -3.0e38
SKIP_IDX = 99999.0  # scatter index sentinel (> bounds_check -> row skipped)

f32 = mybir.dt.float32
bf16 = mybir.dt.bfloat16
u8 = mybir.dt.uint8
i32 = mybir.dt.int32
u32 = mybir.dt.uint32
AX = mybir.AxisListType
OP = mybir.AluOpType
ACTF = mybir.ActivationFunctionType


def build():
    nc = bacc.Bacc("TRN2", target_bir_lowering=False)

    x_d = nc.dram_tensor("x_nat", [L, D], f32, kind="ExternalInput")
    xth_d = nc.dram_tensor("xTh", [D, L], bf16, kind="ExternalInput")
    xtl_d = nc.dram_tensor("xTl", [D, L], bf16, kind="ExternalInput")
    # all bf16 weight tiles + xmean hi/lo packed into one wide row-major
    # tensor: [wqh|wkh|wkl|wvh|wvl] each 4x512 cols, then xmh(4), xml(4)
    wcat_d = nc.dram_tensor("wcat", [P, 20 * 512 + 8], bf16, kind="ExternalInput")
    # A = Wq^T @ Wk (host f64): scores = x_cand @ A @ x^T, so no exact K
    # projection is ever needed on-device
    acat_d = nc.dram_tensor("Acat", [P, 4 * D], f32, kind="ExternalInput")
    maskb_d = nc.dram_tensor("maskb", [L, L], bf16, kind="ExternalInput")
    ident_d = nc.dram_tensor("identf", [P, P], f32, kind="ExternalInput")
    qidx_d = nc.dram_tensor("qidxf", [P, 16], f32, kind="ExternalInput")
    perm_d = nc.dram_tensor("perm16", [16, 8 * P], f32, kind="ExternalInput")
    mask_d = nc.dram_tensor("mask01", [L, L], u8, kind="ExternalInput")
    cnt_d = nc.dram_tensor("countf", [L, L], u8, kind="ExternalInput")
    ctx_d = nc.dram_tensor("ctx", [L, D], f32, kind="ExternalOutput")

    with tile.TileContext(nc) as tc:
        with (
            tc.tile_pool(name="const", bufs=1) as cst,
            tc.tile_pool(name="proj", bufs=1) as proj,       # KT/KTb/QTb/V resident
            tc.tile_pool(name="mstuff", bufs=1) as mst,      # M / topk / sel smalls
            tc.tile_pool(name="mstream", bufs=3) as mstr,    # mask chunks
            tc.tile_pool(name="scr", bufs=3) as scr,         # TTR scratch
            tc.tile_pool(name="acc", bufs=2) as accp,        # per-chunk accums
            tc.tile_pool(name="cand", bufs=1) as cnd,        # exact-stage tiles
            tc.tile_pool(name="ps", bufs=3, space="PSUM") as ps,
            tc.tile_pool(name="ps_s", bufs=4, space="PSUM") as ps_s,  # S_cand (held)
            tc.tile_pool(name="dram", bufs=1, space="DRAM") as drp,
        ):
            # ---------------- constants ----------------
            # sparse_gather is the only library-tracked GPSIMD op left;
            # preload its (small) library before the serial tail
            from concourse import library_config
            nc.gpsimd.load_library(library_config.sparse_gather)
            ident = cst.tile([P, P], f32, tag="ident")
            nc.sync.dma_start(ident[:], ident_d[:])
            ones_r1 = cst.tile([1, P], f32, tag="ones_r1")
            nc.vector.memset(ones_r1[:], 1.0)
            negone = cst.tile([P, 1], f32, tag="negone")
            nc.vector.memset(negone[:], -1.0)
            negbig = cst.tile([P, 1], f32, tag="negbig")
            nc.vector.memset(negbig[:], NEG)
            big9 = cst.tile([P, 1], f32, tag="big9")
            nc.vector.memset(big9[:], SKIP_IDX)
            perm16 = cst.tile([16, 8 * P], f32, tag="perm16")
            nc.sync.dma_start(perm16[:], perm_d[:])
            qidx_f = cst.tile([P, 16], f32, tag="qidx_f")    # value p + 128*c
            nc.sync.dma_start(qidx_f[:], qidx_d[:])

            # resident projection outputs
            KTb = [proj.tile([P, L], bf16, tag=f"KTb{ic}", name=f"KTb{ic}") for ic in range(ND)]
            QTb = [proj.tile([P, L], bf16, tag=f"QTb{ic}", name=f"QTb{ic}") for ic in range(ND)]
            Vb = [proj.tile([P, D], bf16, tag=f"Vb{jc}", name=f"Vb{jc}") for jc in range(NL)]

            with tc.tile_pool(name="xw", bufs=1) as xw:
                # ---------------- phase 0: loads ----------------
                xTh = [xw.tile([P, L], bf16, tag=f"xTh{dc}", name=f"xTh{dc}") for dc in range(ND)]
                xTl = [xw.tile([P, L], bf16, tag=f"xTl{dc}", name=f"xTl{dc}") for dc in range(ND)]
                wcat = xw.tile([P, 20 * 512 + 8], bf16, tag="wcat")
                acat = xw.tile([P, 4 * D], f32, tag="acat")
                # weight-tile views into the packed wcat
                def wview(group, dc):
                    off = group * 4 * 512 + dc * 512
                    return wcat[:, off : off + 512]
                wqh = [wview(0, dc) for dc in range(ND)]
                wkh = [wview(1, dc) for dc in range(ND)]
                wkl = [wview(2, dc) for dc in range(ND)]
                wvh = [wview(3, dc) for dc in range(ND)]
                wvl = [wview(4, dc) for dc in range(ND)]
                xmh = [wcat[:, 20 * 512 + dc : 20 * 512 + dc + 1] for dc in range(ND)]
                xml = [wcat[:, 20 * 512 + 4 + dc : 20 * 512 + 5 + dc] for dc in range(ND)]
                Arc = [acat[:, dc * 512 : (dc + 1) * 512] for dc in range(ND)]
                # DMA order: wqh + xTh first (Q projection starts earliest),
                # then wk/x-lo for K, then wv/xmean, then the f32 wqT (tail
                # only). 1024-col chunks = 2KB rows, spread across queues.
                nc.sync.dma_start(wcat[:, 0:1024], wcat_d[:, 0:1024])
                nc.sync.dma_start(wcat[:, 1024:2048], wcat_d[:, 1024:2048])
                for dc in range(ND):
                    sl = slice(dc * P, (dc + 1) * P)
                    nc.sync.dma_start(xTh[dc][:, 0:1024], xth_d[sl, 0:1024])
                    nc.sync.dma_start(xTh[dc][:, 1024:2048], xth_d[sl, 1024:2048])
                for c0 in range(2048, 4096, 1024):       # wkh (wkl unused)
                    nc.sync.dma_start(wcat[:, c0 : c0 + 1024], wcat_d[:, c0 : c0 + 1024])
                for dc in range(ND):
                    sl = slice(dc * P, (dc + 1) * P)
                    nc.sync.dma_start(xTl[dc][:, 0:1024], xtl_d[sl, 0:1024])
                    nc.sync.dma_start(xTl[dc][:, 1024:2048], xtl_d[sl, 1024:2048])
                for c0 in range(6144, 10240, 1024):      # wvh, wvl
                    nc.sync.dma_start(wcat[:, c0 : c0 + 1024], wcat_d[:, c0 : c0 + 1024])
                nc.sync.dma_start(wcat[:, 10240:10248], wcat_d[:, 10240:10248])
                nc.sync.dma_start(acat[:, 0:1024], acat_d[:, 0:1024])
                nc.sync.dma_start(acat[:, 1024:2048], acat_d[:, 1024:2048])

                # ---------------- phase 1: projections ----------------
                # Q first (single bf16 term, needs only wqh+xTh), jb-major;
                # then K 3-term jb-major so S blocks can start after K jb=0.
                for jb in range(NJ):
                    jsl = slice(jb * 512, (jb + 1) * 512)
                    for ic in range(ND):
                        isl = slice(ic * P, (ic + 1) * P)
                        pq = ps.tile([P, 512], f32, tag="blk")
                        for dc in range(ND):
                            nc.tensor.matmul(
                                pq[:], wqh[dc][:, isl], xTh[dc][:, jsl],
                                start=(dc == 0), stop=(dc == ND - 1),
                            )
                        nc.scalar.copy(QTb[ic][:, jsl], pq[:])
                # K approx: single bf16 term (the exact stage goes through
                # A = Wq^T Wk and never needs an exact K)
                for jb in range(NJ):
                    jsl = slice(jb * 512, (jb + 1) * 512)
                    for ic in range(ND):
                        isl = slice(ic * P, (ic + 1) * P)
                        pk = ps.tile([P, 512], f32, tag="blk")
                        for dc in range(ND):
                            nc.tensor.matmul(
                                pk[:], wkh[dc][:, isl], xTh[dc][:, jsl],
                                start=(dc == 0), stop=(dc == ND - 1),
                            )
                        nc.scalar.copy(KTb[ic][:, jsl], pk[:])

                # ---------------- phase 2: approx M (bf16 S) ----------------
                # per (lc, jb) block: ONE fused TTR (masked product -> bf16
                # scratch, fused max accum). The -sum_s/L term is omitted in
                # the approx M (absorbed into DELTA); the exact stage still
                # uses the full formula.
                M_all = mst.tile([P, 16], f32, tag="M_all")
                amax_all = mst.tile([P, NL * NJ], f32, tag="amax_all")
                for lc in range(NL):
                    lsl = slice(lc * P, (lc + 1) * P)
                    mk = mstr.tile([P, L], bf16, tag="mk")
                    nc.sync.dma_start(mk[:], maskb_d[lsl, :])
                    for jb in range(NJ):
                        jsl = slice(jb * 512, (jb + 1) * 512)
                        k = lc * NJ + jb
                        pss = ps_s.tile([P, 512], f32, tag="psSc", name="pssa")
                        for ic in range(ND):
                            nc.tensor.matmul(
                                pss[:], QTb[ic][:, lsl], KTb[ic][:, jsl],
                                start=(ic == 0), stop=(ic == ND - 1),
                            )
                        # scalar engine drains PSUM to bf16 SBUF so the DVE
                        # mask-multiply runs in 2x packed mode (both operands
                        # bf16 SBUF); reduce_max is 1x regardless.
                        s0 = scr.tile([P, 512], bf16, tag="s0t")
                        nc.scalar.copy(s0[:], pss[:])
                        s1 = scr.tile([P, 512], bf16, tag="scrt")
                        nc.vector.tensor_tensor(
                            out=s1[:], in0=s0[:], in1=mk[:, jsl], op=OP.mult
                        )
                        nc.vector.reduce_max(
                            amax_all[:, k : k + 1], s1[:], axis=AX.X
                        )
                nc.vector.reduce_max(
                    M_all[:], amax_all[:].rearrange("p (c j) -> p c j", j=NJ),
                    axis=AX.X,
                )

                # ---------------- phase 3: approx top-40 -> candidates ------
                # exact T40 of M_approx without GPSIMD kth_largest (its attn
                # library reload cost ~50us of dead time on the serial tail):
                # per-chunk top-16 via vector max8/match_replace on M^T
                # [16,128], union (256 vals) holds the global top-40 w.p.
                # 1-2e-8, pack union into one [1,256] row via one-hot matmul
                # unwrap + transposes, then 5 rounds max8/match_replace.
                pmt = ps.tile([16, P], f32, tag="blk", name="pmt")
                nc.tensor.transpose(pmt[:16, :P], M_all[:], ident[:])
                MT = mst.tile([16, P], f32, tag="MT")
                nc.vector.tensor_copy(MT[:], pmt[:16, :P])
                w16 = mst.tile([16, 16], f32, tag="w16")
                nc.vector.max(out=w16[:, 0:8], in_=MT[:])
                nc.vector.match_replace(
                    out=MT[:], in_to_replace=w16[:, 0:8],
                    in_values=MT[:], imm_value=NEG,
                )
                nc.vector.max(out=w16[:, 8:16], in_=MT[:])

                # V projection (single bf16 term: the upd matmul consumes
                # bf16 anyway) is interleaved through the threshold/compaction
                # chain in 4-chunk groups so PE stays busy while the vector
                # engine and GPSIMD work through the serial tail.
                def v_chunks(lo, hi):
                    for lc in range(lo, hi):
                        lsl = slice(lc * P, (lc + 1) * P)
                        pv = ps.tile([P, 512], f32, tag="blk")
                        for dc in range(ND):
                            nc.tensor.matmul(
                                pv[:], xTh[dc][:, lsl], wvh[dc][:],
                                start=(dc == 0), stop=(dc == ND - 1),
                            )
                        nc.scalar.copy(Vb[lc][:], pv[:])

                v_chunks(0, 6)

                # unwrap w16 [16,16] -> two [128,1] columns (one-hot matmuls),
                # then -> [1,256] row via two PE transposes
                pcu = ps.tile([P, 2], f32, tag="blk", name="pcu")
                for f in range(8):
                    nc.tensor.matmul(
                        pcu[:P, 0:1], perm16[:, f * P : (f + 1) * P],
                        w16[:, f : f + 1],
                        start=(f == 0), stop=(f == 7),
                    )
                for f in range(8):
                    nc.tensor.matmul(
                        pcu[:P, 1:2], perm16[:, f * P : (f + 1) * P],
                        w16[:, 8 + f : 9 + f],
                        start=(f == 0), stop=(f == 7),
                    )
                crow = mst.tile([P, 2], f32, tag="crow")
                nc.vector.tensor_copy(crow[:], pcu[:P, :2])
                pr1 = ps.tile([1, P], f32, tag="blk", name="pr1")
                nc.tensor.transpose(pr1[:1, :P], crow[:, 0:1], ident[:])
                wrow = mst.tile([1, 2 * P], f32, tag="wrow")
                nc.vector.tensor_copy(wrow[:, 0:P], pr1[:1, :P])
                pr2 = ps.tile([1, P], f32, tag="blk", name="pr2")
                nc.tensor.transpose(pr2[:1, :P], crow[:, 1:2], ident[:])
                nc.vector.tensor_copy(wrow[:, P : 2 * P], pr2[:1, :P])

                v_chunks(6, 10)

                etop40 = mst.tile([1, NT], f32, tag="etop40")
                for r in range(5):
                    nc.vector.max(out=etop40[:, 8 * r : 8 * r + 8], in_=wrow[:])
                    if r < 4:
                        nc.vector.match_replace(
                            out=wrow[:], in_to_replace=etop40[:, 8 * r : 8 * r + 8],
                            in_values=wrow[:], imm_value=NEG,
                        )
                ptb = ps.tile([P, 1], f32, tag="blk")
                nc.tensor.matmul(
                    ptb[:P, :1], ones_r1[:], etop40[:, NT - 1 : NT],
                    start=True, stop=True,
                )
                tbc = mst.tile([P, 1], f32, tag="tbc")
                nc.vector.tensor_copy(tbc[:], ptb[:P, :1])

                # selmask = (M - T40) >= -DELTA, one fused op
                selmask = mst.tile([P, 16], u8, tag="selmask")
                nc.vector.tensor_scalar(
                    selmask[:], M_all[:], tbc[:], -DELTA,
                    op0=OP.subtract, op1=OP.is_ge,
                )
                midx = mst.tile([P, 16], f32, tag="midx")
                nc.vector.tensor_copy(midx[:], negone[:].to_broadcast([P, 16]))
                nc.vector.copy_predicated(midx[:], selmask[:], qidx_f[:])

                pwr = ps.tile([16, P], f32, tag="blk", name="pwr")
                nc.tensor.transpose(pwr[:16, :P], midx[:], ident[:])
                wrap_in = mst.tile([16, P], f32, tag="wrap_in")
                nc.vector.tensor_copy(wrap_in[:], pwr[:16, :P])
                spg = mst.tile([16, 8], f32, tag="spg")
                nfound = mst.tile([1, 1], u32, tag="nfound")
                nc.gpsimd.sparse_gather(out=spg[:], in_=wrap_in[:], num_found=nfound[:])

                v_chunks(10, 13)

                spg_cl = mst.tile([16, 8], f32, tag="spg_cl")
                nc.vector.tensor_scalar_max(spg_cl[:], spg[:], 0.0)
                nc.vector.tensor_scalar_min(spg_cl[:], spg_cl[:], float(L - 1))

                # unwrap [16,8] -> [128,1] with 8 tiny one-hot matmuls
                # (perm16[p, f*128+u] = 1 iff u == p + 16*f, shipped constant)
                pcq = ps.tile([P, 1], f32, tag="blk", name="pcq")
                for f in range(8):
                    nc.tensor.matmul(
                        pcq[:P, :1], perm16[:, f * P : (f + 1) * P],
                        spg_cl[:, f : f + 1],
                        start=(f == 0), stop=(f == 7),
                    )
                candq_f = mst.tile([P, 1], f32, tag="candq_f")
                nc.vector.tensor_copy(candq_f[:], pcq[:P, :1])
                candq_i = mst.tile([P, 1], i32, tag="candq_i")
                nc.vector.tensor_copy(candq_i[:], pcq[:P, :1])

                nf_f = mst.tile([1, 1], f32, tag="nf_f")
                nc.vector.tensor_copy(nf_f[:], nfound[:])
                pnb = ps.tile([P, 1], f32, tag="blk")
                nc.tensor.matmul(pnb[:P, :1], ones_r1[:], nf_f[:], start=True, stop=True)
                nbc = mst.tile([P, 1], f32, tag="nbc")
                nc.vector.tensor_copy(nbc[:], pnb[:P, :1])
                invalid = mst.tile([P, 1], u8, tag="invalid")
                nc.vector.tensor_tensor(
                    out=invalid[:], in0=qidx_f[:, 0:1], in1=nbc[:], op=OP.is_ge
                )

                v_chunks(13, 16)

                # Vmean = xmean.T @ Wv.T via bf16x2 3-term, broadcast, ctx init
                pvm = ps.tile([1, 512], f32, tag="blk")
                n = 0
                for dc in range(ND):
                    for lh, rh in (
                        (xmh[dc][:], wvh[dc][:]),
                        (xml[dc][:], wvh[dc][:]),
                        (xmh[dc][:], wvl[dc][:]),
                    ):
                        nc.tensor.matmul(
                            pvm[:1, :], lh, rh,
                            start=(n == 0), stop=(n == 3 * ND - 1),
                        )
                        n += 1
                vmean = mst.tile([1, 512], f32, tag="vmean")
                nc.scalar.copy(vmean[:], pvm[:1, :])
                pvb = ps.tile([P, 512], f32, tag="blk")
                nc.tensor.matmul(pvb[:], ones_r1[:], vmean[:], start=True, stop=True)
                vmean_bc = mst.tile([P, 512], f32, tag="vmean_bc")
                nc.vector.tensor_copy(vmean_bc[:], pvb[:])
                for jc in range(NL):
                    nc.sync.dma_start(ctx_d[jc * P : (jc + 1) * P, :], vmean_bc[:])

                # ---------------- phase 4a: exact candidates ----------------
                x_cand = cnd.tile([P, D], f32, tag="x_cand")
                nc.gpsimd.indirect_dma_start(
                    out=x_cand[:], out_offset=None, in_=x_d[:],
                    in_offset=bass.IndirectOffsetOnAxis(ap=candq_i[:, :1], axis=0),
                )
                xcT = [cnd.tile([P, P], f32, tag=f"xcT{dc}", name=f"xcT{dc}") for dc in range(ND)]
                for dc in range(ND):
                    pxc = ps.tile([P, P], f32, tag="blk")
                    nc.tensor.transpose(
                        pxc[:P, :P], x_cand[:, dc * P : (dc + 1) * P], ident[:]
                    )
                    nc.vector.tensor_copy(xcT[dc][:], pxc[:P, :P])

                # Y^T = (x_cand @ A)^T in f32 via PE, then bf16 hi/lo split
                # for the 3-term S_cand product against xTh/xTl
                YTh = [cnd.tile([P, P], bf16, tag=f"YTh{ic}", name=f"YTh{ic}") for ic in range(ND)]
                YTl = [cnd.tile([P, P], bf16, tag=f"YTl{ic}", name=f"YTl{ic}") for ic in range(ND)]
                for ic in range(ND):
                    isl = slice(ic * P, (ic + 1) * P)
                    pqc = ps.tile([P, P], f32, tag="blk")
                    for dc in range(ND):
                        nc.tensor.matmul(
                            pqc[:P, :P], Arc[dc][:, isl], xcT[dc][:],
                            start=(dc == 0), stop=(dc == ND - 1),
                        )
                    nc.vector.tensor_copy(YTh[ic][:], pqc[:P, :P])
                    nc.vector.tensor_tensor(
                        out=YTl[ic][:], in0=pqc[:P, :P], in1=YTh[ic][:],
                        op=OP.subtract,
                    )

                gm = cnd.tile([P, L], u8, tag="gm")
                nc.gpsimd.indirect_dma_start(
                    out=gm[:], out_offset=None, in_=mask_d[:],
                    in_offset=bass.IndirectOffsetOnAxis(ap=candq_i[:, :1], axis=0),
                )
                gc = cnd.tile([P, L], u8, tag="gc")
                nc.gpsimd.indirect_dma_start(
                    out=gc[:], out_offset=None, in_=cnt_d[:],
                    in_offset=bass.IndirectOffsetOnAxis(ap=candq_i[:, :1], axis=0),
                )

                psS = []
                cmax = cnd.tile([P, NJ], f32, tag="cmax")
                csum = cnd.tile([P, NJ], f32, tag="csum")
                for jb in range(NJ):
                    jsl = slice(jb * 512, (jb + 1) * 512)
                    pss2 = ps_s.tile([P, 512], f32, tag="psSc")
                    psS.append(pss2)
                    n = 0
                    for ic in range(ND):
                        for lh, rh in (
                            (YTh[ic][:], xTh[ic][:, jsl]),
                            (YTl[ic][:], xTh[ic][:, jsl]),
                            (YTh[ic][:], xTl[ic][:, jsl]),
                        ):
                            nc.tensor.matmul(
                                pss2[:], lh, rh,
                                start=(n == 0), stop=(n == 3 * ND - 1),
                            )
                            n += 1
                    s3 = scr.tile([P, 512], f32, tag="scrt")
                    nc.vector.tensor_tensor(
                        out=s3[:], in0=pss2[:], in1=gm[:, jsl], op=OP.mult
                    )
                    nc.vector.reduce_max(cmax[:, jb : jb + 1], s3[:], axis=AX.X)
                    s4 = scr.tile([P, 512], f32, tag="scrt")
                    nc.vector.scalar_tensor_tensor(
                        out=s4[:], in0=pss2[:], scalar=-1.0 / L, in1=gc[:, jsl],
                        op0=OP.mult, op1=OP.mult,
                        accum_out=csum[:, jb : jb + 1],
                    )
                u1 = cnd.tile([P, 1], f32, tag="u1")
                u2 = cnd.tile([P, 1], f32, tag="u2")
                M_cand = cnd.tile([P, 1], f32, tag="M_cand")
                nc.vector.reduce_max(u1[:], cmax[:], axis=AX.X)
                nc.vector.reduce_sum(u2[:], csum[:], axis=AX.X)
                nc.vector.tensor_tensor(out=M_cand[:], in0=u1[:], in1=u2[:], op=OP.add)
                nc.vector.copy_predicated(M_cand[:], invalid[:], negbig[:])

            # xTh/xTl/weights freed here
            with tc.tile_pool(name="expp", bufs=1) as expp:
                # ---------------- phase 4b: softmax + update ----------------
                # no max-subtraction: scores*SCALE is O(10), exp is fp32-safe
                exp_sb = expp.tile([P, L], f32, tag="exp_sb")
                sume4 = expp.tile([P, NJ], f32, tag="sume4")
                for jb in range(NJ):
                    jsl = slice(jb * 512, (jb + 1) * 512)
                    nc.scalar.activation(
                        out=exp_sb[:, jsl], in_=psS[jb][:], func=ACTF.Exp,
                        bias=0.0, scale=SCALE,
                        accum_out=sume4[:, jb : jb + 1],
                    )
                sume = expp.tile([P, 1], f32, tag="sume")
                nc.vector.reduce_sum(sume[:], sume4[:], axis=AX.X)
                recip = expp.tile([P, 1], f32, tag="recip")
                nc.vector.reciprocal(recip[:], sume[:])

                # expT transposes and upd accumulation pipelined: the upd
                # matmul for chunk jc-1 issues right after transpose jc. The
                # PSUM->bf16 drains run on the scalar engine so the vector
                # engine is free for the candidate top-40 rounds below.
                expT = [expp.tile([P, P], bf16, tag=f"expT{jc}", name=f"expT{jc}") for jc in range(NL)]
                pu = ps.tile([P, 512], f32, tag="blk", name="pu")
                for jc in range(NL):
                    pet = ps.tile([P, P], f32, tag="blk")
                    nc.tensor.transpose(
                        pet[:P, :P], exp_sb[:, jc * P : (jc + 1) * P], ident[:]
                    )
                    nc.scalar.copy(expT[jc][:], pet[:P, :P])
                    if jc >= 1:
                        nc.tensor.matmul(
                            pu[:], expT[jc - 1][:], Vb[jc - 1][:],
                            start=(jc == 1), stop=False,
                        )
                nc.tensor.matmul(
                    pu[:], expT[NL - 1][:], Vb[NL - 1][:],
                    start=False, stop=True,
                )

                # exact top-40 threshold among candidates (vector rounds run
                # concurrently with the expT/upd pipeline above)
                pmc = ps.tile([1, P], f32, tag="blk")
                nc.tensor.transpose(pmc[:1, :P], M_cand[:], ident[:])
                mcT = cnd.tile([1, P], f32, tag="mcT")
                nc.vector.tensor_copy(mcT[:], pmc[:1, :P])
                etop = cnd.tile([1, NT], f32, tag="etop")
                for r in range(5):
                    nc.vector.max(out=etop[:, 8 * r : 8 * r + 8], in_=mcT[:])
                    if r < 4:
                        nc.vector.match_replace(
                            out=mcT[:], in_to_replace=etop[:, 8 * r : 8 * r + 8],
                            in_values=mcT[:], imm_value=NEG,
                        )
                pte = ps.tile([P, 1], f32, tag="blk")
                nc.tensor.matmul(
                    pte[:P, :1], ones_r1[:], etop[:, NT - 1 : NT], start=True, stop=True
                )
                tebc = cnd.tile([P, 1], f32, tag="tebc")
                nc.vector.tensor_copy(tebc[:], pte[:P, :1])
                sel2 = cnd.tile([P, 1], u8, tag="sel2")
                nc.vector.tensor_tensor(
                    out=sel2[:], in0=M_cand[:], in1=tebc[:], op=OP.is_ge
                )
                scat_f = cnd.tile([P, 1], f32, tag="scat_f")
                nc.vector.tensor_copy(scat_f[:], big9[:])
                nc.vector.copy_predicated(scat_f[:], sel2[:], candq_f[:])
                scat_i = cnd.tile([P, 1], i32, tag="scat_i")
                nc.vector.tensor_copy(scat_i[:], scat_f[:])

                upd = expp.tile([P, D], f32, tag="upd")
                nc.scalar.activation(
                    out=upd[:], in_=pu[:], func=ACTF.Copy, bias=0.0, scale=recip[:]
                )
                nc.gpsimd.indirect_dma_start(
                    out=ctx_d[:],
                    out_offset=bass.IndirectOffsetOnAxis(ap=scat_i[:, :1], axis=0),
                    in_=upd[:], in_offset=None,
                    bounds_check=L - 1, oob_is_err=False,
                )

    nc.compile()
    return nc


_NC = None


def _get_nc():
    global _NC
    if _NC is None:
        _NC = build()
    return _NC


def _split_bf16(a):
    hi = a.astype(ml_dtypes.bfloat16)
    lo = (a - hi.astype(np.float32)).astype(ml_dtypes.bfloat16)
    return hi, lo


def _row_chunk(w):
    # [512, 512] -> [128, 4*512]: tile dc = rows dc*128..dc*128+127
    return np.concatenate([w[dc * P : (dc + 1) * P, :] for dc in range(4)], axis=1)


def _host_prep(x, Wq, Wk, Wv, index_sample):
    x = np.asarray(x, dtype=np.float32)
    Wq = np.asarray(Wq, dtype=np.float32)
    Wk = np.asarray(Wk, dtype=np.float32)
    Wv = np.asarray(Wv, dtype=np.float32)
    idx = np.asarray(index_sample)

    wqT = np.ascontiguousarray(Wq.T)
    wqh, _ = _split_bf16(wqT)
    wkh, wkl = _split_bf16(np.ascontiguousarray(Wk.T))
    wvh, wvl = _split_bf16(np.ascontiguousarray(Wv.T))
    A = (Wq.T.astype(np.float64) @ Wk.astype(np.float64)).astype(np.float32)

    rows = np.arange(L)[:, None]
    mask01 = np.zeros((L, L), dtype=np.uint8)
    mask01[rows, idx] = 1
    maskb = mask01.astype(ml_dtypes.bfloat16)
    countf = np.zeros((L, L), dtype=np.uint8)
    np.add.at(countf, (rows, idx), 1)

    perm16 = np.zeros((16, 8 * P), dtype=np.float32)
    for f in range(8):
        for p in range(16):
            perm16[p, f * P + p + 16 * f] = 1.0
    identf = np.eye(P, dtype=np.float32)
    qidxf = (np.arange(P, dtype=np.float32)[:, None]
             + 128.0 * np.arange(16, dtype=np.float32)[None, :])
    acat = np.ascontiguousarray(_row_chunk(A))
    shared = {
        "Acat": acat, "mask01": mask01, "maskb": maskb, "countf": countf,
        "perm16": perm16, "identf": identf, "qidxf": qidxf,
    }
    wpart = np.concatenate(
        [_row_chunk(wqh), _row_chunk(wkh), _row_chunk(wkl),
         _row_chunk(wvh), _row_chunk(wvl)],
        axis=1,
    )
    in_maps = []
    for b in range(B):
        xb = np.ascontiguousarray(x[b])
        xT = np.ascontiguousarray(xb.T)
        xth, xtl = _split_bf16(xT)
        xmean = (xb.astype(np.float64).mean(axis=0) / 1.0).astype(np.float32)
        xmeh, xmel = _split_bf16(xmean.reshape(D, 1))
        wcat = np.concatenate(
            [wpart, xmeh.reshape(4, P).T, xmel.reshape(4, P).T], axis=1
        ).astype(ml_dtypes.bfloat16)
        in_maps.append(
            {
                "x_nat": xb,
                "xTh": xth,
                "xTl": xtl,
                "wcat": np.ascontiguousarray(wcat),
                **shared,
            }
        )
    return in_maps


def kernel(x, Wq, Wk, Wv, index_sample, _trace=False, _result_box=None):
    in_maps = _host_prep(x, Wq, Wk, Wv, index_sample)
    nc = _get_nc()
    res = run_bass_kernel_spmd(nc, in_maps, core_ids=list(range(B)), trace=_trace)
    if _result_box is not None:
        _result_box.append(res)
    out = np.stack([np.asarray(res.results[b]["ctx"]) for b in range(B)], axis=0)
    return out



# revision 45
# speedup vs baseline: 1.7227x; 1.0508x over previous
"""Sparse attention (ProbSparse-style) Trainium2 Bass kernel.

Problem (per batch element b, data-parallel over 8 NeuronCores):
  Q = x @ Wq.T ; K = x @ Wk.T ; V = x @ Wv.T            [L=2048, D=512]
  QK_sample[l,s] = Q[l] . K[index_sample[l,s]]           [L, 40]
  M[l] = max_s QK_sample - sum_s QK_sample / L
  sel = top40(M)  (as a set; the reference scatter makes order irrelevant)
  scores = Q[sel] @ K.T / sqrt(D); attn = softmax(scores)
  ctx = broadcast(mean(V)); ctx[sel] = attn @ V

Numerics strategy (top-40 boundary gaps are as small as 0.02 in M):
  - K and V are computed with a 3-term bf16x2 split matmul
    (xh*wh + xl*wh + xh*wl, host-split halves) -> ~1e-5 absolute error,
    fp32-class, at full bf16 PE rate.
  - Approx M for ALL rows uses bf16 Q and bf16 K (error sigma ~0.2),
    extracted from per-chunk S = Q K^T PSUM blocks with fused
    tensor_tensor_reduce against a shipped u8 sample mask
    (multiply-mask max is safe: sampled max > 0 w.p. 1-2^-40;
    dup-count correction is deferred to the exact stage).
  - Candidates = { M_approx >= approx-top40 - DELTA }, DELTA=1.5 covers
    ~8 sigma; measured rank-40 to rank-64 M gap is 2.5-4.8 so the
    candidate count stays well under the 128-slot budget.
  - Exact stage on <= 128 candidate rows: gather x rows from DRAM
    (indirect DMA), exact fp32 Q_cand, exact S_cand vs the fp32-class K,
    TTR with gathered u8 mask+count rows -> exact M_cand -> exact top-40
    threshold -> softmax over S_cand -> upd = attn @ V -> indirect
    scatter of the 40 selected rows into ctx (bounds_check skips the
    rest).

kernel(**inputs) accepts the FULL inputs and returns the FULL
[8, 2048, 512] f32 output; batch is sharded over 8 cores.
"""

import math

import numpy as np
import ml_dtypes

import concourse.bacc as bacc
import concourse.bass as bass
import concourse.mybir as mybir
import concourse.tile as tile
from concourse.bass_utils import run_bass_kernel_spmd

P = 128
L = 2048
D = 512
B = 8
NL = L // P        # 16 query chunks
ND = D // P        # 4 feature chunks
NJ = L // 512      # 4 key blocks of 512
NT = 40
SCALE = 1.0 / math.sqrt(D)
# candidate band below approx T40: covers 2x bf16 dot error (~8 sigma =
# 1.5) plus the omitted -sum_s/L term in approx M (|sum/L| <= ~0.25
# per row at 3.5 sigma, both directions -> +0.5)
DELTA = 2.2
NEG = -3.0e38
SKIP_IDX = 99999.0  # scatter index sentinel (> bounds_check -> row skipped)

f32 = mybir.dt.float32
bf16 = mybir.dt.bfloat16
u8 = mybir.dt.uint8
i32 = mybir.dt.int32
u32 = mybir.dt.uint32
AX = mybir.AxisListType
OP = mybir.AluOpType
ACTF = mybir.ActivationFunctionType


def build():
    nc = bacc.Bacc("TRN2", target_bir_lowering=False)

    x_d = nc.dram_tensor("x_nat", [L, D], f32, kind="ExternalInput")
    xth_d = nc.dram_tensor("xTh", [D, L], bf16, kind="ExternalInput")
    xtl_d = nc.dram_tensor("xTl", [D, L], bf16, kind="ExternalInput")
    # all bf16 weight tiles + xmean hi/lo packed into one wide row-major
    # tensor: [wqh|wkh|wkl|wvh|wvl] each 4x512 cols, then xmh(4), xml(4)
    wcat_d = nc.dram_tensor("wcat", [P, 20 * 512 + 8], bf16, kind="ExternalInput")
    # A = Wq^T @ Wk (host f64): scores = x_cand @ A @ x^T, so no exact K
    # projection is ever needed on-device
    acat_d = nc.dram_tensor("Acat", [P, 4 * D], f32, kind="ExternalInput")
    maskb_d = nc.dram_tensor("maskb", [L, L], bf16, kind="ExternalInput")
    ident_d = nc.dram_tensor("identf", [P, P], f32, kind="ExternalInput")
    qidx_d = nc.dram_tensor("qidxf", [P, 16], f32, kind="ExternalInput")
    perm_d = nc.dram_tensor("perm16", [16, 8 * P], f32, kind="ExternalInput")
    mask_d = nc.dram_tensor("mask01", [L, L], u8, kind="ExternalInput")
    cnt_d = nc.dram_tensor("countf", [L, L], u8, kind="ExternalInput")
    ctx_d = nc.dram_tensor("ctx", [L, D], f32, kind="ExternalOutput")

    with tile.TileContext(nc) as tc:
        with (
            tc.tile_pool(name="const", bufs=1) as cst,
            tc.tile_pool(name="proj", bufs=1) as proj,       # KT/KTb/QTb/V resident
            tc.tile_pool(name="mstuff", bufs=1) as mst,      # M / topk / sel smalls
            tc.tile_pool(name="mstream", bufs=3) as mstr,    # mask chunks
            tc.tile_pool(name="scr", bufs=3) as scr,         # TTR scratch
            tc.tile_pool(name="acc", bufs=2) as accp,        # per-chunk accums
            tc.tile_pool(name="cand", bufs=1) as cnd,        # exact-stage tiles
            tc.tile_pool(name="ps", bufs=3, space="PSUM") as ps,
            tc.tile_pool(name="ps_s", bufs=4, space="PSUM") as ps_s,  # S_cand (held)
            tc.tile_pool(name="dram", bufs=1, space="DRAM") as drp,
        ):
            # ---------------- constants ----------------
            # sparse_gather is the only library-tracked GPSIMD op left;
            # preload its (small) library before the serial tail
            from concourse import library_config
            nc.gpsimd.load_library(library_config.sparse_gather)
            ident = cst.tile([P, P], f32, tag="ident")
            nc.sync.dma_start(ident[:], ident_d[:])
            ones_r1 = cst.tile([1, P], f32, tag="ones_r1")
            nc.vector.memset(ones_r1[:], 1.0)
            negone = cst.tile([P, 1], f32, tag="negone")
            nc.vector.memset(negone[:], -1.0)
            negbig = cst.tile([P, 1], f32, tag="negbig")
            nc.vector.memset(negbig[:], NEG)
            big9 = cst.tile([P, 1], f32, tag="big9")
            nc.vector.memset(big9[:], SKIP_IDX)
            perm16 = cst.tile([16, 8 * P], f32, tag="perm16")
            nc.sync.dma_start(perm16[:], perm_d[:])
            qidx_f = cst.tile([P, 16], f32, tag="qidx_f")    # value p + 128*c
            nc.sync.dma_start(qidx_f[:], qidx_d[:])

            # resident projection outputs
            KTb = [proj.tile([P, L], bf16, tag=f"KTb{ic}", name=f"KTb{ic}") for ic in range(ND)]
            QTb = [proj.tile([P, L], bf16, tag=f"QTb{ic}", name=f"QTb{ic}") for ic in range(ND)]
            Vb = [proj.tile([P, D], bf16, tag=f"Vb{jc}", name=f"Vb{jc}") for jc in range(NL)]

            with tc.tile_pool(name="xw", bufs=1) as xw:
                # ---------------- phase 0: loads ----------------
                xTh = [xw.tile([P, L], bf16, tag=f"xTh{dc}", name=f"xTh{dc}") for dc in range(ND)]
                xTl = [xw.tile([P, L], bf16, tag=f"xTl{dc}", name=f"xTl{dc}") for dc in range(ND)]
                wcat = xw.tile([P, 20 * 512 + 8], bf16, tag="wcat")
                acat = xw.tile([P, 4 * D], f32, tag="acat")
                # weight-tile views into the packed wcat
                def wview(group, dc):
                    off = group * 4 * 512 + dc * 512
                    return wcat[:, off : off + 512]
                wqh = [wview(0, dc) for dc in range(ND)]
                wkh = [wview(1, dc) for dc in range(ND)]
                wkl = [wview(2, dc) for dc in range(ND)]
                wvh = [wview(3, dc) for dc in range(ND)]
                wvl = [wview(4, dc) for dc in range(ND)]
                xmh = [wcat[:, 20 * 512 + dc : 20 * 512 + dc + 1] for dc in range(ND)]
                xml = [wcat[:, 20 * 512 + 4 + dc : 20 * 512 + 5 + dc] for dc in range(ND)]
                Arc = [acat[:, dc * 512 : (dc + 1) * 512] for dc in range(ND)]
                # DMA order: wqh + xTh first (Q projection starts earliest),
                # then wk/x-lo for K, then wv/xmean, then the f32 wqT (tail
                # only). 1024-col chunks = 2KB rows, spread across queues.
                nc.sync.dma_start(wcat[:, 0:1024], wcat_d[:, 0:1024])
                nc.sync.dma_start(wcat[:, 1024:2048], wcat_d[:, 1024:2048])
                for dc in range(ND):
                    sl = slice(dc * P, (dc + 1) * P)
                    nc.sync.dma_start(xTh[dc][:, 0:1024], xth_d[sl, 0:1024])
                    nc.sync.dma_start(xTh[dc][:, 1024:2048], xth_d[sl, 1024:2048])
                for c0 in range(2048, 4096, 1024):       # wkh (wkl unused)
                    nc.sync.dma_start(wcat[:, c0 : c0 + 1024], wcat_d[:, c0 : c0 + 1024])
                for dc in range(ND):
                    sl = slice(dc * P, (dc + 1) * P)
                    nc.sync.dma_start(xTl[dc][:, 0:1024], xtl_d[sl, 0:1024])
                    nc.sync.dma_start(xTl[dc][:, 1024:2048], xtl_d[sl, 1024:2048])
                for c0 in range(6144, 10240, 1024):      # wvh, wvl
                    nc.sync.dma_start(wcat[:, c0 : c0 + 1024], wcat_d[:, c0 : c0 + 1024])
                nc.sync.dma_start(wcat[:, 10240:10248], wcat_d[:, 10240:10248])
                nc.sync.dma_start(acat[:, 0:1024], acat_d[:, 0:1024])
                nc.sync.dma_start(acat[:, 1024:2048], acat_d[:, 1024:2048])

                # ---------------- phase 1: projections ----------------
                # Q first (single bf16 term, needs only wqh+xTh), jb-major;
                # then K 3-term jb-major so S blocks can start after K jb=0.
                for jb in range(NJ):
                    jsl = slice(jb * 512, (jb + 1) * 512)
                    for ic in range(ND):
                        isl = slice(ic * P, (ic + 1) * P)
                        pq = ps.tile([P, 512], f32, tag="blk")
                        for dc in range(ND):
                            nc.tensor.matmul(
                                pq[:], wqh[dc][:, isl], xTh[dc][:, jsl],
                                start=(dc == 0), stop=(dc == ND - 1),
                            )
                        nc.scalar.copy(QTb[ic][:, jsl], pq[:])
                # K approx: single bf16 term (the exact stage goes through
                # A = Wq^T Wk and never needs an exact K)
                for jb in range(NJ):
                    jsl = slice(jb * 512, (jb + 1) * 512)
                    for ic in range(ND):
                        isl = slice(ic * P, (ic + 1) * P)
                        pk = ps.tile([P, 512], f32, tag="blk")
                        for dc in range(ND):
                            nc.tensor.matmul(
                                pk[:], wkh[dc][:, isl], xTh[dc][:, jsl],
                                start=(dc == 0), stop=(dc == ND - 1),
                            )
                        nc.scalar.copy(KTb[ic][:, jsl], pk[:])

                # ---------------- phase 2: approx M (bf16 S) ----------------
                # per (lc, jb) block: ONE fused TTR (masked product -> bf16
                # scratch, fused max accum). The -sum_s/L term is omitted in
                # the approx M (absorbed into DELTA); the exact stage still
                # uses the full formula.
                M_all = mst.tile([P, 16], f32, tag="M_all")
                amax_all = mst.tile([P, NL * NJ * 8], f32, tag="amax_all")
                for lc in range(NL):
                    lsl = slice(lc * P, (lc + 1) * P)
                    mk = mstr.tile([P, L], bf16, tag="mk")
                    nc.sync.dma_start(mk[:], maskb_d[lsl, :])
                    for jb in range(NJ):
                        jsl = slice(jb * 512, (jb + 1) * 512)
                        k = lc * NJ + jb
                        pss = ps_s.tile([P, 512], f32, tag="psSc", name="pssa")
                        for ic in range(ND):
                            nc.tensor.matmul(
                                pss[:], QTb[ic][:, lsl], KTb[ic][:, jsl],
                                start=(ic == 0), stop=(ic == ND - 1),
                            )
                        # scalar engine drains PSUM to bf16 SBUF so the DVE
                        # mask-multiply runs in 2x packed mode (both operands
                        # bf16 SBUF); reduce_max is 1x regardless.
                        s0 = scr.tile([P, 512], bf16, tag="s0t")
                        nc.scalar.copy(s0[:], pss[:])
                        s1 = scr.tile([P, 512], bf16, tag="scrt")
                        nc.vector.tensor_tensor(
                            out=s1[:], in0=s0[:], in1=mk[:, jsl], op=OP.mult
                        )
                        nc.vector.max(
                            out=amax_all[:, k * 8 : (k + 1) * 8], in_=s1[:]
                        )

                # ---------------- phase 3: approx top-40 -> candidates ------
                # V projection (single bf16 term: the upd matmul consumes
                # bf16 anyway) is interleaved through the threshold/compaction
                # chain in chunks so PE stays busy (and HAM stays warm) while
                # the vector engine and GPSIMD work through the serial tail.
                def v_chunks(lo, hi):
                    for lc in range(lo, hi):
                        lsl = slice(lc * P, (lc + 1) * P)
                        pv = ps.tile([P, 512], f32, tag="blk")
                        for dc in range(ND):
                            nc.tensor.matmul(
                                pv[:], xTh[dc][:, lsl], wvh[dc][:],
                                start=(dc == 0), stop=(dc == ND - 1),
                            )
                        nc.scalar.copy(Vb[lc][:], pv[:])

                # V 0..7 first: PE covers the vector drain of the last S
                # blocks + the M_all combine without going HAM-cold
                v_chunks(0, 8)

                nc.vector.reduce_max(
                    M_all[:],
                    amax_all[:].rearrange("p (c j) -> p c j", j=NJ * 8),
                    axis=AX.X,
                )

                # exact T40 of M_approx without GPSIMD kth_largest (its attn
                # library reload cost ~50us of dead time on the serial tail):
                # per-chunk top-16 via vector max8/match_replace on M^T
                # [16,128], union (256 vals) holds the global top-40 w.p.
                # 1-2e-8, pack union into one [1,256] row via one-hot matmul
                # unwrap + transposes, then 5 rounds max8/match_replace.
                pmt = ps.tile([16, P], f32, tag="blk", name="pmt")
                nc.tensor.transpose(pmt[:16, :P], M_all[:], ident[:])
                MT = mst.tile([16, P], f32, tag="MT")
                nc.vector.tensor_copy(MT[:], pmt[:16, :P])
                w16 = mst.tile([16, 16], f32, tag="w16")
                nc.vector.max(out=w16[:, 0:8], in_=MT[:])
                nc.vector.match_replace(
                    out=MT[:], in_to_replace=w16[:, 0:8],
                    in_values=MT[:], imm_value=NEG,
                )
                nc.vector.max(out=w16[:, 8:16], in_=MT[:])

                v_chunks(8, 11)

                # unwrap w16 [16,16] -> two [128,1] columns (one-hot matmuls),
                # then -> [1,256] row via two PE transposes
                pcu = ps.tile([P, 2], f32, tag="blk", name="pcu")
                for f in range(8):
                    nc.tensor.matmul(
                        pcu[:P, 0:1], perm16[:, f * P : (f + 1) * P],
                        w16[:, f : f + 1],
                        start=(f == 0), stop=(f == 7),
                    )
                for f in range(8):
                    nc.tensor.matmul(
                        pcu[:P, 1:2], perm16[:, f * P : (f + 1) * P],
                        w16[:, 8 + f : 9 + f],
                        start=(f == 0), stop=(f == 7),
                    )
                crow = mst.tile([P, 2], f32, tag="crow")
                nc.vector.tensor_copy(crow[:], pcu[:P, :2])
                pr1 = ps.tile([1, P], f32, tag="blk", name="pr1")
                nc.tensor.transpose(pr1[:1, :P], crow[:, 0:1], ident[:])
                wrow = mst.tile([1, 2 * P], f32, tag="wrow")
                nc.vector.tensor_copy(wrow[:, 0:P], pr1[:1, :P])
                pr2 = ps.tile([1, P], f32, tag="blk", name="pr2")
                nc.tensor.transpose(pr2[:1, :P], crow[:, 1:2], ident[:])
                nc.vector.tensor_copy(wrow[:, P : 2 * P], pr2[:1, :P])

                v_chunks(11, 13)

                etop40 = mst.tile([1, NT], f32, tag="etop40")
                for r in range(5):
                    nc.vector.max(out=etop40[:, 8 * r : 8 * r + 8], in_=wrow[:])
                    if r < 4:
                        nc.vector.match_replace(
                            out=wrow[:], in_to_replace=etop40[:, 8 * r : 8 * r + 8],
                            in_values=wrow[:], imm_value=NEG,
                        )
                ptb = ps.tile([P, 1], f32, tag="blk")
                nc.tensor.matmul(
                    ptb[:P, :1], ones_r1[:], etop40[:, NT - 1 : NT],
                    start=True, stop=True,
                )
                tbc = mst.tile([P, 1], f32, tag="tbc")
                nc.vector.tensor_copy(tbc[:], ptb[:P, :1])

                # selmask = (M - T40) >= -DELTA, one fused op
                selmask = mst.tile([P, 16], u8, tag="selmask")
                nc.vector.tensor_scalar(
                    selmask[:], M_all[:], tbc[:], -DELTA,
                    op0=OP.subtract, op1=OP.is_ge,
                )
                midx = mst.tile([P, 16], f32, tag="midx")
                nc.vector.tensor_copy(midx[:], negone[:].to_broadcast([P, 16]))
                nc.vector.copy_predicated(midx[:], selmask[:], qidx_f[:])

                pwr = ps.tile([16, P], f32, tag="blk", name="pwr")
                nc.tensor.transpose(pwr[:16, :P], midx[:], ident[:])
                wrap_in = mst.tile([16, P], f32, tag="wrap_in")
                nc.vector.tensor_copy(wrap_in[:], pwr[:16, :P])
                spg = mst.tile([16, 8], f32, tag="spg")
                nfound = mst.tile([1, 1], u32, tag="nfound")
                nc.gpsimd.sparse_gather(out=spg[:], in_=wrap_in[:], num_found=nfound[:])

                v_chunks(13, 15)

                spg_cl = mst.tile([16, 8], f32, tag="spg_cl")
                nc.vector.tensor_scalar_max(spg_cl[:], spg[:], 0.0)
                nc.vector.tensor_scalar_min(spg_cl[:], spg_cl[:], float(L - 1))

                # unwrap [16,8] -> [128,1] with 8 tiny one-hot matmuls
                # (perm16[p, f*128+u] = 1 iff u == p + 16*f, shipped constant)
                pcq = ps.tile([P, 1], f32, tag="blk", name="pcq")
                for f in range(8):
                    nc.tensor.matmul(
                        pcq[:P, :1], perm16[:, f * P : (f + 1) * P],
                        spg_cl[:, f : f + 1],
                        start=(f == 0), stop=(f == 7),
                    )
                candq_f = mst.tile([P, 1], f32, tag="candq_f")
                nc.vector.tensor_copy(candq_f[:], pcq[:P, :1])
                candq_i = mst.tile([P, 1], i32, tag="candq_i")
                nc.vector.tensor_copy(candq_i[:], pcq[:P, :1])

                nf_f = mst.tile([1, 1], f32, tag="nf_f")
                nc.vector.tensor_copy(nf_f[:], nfound[:])
                pnb = ps.tile([P, 1], f32, tag="blk")
                nc.tensor.matmul(pnb[:P, :1], ones_r1[:], nf_f[:], start=True, stop=True)
                nbc = mst.tile([P, 1], f32, tag="nbc")
                nc.vector.tensor_copy(nbc[:], pnb[:P, :1])
                invalid = mst.tile([P, 1], u8, tag="invalid")
                nc.vector.tensor_tensor(
                    out=invalid[:], in0=qidx_f[:, 0:1], in1=nbc[:], op=OP.is_ge
                )

                v_chunks(15, 16)

                # Vmean = xmean.T @ Wv.T via bf16x2 3-term, broadcast, ctx init
                pvm = ps.tile([1, 512], f32, tag="blk")
                n = 0
                for dc in range(ND):
                    for lh, rh in (
                        (xmh[dc][:], wvh[dc][:]),
                        (xml[dc][:], wvh[dc][:]),
                        (xmh[dc][:], wvl[dc][:]),
                    ):
                        nc.tensor.matmul(
                            pvm[:1, :], lh, rh,
                            start=(n == 0), stop=(n == 3 * ND - 1),
                        )
                        n += 1
                vmean = mst.tile([1, 512], f32, tag="vmean")
                nc.scalar.copy(vmean[:], pvm[:1, :])
                pvb = ps.tile([P, 512], f32, tag="blk")
                nc.tensor.matmul(pvb[:], ones_r1[:], vmean[:], start=True, stop=True)
                vmean_bc = mst.tile([P, 512], f32, tag="vmean_bc")
                nc.vector.tensor_copy(vmean_bc[:], pvb[:])
                for jc in range(NL):
                    nc.sync.dma_start(ctx_d[jc * P : (jc + 1) * P, :], vmean_bc[:])

                # ---------------- phase 4a: exact candidates ----------------
                x_cand = cnd.tile([P, D], f32, tag="x_cand")
                nc.gpsimd.indirect_dma_start(
                    out=x_cand[:], out_offset=None, in_=x_d[:],
                    in_offset=bass.IndirectOffsetOnAxis(ap=candq_i[:, :1], axis=0),
                )
                xcT = [cnd.tile([P, P], f32, tag=f"xcT{dc}", name=f"xcT{dc}") for dc in range(ND)]
                for dc in range(ND):
                    pxc = ps.tile([P, P], f32, tag="blk")
                    nc.tensor.transpose(
                        pxc[:P, :P], x_cand[:, dc * P : (dc + 1) * P], ident[:]
                    )
                    nc.vector.tensor_copy(xcT[dc][:], pxc[:P, :P])

                # Y^T = (x_cand @ A)^T in f32 via PE, then bf16 hi/lo split
                # for the 3-term S_cand product against xTh/xTl
                YTh = [cnd.tile([P, P], bf16, tag=f"YTh{ic}", name=f"YTh{ic}") for ic in range(ND)]
                YTl = [cnd.tile([P, P], bf16, tag=f"YTl{ic}", name=f"YTl{ic}") for ic in range(ND)]
                for ic in range(ND):
                    isl = slice(ic * P, (ic + 1) * P)
                    pqc = ps.tile([P, P], f32, tag="blk")
                    for dc in range(ND):
                        nc.tensor.matmul(
                            pqc[:P, :P], Arc[dc][:, isl], xcT[dc][:],
                            start=(dc == 0), stop=(dc == ND - 1),
                        )
                    nc.vector.tensor_copy(YTh[ic][:], pqc[:P, :P])
                    nc.vector.tensor_tensor(
                        out=YTl[ic][:], in0=pqc[:P, :P], in1=YTh[ic][:],
                        op=OP.subtract,
                    )

                gm = cnd.tile([P, L], u8, tag="gm")
                nc.gpsimd.indirect_dma_start(
                    out=gm[:], out_offset=None, in_=mask_d[:],
                    in_offset=bass.IndirectOffsetOnAxis(ap=candq_i[:, :1], axis=0),
                )
                gc = cnd.tile([P, L], u8, tag="gc")
                nc.gpsimd.indirect_dma_start(
                    out=gc[:], out_offset=None, in_=cnt_d[:],
                    in_offset=bass.IndirectOffsetOnAxis(ap=candq_i[:, :1], axis=0),
                )

                psS = []
                cmax = cnd.tile([P, NJ], f32, tag="cmax")
                csum = cnd.tile([P, NJ], f32, tag="csum")
                for jb in range(NJ):
                    jsl = slice(jb * 512, (jb + 1) * 512)
                    pss2 = ps_s.tile([P, 512], f32, tag="psSc")
                    psS.append(pss2)
                    n = 0
                    for ic in range(ND):
                        for lh, rh in (
                            (YTh[ic][:], xTh[ic][:, jsl]),
                            (YTl[ic][:], xTh[ic][:, jsl]),
                            (YTh[ic][:], xTl[ic][:, jsl]),
                        ):
                            nc.tensor.matmul(
                                pss2[:], lh, rh,
                                start=(n == 0), stop=(n == 3 * ND - 1),
                            )
                            n += 1
                    s3 = scr.tile([P, 512], f32, tag="scrt")
                    nc.vector.tensor_tensor(
                        out=s3[:], in0=pss2[:], in1=gm[:, jsl], op=OP.mult
                    )
                    nc.vector.reduce_max(cmax[:, jb : jb + 1], s3[:], axis=AX.X)
                    s4 = scr.tile([P, 512], f32, tag="scrt")
                    nc.vector.scalar_tensor_tensor(
                        out=s4[:], in0=pss2[:], scalar=-1.0 / L, in1=gc[:, jsl],
                        op0=OP.mult, op1=OP.mult,
                        accum_out=csum[:, jb : jb + 1],
                    )
                u1 = cnd.tile([P, 1], f32, tag="u1")
                u2 = cnd.tile([P, 1], f32, tag="u2")
                M_cand = cnd.tile([P, 1], f32, tag="M_cand")
                nc.vector.reduce_max(u1[:], cmax[:], axis=AX.X)
                nc.vector.reduce_sum(u2[:], csum[:], axis=AX.X)
                nc.vector.tensor_tensor(out=M_cand[:], in0=u1[:], in1=u2[:], op=OP.add)
                nc.vector.copy_predicated(M_cand[:], invalid[:], negbig[:])

            # xTh/xTl/weights freed here
            with tc.tile_pool(name="expp", bufs=1) as expp:
                # ---------------- phase 4b: softmax + update ----------------
                # no max-subtraction: scores*SCALE is O(10), exp is fp32-safe
                exp_sb = expp.tile([P, L], f32, tag="exp_sb")
                sume4 = expp.tile([P, NJ], f32, tag="sume4")
                for jb in range(NJ):
                    jsl = slice(jb * 512, (jb + 1) * 512)
                    nc.scalar.activation(
                        out=exp_sb[:, jsl], in_=psS[jb][:], func=ACTF.Exp,
                        bias=0.0, scale=SCALE,
                        accum_out=sume4[:, jb : jb + 1],
                    )
                sume = expp.tile([P, 1], f32, tag="sume")
                nc.vector.reduce_sum(sume[:], sume4[:], axis=AX.X)
                recip = expp.tile([P, 1], f32, tag="recip")
                nc.vector.reciprocal(recip[:], sume[:])

                # kick off the candidate top-40 rounds on the vector engine
                # first (they gate the scatter), then run the expT/upd PE
                # pipeline concurrently with them
                pmc = ps.tile([1, P], f32, tag="blk")
                nc.tensor.transpose(pmc[:1, :P], M_cand[:], ident[:])
                mcT = cnd.tile([1, P], f32, tag="mcT")
                nc.vector.tensor_copy(mcT[:], pmc[:1, :P])
                etop = cnd.tile([1, NT], f32, tag="etop")
                for r in range(5):
                    nc.vector.max(out=etop[:, 8 * r : 8 * r + 8], in_=mcT[:])
                    if r < 4:
                        nc.vector.match_replace(
                            out=mcT[:], in_to_replace=etop[:, 8 * r : 8 * r + 8],
                            in_values=mcT[:], imm_value=NEG,
                        )

                # expT transposes and upd accumulation pipelined: the upd
                # matmul for chunk jc-1 issues right after transpose jc. The
                # PSUM->bf16 drains run on the scalar engine so the vector
                # engine is free for the candidate top-40 rounds above.
                expT = [expp.tile([P, P], bf16, tag=f"expT{jc}", name=f"expT{jc}") for jc in range(NL)]
                pu = ps.tile([P, 512], f32, tag="blk", name="pu")
                for jc in range(NL):
                    pet = ps.tile([P, P], f32, tag="blk")
                    nc.tensor.transpose(
                        pet[:P, :P], exp_sb[:, jc * P : (jc + 1) * P], ident[:]
                    )
                    nc.scalar.copy(expT[jc][:], pet[:P, :P])
                    if jc >= 1:
                        nc.tensor.matmul(
                            pu[:], expT[jc - 1][:], Vb[jc - 1][:],
                            start=(jc == 1), stop=False,
                        )
                nc.tensor.matmul(
                    pu[:], expT[NL - 1][:], Vb[NL - 1][:],
                    start=False, stop=True,
                )

                # threshold broadcast + selection (vector rounds already done)
                pte = ps.tile([P, 1], f32, tag="blk")
                nc.tensor.matmul(
                    pte[:P, :1], ones_r1[:], etop[:, NT - 1 : NT], start=True, stop=True
                )
                tebc = cnd.tile([P, 1], f32, tag="tebc")
                nc.vector.tensor_copy(tebc[:], pte[:P, :1])
                sel2 = cnd.tile([P, 1], u8, tag="sel2")
                nc.vector.tensor_tensor(
                    out=sel2[:], in0=M_cand[:], in1=tebc[:], op=OP.is_ge
                )
                scat_f = cnd.tile([P, 1], f32, tag="scat_f")
                nc.vector.tensor_copy(scat_f[:], big9[:])
                nc.vector.copy_predicated(scat_f[:], sel2[:], candq_f[:])
                scat_i = cnd.tile([P, 1], i32, tag="scat_i")
                nc.vector.tensor_copy(scat_i[:], scat_f[:])

                upd = expp.tile([P, D], f32, tag="upd")
                nc.scalar.activation(
                    out=upd[:], in_=pu[:], func=ACTF.Copy, bias=0.0, scale=recip[:]
                )
                nc.gpsimd.indirect_dma_start(
                    out=ctx_d[:],
                    out_offset=bass.IndirectOffsetOnAxis(ap=scat_i[:, :1], axis=0),
                    in_=upd[:], in_offset=None,
                    bounds_check=L - 1, oob_is_err=False,
                )

    nc.compile()
    return nc


_NC = None


def _get_nc():
    global _NC
    if _NC is None:
        _NC = build()
    return _NC


def _split_bf16(a):
    hi = a.astype(ml_dtypes.bfloat16)
    lo = (a - hi.astype(np.float32)).astype(ml_dtypes.bfloat16)
    return hi, lo


def _row_chunk(w):
    # [512, 512] -> [128, 4*512]: tile dc = rows dc*128..dc*128+127
    return np.concatenate([w[dc * P : (dc + 1) * P, :] for dc in range(4)], axis=1)


def _host_prep(x, Wq, Wk, Wv, index_sample):
    x = np.asarray(x, dtype=np.float32)
    Wq = np.asarray(Wq, dtype=np.float32)
    Wk = np.asarray(Wk, dtype=np.float32)
    Wv = np.asarray(Wv, dtype=np.float32)
    idx = np.asarray(index_sample)

    wqT = np.ascontiguousarray(Wq.T)
    wqh, _ = _split_bf16(wqT)
    wkh, wkl = _split_bf16(np.ascontiguousarray(Wk.T))
    wvh, wvl = _split_bf16(np.ascontiguousarray(Wv.T))
    A = (Wq.T.astype(np.float64) @ Wk.astype(np.float64)).astype(np.float32)

    rows = np.arange(L)[:, None]
    mask01 = np.zeros((L, L), dtype=np.uint8)
    mask01[rows, idx] = 1
    maskb = mask01.astype(ml_dtypes.bfloat16)
    countf = np.zeros((L, L), dtype=np.uint8)
    np.add.at(countf, (rows, idx), 1)

    perm16 = np.zeros((16, 8 * P), dtype=np.float32)
    for f in range(8):
        for p in range(16):
            perm16[p, f * P + p + 16 * f] = 1.0
    identf = np.eye(P, dtype=np.float32)
    qidxf = (np.arange(P, dtype=np.float32)[:, None]
             + 128.0 * np.arange(16, dtype=np.float32)[None, :])
    acat = np.ascontiguousarray(_row_chunk(A))
    shared = {
        "Acat": acat, "mask01": mask01, "maskb": maskb, "countf": countf,
        "perm16": perm16, "identf": identf, "qidxf": qidxf,
    }
    wpart = np.concatenate(
        [_row_chunk(wqh), _row_chunk(wkh), _row_chunk(wkl),
         _row_chunk(wvh), _row_chunk(wvl)],
        axis=1,
    )
    in_maps = []
    for b in range(B):
        xb = np.ascontiguousarray(x[b])
        xT = np.ascontiguousarray(xb.T)
        xth, xtl = _split_bf16(xT)
        xmean = (xb.astype(np.float64).mean(axis=0) / 1.0).astype(np.float32)
        xmeh, xmel = _split_bf16(xmean.reshape(D, 1))
        wcat = np.concatenate(
            [wpart, xmeh.reshape(4, P).T, xmel.reshape(4, P).T], axis=1
        ).astype(ml_dtypes.bfloat16)
        in_maps.append(
            {
                "x_nat": xb,
                "xTh": xth,
                "xTl": xtl,
                "wcat": np.ascontiguousarray(wcat),
                **shared,
            }
        )
    return in_maps


def kernel(x, Wq, Wk, Wv, index_sample, _trace=False, _result_box=None):
    in_maps = _host_prep(x, Wq, Wk, Wv, index_sample)
    nc = _get_nc()
    res = run_bass_kernel_spmd(nc, in_maps, core_ids=list(range(B)), trace=_trace)
    if _result_box is not None:
        _result_box.append(res)
    out = np.stack([np.asarray(res.results[b]["ctx"]) for b in range(B)], axis=0)
    return out

